# revision 16
# baseline (speedup 1.0000x reference)
"""Trainium2 Bass kernel for nn_MedicalVisionTransformer (MoE-LoRA ViT).

Strategy: data-parallel over batch (8 cores x 8 batch items). Each core holds
its 256-token (8 batches x 32 slots: 30 real + 2 pad) residual stream in SBUF
for all 12 layers; only weights stream from HBM in fp16. MoE LoRA experts are
collapsed algebraically (rank-8 C matrices; per-expert LayerNorm folded into
per-token scalars via B_down Gram matrices) so no [B,S,E,H]/[B,S,E,D] tensor
is ever materialized.

v2: weights stream as a few large per-layer DMAs (SP sequencer / HWDGE were
instruction-count bound at ~100 DMAs/layer); per-partition biases (qk, FFN-up)
are folded into Activation-engine biased copies instead of K=1 matmuls; the
Exp/Gelu activation-table switches are hoisted off the critical path with
dummy ops.
"""

import sys

sys.path.insert(0, "/opt/trn_rl_repo")

import numpy as np

import concourse.bass as bass
import concourse.mybir as mybir
import concourse.tile as tile
from concourse import bacc
from concourse import bass_utils

f32 = np.float32
F32 = mybir.dt.float32
F16 = mybir.dt.float16
F32R = mybir.dt.float32r
I32 = mybir.dt.int32

B, SR, D, H, L, NH, ND, E, RK = 64, 29, 768, 3072, 12, 12, 14, 15, 8
S = SR + 1
SCALE = f32(16.0 / 8.0)
NE = L // 2
DH = D // NH
NC = 8
BPC = B // NC          # batches per core
TS = 32                # token slot per batch (30 real + 2 pad)
NT = BPC * TS          # 256 tokens per core
D2 = D // 2            # 384
ER = E * RK            # 120
KC = D // 128          # 6 feature chunks
HC = H // 128          # 24 hidden chunks

AluOp = mybir.AluOpType
Act = mybir.ActivationFunctionType

_PHASES = []   # (label, first_instruction_id) markers for profiling

# ---- packed weight block column offsets (f16 cols) ----
WQK_COLS = 9216      # 18 blocks of 512  (g*6+c)
WVO_V = 0            # 6 blocks of 768
WVO_AO = 4608        # 6 blocks of 768
WVO_COLS = 9216
WI_COLS = 9216       # 18 blocks of 512 per half ((g%3)*6+c)
WO_COLS = 9216       # 12 blocks of 768 per half
BQK = 0              # [128,12] per-chunk qk bias
BI = 12              # [128,24] per-chunk FFN-up bias
BV = 40              # row-0 strips
BAO = 808
BO = 1576
B_COLS = 2344
# moe pack offsets
AU_O, AD_O, BDD_O = 0, 720, 3600
CB_O, GR_O, BDM_O, BDF_O = 4320, 4440, 4560, 4575
CLS_O, CB2_O, LUP_O = 5343, 6879, 6880
M1_COLS = 7000
# const pack (f16) offsets
SEL2_O, RM_O, SELJ_O, SELB_O = 0, 128, 142, 150
ATTLT_O, ATTRT_O, MPOOL_O, IND2_O = 374, 502, 2038, 2150
SEG_O, REP_O = 2406, 2421
MASKS_O, FG_O, FB_O = 2560, 2574, 3342
ATTM2_O = 4110
ATTM2T_O = 4240
CPK_COLS = 4500


# ----------------------------------------------------------------------------
# Host-side weight preparation (pure numpy; done once per kernel() call)
# ----------------------------------------------------------------------------

def _prep(inputs):
    P = {}
    qs = f32(1.0 / np.sqrt(DH))

    WQK = np.zeros((L, 128, WQK_COLS), np.float16)
    WVO = np.zeros((L, 128, WVO_COLS), np.float16)
    WI1 = np.zeros((L, 128, WI_COLS), np.float16)
    WI2 = np.zeros((L, 128, WI_COLS), np.float16)
    WO1 = np.zeros((L, 128, WO_COLS), np.float16)
    WO2 = np.zeros((L, 128, WO_COLS), np.float16)
    BIA = np.zeros((L, 128, B_COLS), np.float16)

    for i in range(L):
        g1, b1 = inputs['ln1_g'][i], inputs['ln1_b'][i]
        g2, b2 = inputs['ln2_g'][i], inputs['ln2_b'][i]
        WqT = (inputs['Wq'][i] * g1[None, :]).T * qs      # [in, out]
        WkT = (inputs['Wk'][i] * g1[None, :]).T
        bq = (b1 @ inputs['Wq'][i].T + inputs['bq'][i]) * qs
        bk = b1 @ inputs['Wk'][i].T + inputs['bk'][i]
        qk = np.concatenate([WqT, WkT], axis=1)           # [768, 1536]
        for g in range(3):
            for c in range(KC):
                WQK[i, :, (g * 6 + c) * 512:(g * 6 + c + 1) * 512] = \
                    qk[c * 128:(c + 1) * 128,
                       g * 512:(g + 1) * 512].astype(np.float16)
        bqk_full = np.concatenate([bq, bk]).astype(np.float16)   # [1536]
        BIA[i, :, BQK:BQK + 12] = bqk_full.reshape(12, 128).T
        WvT = (inputs['Wv'][i] * g1[None, :]).T
        WaoT = inputs['Wao'][i].T
        for c in range(KC):
            WVO[i, :, WVO_V + c * 768:WVO_V + (c + 1) * 768] = \
                WvT[c * 128:(c + 1) * 128].astype(np.float16)
            WVO[i, :, WVO_AO + c * 768:WVO_AO + (c + 1) * 768] = \
                WaoT[c * 128:(c + 1) * 128].astype(np.float16)
        BIA[i, 0, BV:BV + D] = (b1 @ inputs['Wv'][i].T
                                + inputs['bv'][i]).astype(np.float16)
        BIA[i, 0, BAO:BAO + D] = inputs['bao'][i].astype(np.float16)
        WiT = (inputs['Wi'][i] * g2[None, :]).T           # [768, 3072]
        for g in range(6):
            dst = WI1 if g < 3 else WI2
            gg = g % 3
            for c in range(KC):
                dst[i, :, (gg * 6 + c) * 512:(gg * 6 + c + 1) * 512] = \
                    WiT[c * 128:(c + 1) * 128,
                        g * 512:(g + 1) * 512].astype(np.float16)
        bi_full = (b2 @ inputs['Wi'][i].T + inputs['bi'][i]).astype(np.float16)
        BIA[i, :, BI:BI + 24] = bi_full.reshape(24, 128).T
        WoT = inputs['Wo'][i].T                            # [3072, 768]
        for c in range(HC):
            dst = WO1 if c < 12 else WO2
            cc = c % 12
            dst[i, :, cc * 768:(cc + 1) * 768] = \
                WoT[c * 128:(c + 1) * 128].astype(np.float16)
        BIA[i, 0, BO:BO + D] = inputs['bo'][i].astype(np.float16)

    P.update(WQK=WQK, WVO=WVO, WI1=WI1, WI2=WI2, WO1=WO1, WO2=WO2, BIA=BIA)

    # MoE / classifier packed tensors
    MOE = np.zeros((NE, 128, M1_COLS), np.float16)
    CW1 = np.zeros((NE, 7, 128, 2 * KC * D2), np.float16)

    for e in range(NE):
        i = 2 * e
        g2, b2 = inputs['ln2_g'][i], inputs['ln2_b'][i]
        Au = inputs['A_up'][e]; Bu = inputs['B_up'][e]
        Ad = inputs['A_down'][e]; Bd = inputs['B_down'][e]
        AuTf = np.concatenate([(Au[ee] * g2[None, :]).T for ee in range(E)], axis=1)
        for c in range(KC):
            MOE[e, :, AU_O + c * ER:AU_O + (c + 1) * ER] = \
                AuTf[c * 128:(c + 1) * 128].astype(np.float16)
        MOE[e, 0, LUP_O:LUP_O + ER] = np.concatenate(
            [b2 @ Au[ee].T for ee in range(E)]).astype(np.float16)
        AdTf = np.concatenate([Ad[ee].T for ee in range(E)], axis=1)   # [H, 120]
        for c in range(HC):
            MOE[e, :, AD_O + c * ER:AD_O + (c + 1) * ER] = \
                AdTf[c * 128:(c + 1) * 128].astype(np.float16)
        for ee in range(E):
            Cm = Ad[ee] @ Bu[ee]                                        # [r, r']
            MOE[e, ee * RK:(ee + 1) * RK,
                CB_O + ee * RK:CB_O + (ee + 1) * RK] = \
                (SCALE * Cm.T).astype(np.float16)
        Bdf = np.concatenate([Bd[ee].T for ee in range(E)], axis=0) * SCALE  # [120, D]
        MOE[e, :ER, BDF_O:BDF_O + D] = Bdf.astype(np.float16)
        BdfDf = (2.0 * Bdf.T / f32(D))                                  # [D, 120]
        for c in range(KC):
            MOE[e, :, BDD_O + c * ER:BDD_O + (c + 1) * ER] = \
                BdfDf[c * 128:(c + 1) * 128].astype(np.float16)
        Bdm = Bdf.mean(axis=1)                                          # [120]
        for ee in range(E):
            MOE[e, ee * RK:(ee + 1) * RK, BDM_O + ee] = \
                Bdm[ee * RK:(ee + 1) * RK].astype(np.float16)
            sl = slice(ee * RK, (ee + 1) * RK)
            MOE[e, ee * RK:(ee + 1) * RK, GR_O + ee * RK:GR_O + (ee + 1) * RK] = \
                ((Bdf[sl] @ Bdf[sl].T) / f32(D)).astype(np.float16)
        # classifier weights; cW1 in 7 blocks of 2 diseases
        cW1e = inputs['cW1'][e]
        for d in range(ND):
            W1T = cW1e[d].T                                             # [768, 384]
            dd, dh = d // 2, d % 2
            for c in range(KC):
                CW1[e, dd, :, (dh * KC + c) * D2:(dh * KC + c + 1) * D2] = \
                    W1T[c * 128:(c + 1) * 128].astype(np.float16)
        # clsPack rows (d*8+j): g|b|w2|bias, then cb2 col
        for d in range(ND):
            for j in range(8):
                r = d * 8 + j
                MOE[e, r, CLS_O + 0:CLS_O + D2] = inputs['clng'][e][d]
                MOE[e, r, CLS_O + D2:CLS_O + 2 * D2] = inputs['clnb'][e][d]
                MOE[e, r, CLS_O + 2 * D2:CLS_O + 3 * D2] = inputs['cW2'][e][d]
                MOE[e, r, CLS_O + 3 * D2:CLS_O + 4 * D2] = inputs['cb1'][e][d]
                MOE[e, r, CB2_O] = inputs['cb2'][e][d]

    P.update(MOE=MOE, CW1=CW1)

    # ---- constant packs ----
    CPK = np.zeros((128, CPK_COLS), np.float16)

    CPK[0, SEL2_O:SEL2_O + 64] = 1.0
    CPK[1, SEL2_O + 64:SEL2_O + 128] = 1.0
    for d in range(ND):
        for j in range(8):
            CPK[d * 8 + j, RM_O + d] = 1.0
            CPK[d * 8 + j, SELJ_O + j] = 1.0
    for j in range(8):
        CPK[j, SELB_O + 104 + j] = 1.0

    ks = np.arange(128)
    CPK[0, ATTLT_O:ATTLT_O + 128] = 1.0
    qreal = (ks % TS < S).astype(np.float16)
    for h in range(NH):
        CPK[0, ATTRT_O + h * 128:ATTRT_O + (h + 1) * 128] = -30000.0 * qreal
    for j in range(4):
        CPK[1 + j, ATTLT_O:ATTLT_O + 128] = \
            ((ks // TS == j) & (ks % TS < S)).astype(np.float16)
        blk = ((ks // TS == j) & (ks % TS < S)).astype(np.float16) * 30000.0
        for h in range(NH):
            CPK[1 + j, ATTRT_O + h * 128:ATTRT_O + (h + 1) * 128] = blk

    mask = inputs['mask']; cnt = mask.sum(axis=0)
    for tg in range(2):
        for d in range(ND):
            for bl in range(4):
                col = d * 4 + bl
                CPK[bl * TS + 1: bl * TS + 1 + SR,
                    MPOOL_O + tg * 56 + col] = (mask[:, d] / cnt[d]).astype(np.float16)
    for tg in range(2):
        for bl in range(4):
            j = tg * 4 + bl
            CPK[j, IND2_O + tg * 128 + bl * TS:IND2_O + tg * 128 + (bl + 1) * TS] = 1.0
    for ee in range(E):
        CPK[ee * RK:(ee + 1) * RK, SEG_O + ee] = 1.0
        CPK[ee, REP_O + ee * RK:REP_O + (ee + 1) * RK] = 1.0

    qq, kk = np.meshgrid(np.arange(128), np.arange(128), indexing='ij')
    CPK[:, ATTM2_O:ATTM2_O + 128] = (((qq // TS) == (kk // TS))
                                     & ((kk % TS) < S)).astype(np.float16)
    m2t = (((qq // TS) == (kk // TS)) & ((qq % TS) < S)).astype(np.float16)
    CPK[:, ATTM2T_O:ATTM2T_O + 128] = m2t
    CPK[:, ATTM2T_O + 128:ATTM2T_O + 256] = m2t

    maskS = np.zeros((TS, ND), f32)
    maskS[1:1 + SR] = mask
    CPK[:, MASKS_O:MASKS_O + ND] = np.tile(maskS, (4, 1)).astype(np.float16)
    CPK[:, FG_O:FG_O + D] = np.tile(inputs['fg'][None, :],
                                    (128, 1)).astype(np.float16)
    CPK[:, FB_O:FB_O + D] = np.tile(inputs['fb'][None, :],
                                    (128, 1)).astype(np.float16)

    P.update(CPK=CPK)
    return P


def _shard_x0(inputs):
    """Per-core [128, 2, 768] initial residual streams (token = tg*128+p)."""
    cls = np.asarray(inputs['cls_token'][0, 0], f32)
    rf = np.asarray(inputs['region_features'], f32)
    shards = []
    for c in range(NC):
        x0 = np.zeros((NT, D), f32)
        for bl in range(BPC):
            b = c * BPC + bl
            x0[bl * TS] = cls
            x0[bl * TS + 1: bl * TS + 1 + SR] = rf[b]
        shards.append(np.ascontiguousarray(
            x0.reshape(2, 128, D).transpose(1, 0, 2)))
    return shards


# ----------------------------------------------------------------------------
# Bass/Tile program
# ----------------------------------------------------------------------------

def _build(sim_gelu=False):
    nc = bacc.Bacc("TRN2", target_bir_lowering=False, debug=False)
    _PHASES.clear()

    def _mark(label):
        _PHASES.append((label, nc.next_id()))

    def din(name, shape, dt):
        return nc.dram_tensor(name, list(shape), dt, kind="ExternalInput")

    t_x0 = din("x0", (128, 2, D), F32)
    t_WQK = din("WQK", (L, 128, WQK_COLS), F16)
    t_WVO = din("WVO", (L, 128, WVO_COLS), F16)
    t_WI1 = din("WI1", (L, 128, WI_COLS), F16)
    t_WI2 = din("WI2", (L, 128, WI_COLS), F16)
    t_WO1 = din("WO1", (L, 128, WO_COLS), F16)
    t_WO2 = din("WO2", (L, 128, WO_COLS), F16)
    t_BIA = din("BIA", (L, 128, B_COLS), F16)
    t_MOE = din("MOE", (NE, 128, M1_COLS), F16)
    t_CW1 = din("CW1", (NE, 7, 128, 2 * KC * D2), F16)
    t_CPK = din("CPK", (128, CPK_COLS), F16)
    t_out = nc.dram_tensor("out", [128, 2, D], F32, kind="ExternalOutput")

    with tile.TileContext(nc) as tc:
        with (
            tc.tile_pool(name="const", bufs=1) as cpool,
            tc.tile_pool(name="resid", bufs=1) as hpool,
            tc.tile_pool(name="wstream", bufs=3) as wpool,
            tc.tile_pool(name="wbias", bufs=2) as bpool,
            tc.tile_pool(name="wmoe", bufs=1) as wmpool,
            tc.tile_pool(name="wcls", bufs=2) as wcpool,
            tc.tile_pool(name="acts", bufs=1) as apool,
            tc.tile_pool(name="scrA", bufs=3) as sapool,
            tc.tile_pool(name="scrB", bufs=1) as spool,
            tc.tile_pool(name="small", bufs=1) as mpool,
            tc.tile_pool(name="psG", bufs=2, space="PSUM") as psG,
            tc.tile_pool(name="psY", bufs=3, space="PSUM") as psY,
        ):
            dma = nc.sync.dma_start

            def act_gelu(dst, src, bias=None):
                if not sim_gelu:
                    if bias is None:
                        nc.scalar.activation(dst, src, Act.Gelu)
                    else:
                        nc.scalar.activation(dst, src, Act.Gelu, bias=bias,
                                             scale=1.0)
                    return
                shp = list(dst.shape)
                y = sapool.tile(shp, F32, tag="gel_y", name="gel_y")
                if bias is None:
                    nc.scalar.activation(y[:], src, Act.Identity)
                else:
                    nc.scalar.activation(y[:], src, Act.Identity, bias=bias,
                                         scale=1.0)
                u = sapool.tile(shp, F32, tag="gel_u", name="gel_u")
                nc.vector.tensor_tensor(out=u[:], in0=y[:], in1=y[:],
                                        op=AluOp.mult)
                nc.vector.tensor_tensor(out=u[:], in0=u[:], in1=y[:],
                                        op=AluOp.mult)
                nc.vector.tensor_scalar(out=u[:], in0=u[:], scalar1=0.044715,
                                        scalar2=None, op0=AluOp.mult)
                nc.vector.tensor_tensor(out=u[:], in0=u[:], in1=y[:],
                                        op=AluOp.add)
                nc.scalar.activation(u[:], u[:], Act.Tanh, scale=0.7978845608)
                nc.vector.tensor_scalar(out=u[:], in0=u[:], scalar1=1.0,
                                        scalar2=0.5, op0=AluOp.add,
                                        op1=AluOp.mult)
                nc.vector.tensor_tensor(out=dst, in0=u[:], in1=y[:],
                                        op=AluOp.mult)

            # ---------------- constants ----------------
            ident = cpool.tile([128, 128], F16)
            from concourse.masks import make_identity
            make_identity(nc, ident[:])
            ident32 = cpool.tile([128, 128], F32)
            make_identity(nc, ident32[:])
            onesc = cpool.tile([1, 512], F16)   # K=1 matmul lhsT/rhs ones
            nc.vector.memset(onesc[:], 1.0)
            ones_k = cpool.tile([128, 1], F16)  # column-sum matmul rhs
            nc.vector.memset(ones_k[:], 1.0)
            ones15 = cpool.tile([E, 1], F16)
            nc.vector.memset(ones15[:], 1.0)
            cpk = cpool.tile([128, CPK_COLS], F16)
            dma(cpk[:], t_CPK[:])
            c_sel2 = cpk[0:2, SEL2_O:SEL2_O + 128]
            c_Rm = cpk[0:112, RM_O:RM_O + ND].rearrange(
                "p (a b) -> p a b", a=1)
            c_SelJ = cpk[0:112, SELJ_O:SELJ_O + 8]
            c_selB = cpk[0:8, SELB_O:SELB_O + 224]
            c_attLT = cpk[0:5, ATTLT_O:ATTLT_O + 128]
            c_attRT = cpk[0:5, ATTRT_O:ATTRT_O + NH * 128]
            c_Mpool = cpk[:, MPOOL_O:MPOOL_O + 112].rearrange(
                "p (t m) -> p t m", t=2)
            c_Ind2 = cpk[0:8, IND2_O:IND2_O + 256].rearrange(
                "p (t m) -> p t m", t=2)
            c_SegSel0 = cpk[0:ER, SEG_O:SEG_O + E]
            c_RepSel = cpk[0:E, REP_O:REP_O + ER]
            c_maskS = cpk[:, MASKS_O:MASKS_O + ND]
            c_M2 = cpk[:, ATTM2_O:ATTM2_O + 128].rearrange(
                "p (a b) -> p a b", a=1)
            c_M2T2 = cpk[:, ATTM2T_O:ATTM2T_O + 256].rearrange(
                "p (a b) -> p a b", a=2)
            c_fgB = cpk[:, FG_O:FG_O + D]
            c_fbB = cpk[:, FB_O:FB_O + D]
            magic_t = cpool.tile([128, 256], I32)
            nc.vector.memset(magic_t[:], 0x5f3759df)
            dumact = cpool.tile([1, 2], F32)
            nc.vector.memset(dumact[:], 0.0)

            def rsqrt_dve(dst, var_ap, eps):
                """dst = 1/sqrt(var_ap + eps), DVE-only (bit hack + 2 Newton)."""
                shp = list(dst.shape)
                p, n = shp[0], int(np.prod(shp[1:]))
                v = mpool.tile(shp, F32, tag="rsqv", bufs=2, name="rsqv")
                nc.vector.tensor_scalar(out=v[:], in0=var_ap, scalar1=float(eps),
                                        scalar2=None, op0=AluOp.add)
                ti_ = mpool.tile(shp, I32, tag="rsqt", bufs=2, name="rsqt")
                nc.vector.tensor_scalar(out=ti_[:], in0=v[:].bitcast(I32),
                                        scalar1=1, scalar2=None,
                                        op0=AluOp.logical_shift_right)
                mg = magic_t[:p].rearrange("p n -> p n")[:, :n]
                nc.vector.tensor_tensor(out=ti_[:], in0=mg.rearrange(
                    "p (a b) -> p a b", a=1) if len(shp) == 3 else mg,
                    in1=ti_[:], op=AluOp.subtract)
                y = ti_[:].bitcast(F32)
                a_ = mpool.tile(shp, F32, tag="rsqa", bufs=2, name="rsqa")
                for it in range(2):
                    nc.vector.tensor_tensor(out=a_[:], in0=v[:], in1=y,
                                            op=AluOp.mult)
                    nc.vector.tensor_tensor(out=a_[:], in0=a_[:], in1=y,
                                            op=AluOp.mult)
                    nc.vector.tensor_scalar(out=a_[:], in0=a_[:], scalar1=-0.5,
                                            scalar2=1.5, op0=AluOp.mult,
                                            op1=AluOp.add)
                    nc.vector.tensor_tensor(out=dst if it == 1 else
                                            ti_[:].bitcast(F32),
                                            in0=y, in1=a_[:], op=AluOp.mult)

            # ---------------- persistent activations ----------------
            h = hpool.tile([128, 2, D], F32)
            res1 = hpool.tile([128, 2, D], F32)
            dma(h[:], t_x0[:])

            def ln_stats_tg(src, mvs, tg):
                st = mpool.tile([128, 2, 6], F32, tag="lnst", bufs=2)
                xs = src[:, tg, :].rearrange("p (a b) -> p a b", a=2)
                for a in range(2):
                    nc.vector.bn_stats(st[:, a, :], xs[:, a, :])
                nc.vector.bn_aggr(mvs[:, tg, :], st[:])

            def ln_stats(src, eps):
                """emit stats+rsqrt for both tgs; returns (mvs, rst2)."""
                mvs = mpool.tile([128, 2, 2], F32, tag="lnmv", bufs=2)
                for tg in range(2):
                    ln_stats_tg(src, mvs, tg)
                rst2 = mpool.tile([128, 2], F32, tag="lnrs", bufs=2)
                rsqrt_dve(rst2[:], mvs[:, :, 1], eps)
                return mvs, rst2

            def ln_apply(src, dst, stats, per_tg=None):
                mvs, rst2 = stats
                for tg in range(2):
                    nc.vector.tensor_scalar(
                        out=dst[:, tg, :], in0=src[:, tg, :],
                        scalar1=mvs[:, tg, 0:1], scalar2=rst2[:, tg:tg + 1],
                        op0=AluOp.subtract, op1=AluOp.mult)
                    if per_tg is not None:
                        per_tg(tg)

            def layernorm_16(src, dst, eps, per_tg=None):
                ln_apply(src, dst, ln_stats(src, eps), per_tg)

            def transpose6_tg(src16, dst, tg):
                for c in range(KC):
                    pt = psY.tile([128, 128], F16, tag="sm")
                    nc.tensor.transpose(pt[:], src16[:, tg, c * 128:(c + 1) * 128],
                                        ident[:])
                    nc.scalar.activation(dst[:, c, tg * 128:(tg + 1) * 128],
                                         pt[:], Act.Copy)

            def transpose6(src16, dst):
                """src16 [128, 2, D] f16 -> dst [128, KC, 256] f16 (feature-major)."""
                for c in range(KC):
                    for tg in range(2):
                        pt = psY.tile([128, 128], F16, tag="sm")
                        nc.tensor.transpose(pt[:], src16[:, tg, c * 128:(c + 1) * 128],
                                            ident[:])
                        nc.scalar.activation(dst[:, c, tg * 128:(tg + 1) * 128],
                                             pt[:], Act.Copy)

            # ---------------- layers ----------------
            h_stats = ln_stats(h, 1e-12)
            for i in range(L):
                even = (i % 2 == 0)
                e = i // 2
                _mark(f'L{i}.dma')
                # ---- all weight DMAs for the layer, in consumption order ----
                b_t = bpool.tile([128, B_COLS], F16, tag="bias")
                dma(b_t[:], t_BIA[i, :, :])
                w_qk = wpool.tile([128, WQK_COLS], F16, tag="w", name="w_qk")
                dma(w_qk[:], t_WQK[i, :, :])
                w_vo = wpool.tile([128, WVO_COLS], F16, tag="w", name="w_vo")
                dma(w_vo[:], t_WVO[i, :, :])
                w_i1 = wpool.tile([128, WI_COLS], F16, tag="w", name="w_i1")
                dma(w_i1[:], t_WI1[i, :, :])
                w_i2 = wpool.tile([128, WI_COLS], F16, tag="w", name="w_i2")
                dma(w_i2[:], t_WI2[i, :, :])
                w_o1 = wpool.tile([128, WO_COLS], F16, tag="w", name="w_o1")
                dma(w_o1[:], t_WO1[i, :, :])
                w_o2 = wpool.tile([128, WO_COLS], F16, tag="w", name="w_o2")
                dma(w_o2[:], t_WO2[i, :, :])

                _mark(f'L{i}.ln1')
                # LN1 -> n1 (f16) -> n1T (stats precomputed at end of prev layer)
                n1 = apool.tile([128, 2, D], F16, tag="n1")
                n1T = apool.tile([128, KC, 256], F16, tag="n1T")
                ln_apply(h, n1, h_stats,
                         per_tg=lambda tg: transpose6_tg(n1, n1T, tg))
                # hoist Exp act-table load off the attention critical path
                if not sim_gelu:
                    nc.scalar.activation(dumact[:, 0:1], dumact[:, 0:1],
                                         Act.Exp)

                _mark(f'L{i}.qk')
                # QK^T (transposed out; bias via Act-Identity biased drains)
                qkT = apool.tile([128, 12, 256], F16, tag="bigact")
                for g in range(3):
                    grps = [psG.tile([128, 2, 512], F32, tag="grp",
                                     name=f"qkg{g}{hf}") for hf in range(2)]
                    for c in range(KC):
                        for j in range(4):
                            nc.tensor.matmul(grps[j // 2][:, j % 2, :256],
                                             w_qk[:, (g * 6 + c) * 512 + j * 128:
                                                  (g * 6 + c) * 512 + (j + 1) * 128],
                                             n1T[:, c, :],
                                             start=(c == 0), stop=(c == KC - 1))
                    for hf in range(2):
                        for sub in range(2):
                            fc = g * 4 + 2 * hf + sub
                            nc.scalar.activation(
                                qkT[:, fc, :], grps[hf][:, sub, :256],
                                Act.Identity, bias=b_t[:, BQK + fc:BQK + fc + 1],
                                scale=1.0)

                _mark(f'L{i}.v')
                # V (untransposed: [tok, dv]); bias via K=1 matmul
                V = apool.tile([128, 2, D], F16, tag="V")
                grpv = [psG.tile([128, 2, 512], F32, tag="grp",
                                 name=f"vg{tg}") for tg in range(2)]
                for j in range(4):
                    sl = slice((j % 2) * 384, (j % 2 + 1) * 384)
                    nc.tensor.matmul(grpv[j // 2][:, j % 2, :384],
                                     onesc[:1, :128],
                                     b_t[0:1, BV + sl.start:BV + sl.stop],
                                     start=True, stop=False)
                for c in range(KC):
                    for j in range(4):
                        tg, n = j // 2, j % 2
                        sl = slice(n * 384, (n + 1) * 384)
                        nc.tensor.matmul(grpv[tg][:, n, :384],
                                         n1T[:, c, tg * 128:(tg + 1) * 128],
                                         w_vo[:, WVO_V + c * 768 + sl.start:
                                              WVO_V + c * 768 + sl.stop],
                                         start=False, stop=(c == KC - 1))
                for tg in range(2):
                    nc.scalar.activation(
                        V[:, tg, :].rearrange("p (n d) -> p n d", n=2),
                        grpv[tg][:, :, :384], Act.Copy)

                _mark(f'L{i}.attn')
                # attention — scoresT[k, q] layout (keys restricted to own tg),
                # no transposes: V [tok, dv] is directly the o-matmul lhsT.
                oT = apool.tile([128, KC, 256], F16, tag="oT")
                for tg in range(2):
                    tgs = slice(tg * 128, (tg + 1) * 128)
                    expT = sapool.tile([128, NH, 128], F16, tag="expT")
                    for fc in range(KC):
                        pssc = psY.tile([128, 256], F32, tag="sm")
                        for pp in range(2):
                            hd = 2 * fc + pp
                            off = pp * 64
                            ps_h = slice(pp * 128, (pp + 1) * 128)
                            nc.tensor.matmul(pssc[:, ps_h], c_attLT[:],
                                             c_attRT[:, hd * 128:(hd + 1) * 128],
                                             start=True, stop=False)
                            nc.tensor.matmul(pssc[:, ps_h],
                                             qkT[off:off + 64, 6 + fc, tgs],
                                             qkT[off:off + 64, fc, tgs],
                                             start=False, stop=True)
                        nc.scalar.activation(
                            expT[:, 2 * fc:2 * fc + 2, :], pssc[:], Act.Exp)
                    # per-(q, head) softmax denominators via PE column sums
                    ps_rs = psY.tile([128, NH], F32, tag="sm")
                    for hd in range(NH):
                        nc.tensor.matmul(ps_rs[:, hd:hd + 1],
                                         expT[:, hd, :],
                                         ones_k[:], start=True, stop=True)
                    rinv = mpool.tile([128, NH], F32, tag="rinv", bufs=2)
                    nc.vector.reciprocal(rinv[:], ps_rs[:])
                    rinv16 = mpool.tile([128, NH], F16, tag="rinv16", bufs=2)
                    nc.scalar.activation(rinv16[:], rinv[:], Act.Copy)
                    for c in range(KC):
                        # psn[p, q] = rinv16[q, 2c + p//64] via stride-0 lhsT
                        psn = psY.tile([128, 128], F32, tag="sm")
                        for hh in range(2):
                            lhsT = rinv16[:, 2 * c + hh:2 * c + hh + 1] \
                                .to_broadcast((128, 1, 64))[:, 0, :]
                            nc.tensor.matmul(psn[hh * 64:(hh + 1) * 64, :],
                                             lhsT, ident[:],
                                             start=True, stop=True)
                        psnS = sapool.tile([128, 128], F16, tag="psnS")
                        nc.scalar.activation(psnS[:], psn[:], Act.Copy)
                        pso = psY.tile([128, 128], F32, tag="sm")
                        for hh in range(2):
                            hd = 2 * c + hh
                            nc.tensor.matmul(pso[hh * 64:(hh + 1) * 64, :],
                                             V[:, tg, hd * 64:(hd + 1) * 64],
                                             expT[:, hd, :],
                                             start=True, stop=True)
                        nc.vector.tensor_tensor(out=oT[:, c, tgs],
                                                in0=pso[:], in1=psnS[:],
                                                op=AluOp.mult)
                # hoist Gelu act-table load off the FFN critical path
                if not sim_gelu:
                    nc.scalar.activation(dumact[:, 1:2], dumact[:, 1:2],
                                         Act.Gelu)

                _mark(f'L{i}.ao')
                # AO projection + residual
                if even:
                    attnH = apool.tile([128, 2, D], F16, tag="n1")
                grpa = [psG.tile([128, 2, 512], F32, tag="grp",
                                 name=f"aog{tg}") for tg in range(2)]
                for j in range(4):
                    sl = slice((j % 2) * 384, (j % 2 + 1) * 384)
                    nc.tensor.matmul(grpa[j // 2][:, j % 2, :384],
                                     onesc[:1, :128],
                                     b_t[0:1, BAO + sl.start:BAO + sl.stop],
                                     start=True, stop=False)
                for c in range(KC):
                    for j in range(4):
                        tg, n = j // 2, j % 2
                        sl = slice(n * 384, (n + 1) * 384)
                        nc.tensor.matmul(grpa[tg][:, n, :384],
                                         oT[:, c, tg * 128:(tg + 1) * 128],
                                         w_vo[:, WVO_AO + c * 768 + sl.start:
                                              WVO_AO + c * 768 + sl.stop],
                                         start=False, stop=(c == KC - 1))
                mvs2 = mpool.tile([128, 2, 2], F32, tag="lnmv", bufs=2)
                for tg in range(2):
                    if even:
                        nc.scalar.activation(
                            attnH[:, tg, :].rearrange("p (n d) -> p n d", n=2),
                            grpa[tg][:, :, :384], Act.Copy)
                    nc.vector.tensor_tensor(
                        out=res1[:, tg, :].rearrange("p (n d) -> p n d", n=2),
                        in0=grpa[tg][:, :, :384],
                        in1=h[:, tg, :].rearrange("p (n d) -> p n d", n=2),
                        op=AluOp.add)
                    ln_stats_tg(res1, mvs2, tg)
                rst2b = mpool.tile([128, 2], F32, tag="lnrs", bufs=2)
                rsqrt_dve(rst2b[:], mvs2[:, :, 1], 1e-12)

                if even:
                    _mark(f'L{i}.moe_pool')
                    # pooled^T [128, KC, 112] (cols (d, tg*4+bl) after scatter)
                    pooledT = apool.tile([128, KC, 112], F16, tag="pooledT")
                    pview = pooledT.rearrange("p c (d g) -> p c d g", g=8)
                    for c in range(KC):
                        for tg in range(2):
                            ps = psY.tile([128, 4 * ND], F32, tag="sm")
                            nc.tensor.matmul(ps[:],
                                             attnH[:, tg, c * 128:(c + 1) * 128],
                                             c_Mpool[:, tg, :],
                                             start=True, stop=True)
                            pv = ps[:].rearrange("p (d g) -> p d g", g=4)
                            nc.vector.tensor_copy(
                                pview[:, c, :, tg * 4:tg * 4 + 4], pv)
                    # single merged MoE pack DMA (weights for the whole tail)
                    moepk = wmpool.tile([128, M1_COLS], F16, tag="moepk")
                    dma(moepk[:], t_MOE[e, :, :])
                    c_cg = moepk[0:112, CLS_O + 0:CLS_O + D2]
                    c_cbt = moepk[0:112, CLS_O + D2:CLS_O + 2 * D2]
                    c_w2 = moepk[0:112, CLS_O + 2 * D2:CLS_O + 3 * D2]
                    c_b1 = moepk[0:112, CLS_O + 3 * D2:CLS_O + 4 * D2]
                    c_c2 = moepk[0:112, CB2_O:CB2_O + 1]
                    w_au = moepk[:, AU_O:AU_O + KC * ER].rearrange(
                        "p (c r) -> p c r", r=ER)
                    b_lup = moepk[0:1, LUP_O:LUP_O + ER]
                    w_ad = moepk[:, AD_O:AD_O + HC * ER].rearrange(
                        "p (c r) -> p c r", r=ER)
                    w_cb = moepk[0:ER, CB_O:CB_O + ER]
                    w_bdd = moepk[:, BDD_O:BDD_O + KC * ER].rearrange(
                        "p (c r) -> p c r", r=ER)
                    w_gram = moepk[0:ER, GR_O:GR_O + ER]
                    w_bdm = moepk[0:ER, BDM_O:BDM_O + E]
                    w_bdf = moepk[0:ER, BDF_O:BDF_O + D]
                    psz = psY.tile([112, D2], F32, tag="zacc", bufs=1)
                    cls_state = {}

                    def emit_cls_dd(dd):
                        w_ch = wcpool.tile([128, 2, KC, D2], F16, tag="wc1",
                                           name="wc1_c")
                        dma(w_ch[:], t_CW1[e, dd, :, :].rearrange(
                            "p (h c z) -> p h c z", h=2, c=KC))
                        for dh in range(2):
                            d = 2 * dd + dh
                            psd_ = psY.tile([8, D2], F32, tag="sm")
                            for c in range(KC):
                                nc.tensor.matmul(psd_[:],
                                                 pooledT[:, c, d * 8:(d + 1) * 8],
                                                 w_ch[:, dh, c, :],
                                                 start=(c == 0), stop=(c == KC - 1))
                            zd = sapool.tile([8, D2], F16, tag="zd")
                            nc.scalar.activation(zd[:], psd_[:], Act.Copy)
                            nc.tensor.matmul(psz[:],
                                             c_selB[:, 104 - 8 * d:216 - 8 * d],
                                             zd[:], start=(d == 0),
                                             stop=(d == ND - 1))

                def emit_cls_finish():
                    zsb = spool.tile([112, D2], F32, tag="zsb")
                    nc.vector.tensor_tensor(out=zsb[:], in0=psz[:], in1=c_b1[:],
                                            op=AluOp.add)
                    zst = mpool.tile([112, 6], F32, tag="lnstz")
                    nc.vector.bn_stats(zst[:], zsb[:])
                    zmv = mpool.tile([112, 2], F32, tag="lnmvz")
                    nc.vector.bn_aggr(zmv[:], zst[:])
                    zrstd = mpool.tile([112, 1], F32, tag="zrstd")
                    rsqrt_dve(zrstd[:], zmv[:, 1:2], 1e-5)
                    zn = spool.tile([112, D2], F32, tag="zn")
                    nc.vector.tensor_scalar(out=zn[:], in0=zsb[:],
                                            scalar1=zmv[:, 0:1], scalar2=zrstd[:],
                                            op0=AluOp.subtract, op1=AluOp.mult)
                    nc.vector.tensor_tensor(out=zn[:], in0=zn[:], in1=c_cg[:],
                                            op=AluOp.mult)
                    nc.vector.tensor_tensor(out=zn[:], in0=zn[:], in1=c_cbt[:],
                                            op=AluOp.add)
                    zg = spool.tile([112, D2], F32, tag="zg")
                    act_gelu(zg[:], zn[:])
                    nc.vector.tensor_tensor(out=zg[:], in0=zg[:], in1=c_w2[:],
                                            op=AluOp.mult)
                    ppre = mpool.tile([112, 1], F32, tag="ppre")
                    nc.vector.reduce_sum(ppre[:], zg[:], axis=mybir.AxisListType.X)
                    nc.vector.tensor_tensor(out=ppre[:], in0=ppre[:], in1=c_c2[:],
                                            op=AluOp.add)
                    rp = mpool.tile([112, 1, ND], F16, tag="rp")
                    nc.vector.tensor_tensor(out=rp[:], in0=c_Rm[:],
                                            in1=ppre[:].to_broadcast((112, 1, ND)),
                                            op=AluOp.mult)
                    psda = psY.tile([8, ND], F32, tag="sm")
                    nc.tensor.matmul(psda[:], c_SelJ[:], rp[:, 0, :],
                                     start=True, stop=True)
                    da = mpool.tile([8, ND], F16, tag="da")
                    nc.vector.tensor_scalar(out=da[:], in0=psda[:], scalar1=0.0,
                                            scalar2=None, op0=AluOp.is_gt)

                    _mark(f'L{i}.moe_rout')
                    # routing weights w [128, tg, E] f32
                    w_rt = spool.tile([128, 2, E], F32, tag="wrt")
                    nact = mpool.tile([128, 2], F32, tag="nact")
                    for tg in range(2):
                        psd = psY.tile([128, ND], F32, tag="sm")
                        nc.tensor.matmul(psd[:], c_Ind2[:, tg, :], da[:],
                                         start=True, stop=True)
                        nc.vector.tensor_tensor(out=w_rt[:, tg, 0:ND], in0=psd[:],
                                                in1=c_maskS[:], op=AluOp.mult)
                        nc.vector.reduce_sum(nact[:, tg:tg + 1], w_rt[:, tg, 0:ND],
                                             axis=mybir.AxisListType.X)
                        nc.vector.tensor_scalar(out=nact[:, tg:tg + 1],
                                                in0=nact[:, tg:tg + 1],
                                                scalar1=1.0, scalar2=None,
                                                op0=AluOp.add)
                    rnact = mpool.tile([128, 2], F32, tag="rnact")
                    nc.vector.reciprocal(rnact[:], nact[:])
                    for tg in range(2):
                        nc.vector.tensor_scalar(out=w_rt[:, tg, 0:ND],
                                                in0=w_rt[:, tg, 0:ND],
                                                scalar1=rnact[:, tg:tg + 1],
                                                scalar2=None, op0=AluOp.mult)
                        nc.vector.tensor_copy(w_rt[:, tg, ND:E], rnact[:, tg:tg + 1])
                    wT = mpool.tile([E, 256], F32, tag="wT")
                    for tg in range(2):
                        pt = psY.tile([E, 128], F32, tag="sm")
                        nc.tensor.transpose(pt[:], w_rt[:, tg, :], ident32[:])
                        nc.vector.tensor_copy(wT[:, tg * 128:(tg + 1) * 128], pt[:])
                    cls_state['wT'] = wT

                _mark(f'L{i}.ln2')
                # LN2 -> n2 -> n2T (stats computed during AO drains)
                n2 = apool.tile([128, 2, D], F16, tag="n2")
                n2T = apool.tile([128, KC, 256], F16, tag="n2T")
                ln_apply(res1, n2, (mvs2, rst2b),
                         per_tg=lambda tg: transpose6_tg(n2, n2T, tg))

                _mark(f'L{i}.up')
                # FFN up (transposed out) + gelu with folded bias
                interT = apool.tile([128, HC, 256], F16, tag="bigact")
                for g in range(6):
                    w_i = w_i1 if g < 3 else w_i2
                    gg = g % 3
                    grps = [psG.tile([128, 2, 512], F32, tag="grp",
                                     name=f"upg{g}{hf}") for hf in range(2)]
                    for c in range(KC):
                        for j in range(4):
                            nc.tensor.matmul(grps[j // 2][:, j % 2, :256],
                                             w_i[:, (gg * 6 + c) * 512 + j * 128:
                                                 (gg * 6 + c) * 512 + (j + 1) * 128],
                                             n2T[:, c, :],
                                             start=(c == 0), stop=(c == KC - 1))
                    for hf in range(2):
                        for sub in range(2):
                            hc = g * 4 + 2 * hf + sub
                            act_gelu(interT[:, hc, :], grps[hf][:, sub, :256],
                                     bias=b_t[:, BI + hc:BI + hc + 1])
                    if even:
                        _mark(f'L{i}.moe_cls')
                        if g < 5:
                            emit_cls_dd(g)
                        else:
                            emit_cls_dd(5)
                            emit_cls_dd(6)
                        _mark(f'L{i}.up')

                if even:
                    _mark(f'L{i}.moe_lora')
                    # LoRA rails
                    ps = psY.tile([ER, 256], F32, tag="sm")
                    nc.tensor.matmul(ps[:], b_lup, onesc[:1, :256],
                                     start=True, stop=False)
                    for c in range(KC):
                        nc.tensor.matmul(ps[:], w_au[:, c, :], n2T[:, c, :],
                                         start=False, stop=(c == KC - 1))
                    lup_rT = spool.tile([ER, 256], F16, tag="luprT")
                    nc.vector.tensor_copy(lup_rT[:], ps[:])

                    ps2 = psY.tile([ER, 256], F32, tag="sm")
                    nc.tensor.matmul(ps2[:], w_cb, lup_rT[:], start=True, stop=False)
                    for c in range(HC):
                        nc.tensor.matmul(ps2[:], w_ad[:, c, :], interT[:, c, :],
                                         start=False, stop=(c == HC - 1))
                    ldr16 = spool.tile([ER, 256], F16, tag="ldr16")
                    nc.vector.tensor_copy(ldr16[:], ps2[:])
                    ldr32 = spool.tile([ER, 256], F32, tag="ldr32")
                    nc.vector.tensor_copy(ldr32[:], ps2[:])

                _mark(f'L{i}.down')
                # FFN down
                if even:
                    base = apool.tile([128, 2, D], F16, tag="base")
                grpd = [psG.tile([128, 2, 512], F32, tag="grp",
                                 name=f"dng{tg}") for tg in range(2)]
                for j in range(4):
                    sl = slice((j % 2) * 384, (j % 2 + 1) * 384)
                    nc.tensor.matmul(grpd[j // 2][:, j % 2, :384],
                                     onesc[:1, :128],
                                     b_t[0:1, BO + sl.start:BO + sl.stop],
                                     start=True, stop=False)
                for c in range(HC):
                    w_o = w_o1 if c < 12 else w_o2
                    cc = c % 12
                    for j in range(4):
                        tg, n = j // 2, j % 2
                        sl = slice(n * 384, (n + 1) * 384)
                        nc.tensor.matmul(grpd[tg][:, n, :384],
                                         interT[:, c, tg * 128:(tg + 1) * 128],
                                         w_o[:, cc * 768 + sl.start:
                                             cc * 768 + sl.stop],
                                         start=False, stop=(c == HC - 1))
                if even:
                    _mark(f'L{i}.moe_cls2')
                    emit_cls_finish()
                for tg in range(2):
                    if not even:
                        nc.vector.tensor_tensor(
                            out=h[:, tg, :].rearrange("p (n d) -> p n d", n=2),
                            in0=grpd[tg][:, :, :384],
                            in1=res1[:, tg, :].rearrange("p (n d) -> p n d", n=2),
                            op=AluOp.add)
                    else:
                        nc.vector.tensor_copy(
                            base[:, tg, :].rearrange("p (n d) -> p n d", n=2),
                            grpd[tg][:, :, :384])

                if not even:
                    h_stats = ln_stats(h, 1e-12)
                    continue

                _mark(f'L{i}.moe_stats')
                # ================= MoE / classifier tail =================
                # base stats (mu, ms = var + mu^2), transposed to rows
                mums = mpool.tile([128, 2, 2], F32, tag="mums")   # [:, tg, (mu,ms)]
                for tg in range(2):
                    st = mpool.tile([128, 3, 6], F32, tag="lnst", bufs=2)
                    xs = base[:, tg, :].rearrange("p (a b) -> p a b", a=3)
                    for a in range(3):
                        nc.vector.bn_stats(st[:, a, :], xs[:, a, :])
                    mv = mpool.tile([128, 2], F32, tag="lnmv", bufs=2)
                    nc.vector.bn_aggr(mv[:], st[:])
                    nc.vector.tensor_copy(mums[:, tg, 0:1], mv[:, 0:1])
                    # ms = var + mu^2
                    musq = mpool.tile([128, 1], F32, tag="musq")
                    nc.vector.tensor_tensor(out=musq[:], in0=mv[:, 0:1],
                                            in1=mv[:, 0:1], op=AluOp.mult)
                    nc.vector.tensor_tensor(out=mums[:, tg, 1:2], in0=mv[:, 1:2],
                                            in1=musq[:], op=AluOp.add)
                muT = mpool.tile([1, 256], F32, tag="muT")
                msT = mpool.tile([1, 256], F32, tag="msT")
                for tg in range(2):
                    pt = psY.tile([1, 128], F32, tag="sm")
                    nc.tensor.transpose(pt[:], mums[:, tg, 0:1], ident32[:])
                    nc.vector.tensor_copy(muT[:, tg * 128:(tg + 1) * 128], pt[:])
                    pt2 = psY.tile([1, 128], F32, tag="sm")
                    nc.tensor.transpose(pt2[:], mums[:, tg, 1:2], ident32[:])
                    nc.vector.tensor_copy(msT[:, tg * 128:(tg + 1) * 128], pt2[:])

                baseT = apool.tile([128, KC, 256], F16, tag="n1T")
                transpose6(base, baseT)

                # (cls finish + routing emitted during down via emit_cls_finish)
                _mark(f'L{i}.moe_g')
                # G^T (cross term, x2 folded in BdfD) and quad term
                psg = psY.tile([ER, 256], F32, tag="sm")
                for c in range(KC):
                    nc.tensor.matmul(psg[:], w_bdd[:, c, :], baseT[:, c, :],
                                     start=(c == 0), stop=(c == KC - 1))
                Pcross = spool.tile([ER, 256], F16, tag="pcross")
                nc.vector.tensor_tensor(out=Pcross[:], in0=psg[:],
                                        in1=ldr32[:], op=AluOp.mult)

                psq = psY.tile([ER, 256], F32, tag="sm")
                nc.tensor.matmul(psq[:], w_gram, ldr16[:],
                                 start=True, stop=True)
                Pquad = spool.tile([ER, 256], F16, tag="pquad")
                nc.vector.tensor_tensor(out=Pquad[:], in0=psq[:], in1=ldr32[:],
                                        op=AluOp.mult)

                # mu_e^T [E, 256]
                muT16 = mpool.tile([1, 256], F16, tag="muT16")
                nc.vector.tensor_copy(muT16[:], muT[:])
                msT16 = mpool.tile([1, 256], F16, tag="msT16")
                nc.vector.tensor_copy(msT16[:], msT[:])
                psmu = psY.tile([E, 256], F32, tag="sm")
                nc.tensor.matmul(psmu[:], w_bdm, ldr16[:], start=True, stop=False)
                nc.tensor.matmul(psmu[:], onesc[:1, :E], muT16[:],
                                 start=False, stop=True)
                muE = mpool.tile([E, 256], F32, tag="muE")
                nc.vector.tensor_copy(muE[:], psmu[:])

                # ms^T then var, rho
                psms = psY.tile([E, 256], F32, tag="sm")
                nc.tensor.matmul(psms[:], c_SegSel0, Pcross[:],
                                 start=True, stop=False)
                nc.tensor.matmul(psms[:], c_SegSel0, Pquad[:],
                                 start=False, stop=False)
                nc.tensor.matmul(psms[:], onesc[:1, :E], msT16[:],
                                 start=False, stop=True)
                musqE = mpool.tile([E, 256], F32, tag="musqE")
                nc.vector.tensor_tensor(out=musqE[:], in0=muE[:], in1=muE[:],
                                        op=AluOp.mult)
                varE = mpool.tile([E, 256], F32, tag="varE")
                nc.vector.tensor_tensor(out=varE[:], in0=psms[:], in1=musqE[:],
                                        op=AluOp.subtract)
                rho = mpool.tile([E, 256], F32, tag="rho")
                rsqrt_dve(rho[:], varE[:], 1e-5)

                # s_e = w * rho ; pack [sE | sE*muE] -> column sums -> scal/off
                packSO = mpool.tile([E, 512], F16, tag="packSO")
                wT = cls_state['wT']
                nc.vector.tensor_tensor(out=packSO[:, 0:256], in0=wT[:], in1=rho[:],
                                        op=AluOp.mult)
                nc.vector.tensor_tensor(out=packSO[:, 256:512],
                                        in0=packSO[:, 0:256], in1=muE[:],
                                        op=AluOp.mult)
                psso = psY.tile([1, 512], F32, tag="sm")
                nc.tensor.matmul(psso[:], ones15[:], packSO[:],
                                 start=True, stop=True)
                soT = mpool.tile([1, 512], F32, tag="soT")
                nc.vector.tensor_copy(soT[:], psso[:])
                scal = mpool.tile([128, 2], F32, tag="scal")
                off = mpool.tile([128, 2], F32, tag="off")
                for tg in range(2):
                    pt = psY.tile([128, 1], F32, tag="sm")
                    nc.tensor.transpose(pt[:], soT[:, tg * 128:(tg + 1) * 128],
                                        ident32[:1, :1])
                    nc.vector.tensor_copy(scal[:, tg:tg + 1], pt[:])
                    pt2 = psY.tile([128, 1], F32, tag="sm")
                    nc.tensor.transpose(pt2[:],
                                        soT[:, 256 + tg * 128:256 + (tg + 1) * 128],
                                        ident32[:1, :1])
                    nc.vector.tensor_copy(off[:, tg:tg + 1], pt2[:])

                # ls^T = ldown_r^T * repeat(s_e)
                psrep = psY.tile([ER, 256], F32, tag="sm")
                nc.tensor.matmul(psrep[:], c_RepSel, packSO[:, 0:256],
                                 start=True, stop=True)
                srep = mpool.tile([ER, 256], F32, tag="srep")
                nc.vector.tensor_copy(srep[:], psrep[:])
                lsT = spool.tile([ER, 256], F16, tag="lsT")
                nc.vector.tensor_tensor(out=lsT[:], in0=srep[:], in1=ldr32[:],
                                        op=AluOp.mult)

                _mark(f'L{i}.moe_fin')
                # final: h = (res1 - off) + (base*scal + ldown_mix)
                grpf = [psG.tile([128, 2, 512], F32, tag="grp",
                                 name=f"fing{tg}") for tg in range(2)]
                for j in range(4):
                    tg, n = j // 2, j % 2
                    sl = slice(n * 384, (n + 1) * 384)
                    nc.tensor.matmul(grpf[tg][:, n, :384],
                                     lsT[:, tg * 128:(tg + 1) * 128],
                                     w_bdf[:, sl], start=True, stop=True)
                for j in range(4):
                    tg, n = j // 2, j % 2
                    sl = slice(n * 384, (n + 1) * 384)
                    tmp = spool.tile([128, 384], F32, tag="ffn_tmp")
                    nc.vector.scalar_tensor_tensor(
                        out=tmp[:], in0=base[:, tg, sl],
                        scalar=scal[:, tg:tg + 1],
                        in1=grpf[tg][:, n, :384], op0=AluOp.mult, op1=AluOp.add)
                    nc.vector.scalar_tensor_tensor(
                        out=h[:, tg, sl], in0=res1[:, tg, sl],
                        scalar=off[:, tg:tg + 1], in1=tmp[:],
                        op0=AluOp.subtract, op1=AluOp.add)
                h_stats = ln_stats(h, 1e-12)

            _mark('final_ln')
            # ---------------- final LN ----------------
            hf = apool.tile([128, 2, D], F32, tag="base")
            ln_apply(h, hf, h_stats)   # writes f32 since tile dtype f32
            ot = apool.tile([128, 2, D], F32, tag="bigact",
                            name="ot")
            for tg in range(2):
                nc.vector.tensor_tensor(out=ot[:, tg, :], in0=hf[:, tg, :],
                                        in1=c_fgB, op=AluOp.mult)
                nc.vector.tensor_tensor(out=ot[:, tg, :], in0=ot[:, tg, :],
                                        in1=c_fbB, op=AluOp.add)
            dma(t_out[:], ot[:])

    nc.compile()
    return nc


_CACHE = {}


def _get_nc(sim_gelu=False):
    key = ("nc", sim_gelu)
    if key not in _CACHE:
        _CACHE[key] = _build(sim_gelu)
    return _CACHE[key]


def kernel(**inputs):
    inputs = {k: np.asarray(v) for k, v in inputs.items()}
    P = _prep(inputs)
    shards = _shard_x0(inputs)
    nc = _get_nc()
    base_map = {k: np.ascontiguousarray(v) for k, v in P.items()}
    in_maps = []
    for c in range(NC):
        m = dict(base_map)
        m["x0"] = np.ascontiguousarray(shards[c])
        in_maps.append(m)
    res = bass_utils.run_bass_kernel_spmd(nc, in_maps, core_ids=list(range(NC)))
    out = np.zeros((B, S, D), f32)
    for c in range(NC):
        oc = res.results[c]["out"].transpose(1, 0, 2).reshape(NT, D)
        for bl in range(BPC):
            out[c * BPC + bl] = oc[bl * TS: bl * TS + S]
    return out


# revision 17
# speedup vs baseline: 1.0408x; 1.0408x over previous
"""Trainium2 Bass kernel for nn_MedicalVisionTransformer (MoE-LoRA ViT).

Strategy: data-parallel over batch (8 cores x 8 batch items). Each core holds
its 256-token (8 batches x 32 slots: 30 real + 2 pad) residual stream in SBUF
for all 12 layers; only weights stream from HBM in fp16. MoE LoRA experts are
collapsed algebraically (rank-8 C matrices; per-expert LayerNorm folded into
per-token scalars via B_down Gram matrices) so no [B,S,E,H]/[B,S,E,D] tensor
is ever materialized.

v2: weights stream as a few large per-layer DMAs (SP sequencer / HWDGE were
instruction-count bound at ~100 DMAs/layer); per-partition biases (qk, FFN-up)
are folded into Activation-engine biased copies instead of K=1 matmuls; the
Exp/Gelu activation-table switches are hoisted off the critical path with
dummy ops.
"""

import sys

sys.path.insert(0, "/opt/trn_rl_repo")

import numpy as np

import concourse.bass as bass
import concourse.mybir as mybir
import concourse.tile as tile
from concourse import bacc
from concourse import bass_utils

f32 = np.float32
F32 = mybir.dt.float32
F16 = mybir.dt.float16
F32R = mybir.dt.float32r
I32 = mybir.dt.int32

B, SR, D, H, L, NH, ND, E, RK = 64, 29, 768, 3072, 12, 12, 14, 15, 8
S = SR + 1
SCALE = f32(16.0 / 8.0)
NE = L // 2
DH = D // NH
NC = 8
BPC = B // NC          # batches per core
TS = 32                # token slot per batch (30 real + 2 pad)
NT = BPC * TS          # 256 tokens per core
D2 = D // 2            # 384
ER = E * RK            # 120
KC = D // 128          # 6 feature chunks
HC = H // 128          # 24 hidden chunks

AluOp = mybir.AluOpType
Act = mybir.ActivationFunctionType

_PHASES = []   # (label, first_instruction_id) markers for profiling

# ---- packed weight block column offsets (f16 cols) ----
WQK_COLS = 9216      # 18 blocks of 512  (g*6+c)
WVO_V = 0            # 6 blocks of 768
WVO_AO = 4608        # 6 blocks of 768
WVO_COLS = 9216
WI_COLS = 9216       # 18 blocks of 512 per half ((g%3)*6+c)
WO_COLS = 9216       # 12 blocks of 768 per half
BQK = 0              # [128,12] per-chunk qk bias
BI = 12              # [128,24] per-chunk FFN-up bias
BV = 40              # row-0 strips
BAO = 808
BO = 1576
B_COLS = 2344
# moe pack offsets
AU_O, AD_O, BDD_O = 0, 720, 3600
CB_O, GR_O, BDM_O, BDF_O = 4320, 4440, 4560, 4575
CLS_O, CB2_O, LUP_O = 5343, 6879, 6880
M1_COLS = 7000
# const pack (f16) offsets
SEL2_O, RM_O, SELJ_O, SELB_O = 0, 128, 142, 150
ATTLT_O, ATTRT_O, MPOOL_O, IND2_O = 374, 502, 2038, 2150
SEG_O, REP_O = 2406, 2421
MASKS_O, FG_O, FB_O = 2560, 2574, 3342
ATTM2_O = 4110
ATTM2T_O = 4240
CPK_COLS = 4500


# ----------------------------------------------------------------------------
# Host-side weight preparation (pure numpy; done once per kernel() call)
# ----------------------------------------------------------------------------

def _prep(inputs):
    P = {}
    qs = f32(1.0 / np.sqrt(DH))

    WQK = np.zeros((L, 128, WQK_COLS), np.float16)
    WVO = np.zeros((L, 128, WVO_COLS), np.float16)
    WI1 = np.zeros((L, 128, WI_COLS), np.float16)
    WI2 = np.zeros((L, 128, WI_COLS), np.float16)
    WO1 = np.zeros((L, 128, WO_COLS), np.float16)
    WO2 = np.zeros((L, 128, WO_COLS), np.float16)
    BIA = np.zeros((L, 128, B_COLS), np.float16)

    for i in range(L):
        g1, b1 = inputs['ln1_g'][i], inputs['ln1_b'][i]
        g2, b2 = inputs['ln2_g'][i], inputs['ln2_b'][i]
        WqT = (inputs['Wq'][i] * g1[None, :]).T * qs      # [in, out]
        WkT = (inputs['Wk'][i] * g1[None, :]).T
        bq = (b1 @ inputs['Wq'][i].T + inputs['bq'][i]) * qs
        bk = b1 @ inputs['Wk'][i].T + inputs['bk'][i]
        qk = np.concatenate([WqT, WkT], axis=1)           # [768, 1536]
        for g in range(3):
            for c in range(KC):
                WQK[i, :, (g * 6 + c) * 512:(g * 6 + c + 1) * 512] = \
                    qk[c * 128:(c + 1) * 128,
                       g * 512:(g + 1) * 512].astype(np.float16)
        bqk_full = np.concatenate([bq, bk]).astype(np.float16)   # [1536]
        BIA[i, :, BQK:BQK + 12] = bqk_full.reshape(12, 128).T
        WvT = (inputs['Wv'][i] * g1[None, :]).T
        WaoT = inputs['Wao'][i].T
        for c in range(KC):
            WVO[i, :, WVO_V + c * 768:WVO_V + (c + 1) * 768] = \
                WvT[c * 128:(c + 1) * 128].astype(np.float16)
            WVO[i, :, WVO_AO + c * 768:WVO_AO + (c + 1) * 768] = \
                WaoT[c * 128:(c + 1) * 128].astype(np.float16)
        BIA[i, 0, BV:BV + D] = (b1 @ inputs['Wv'][i].T
                                + inputs['bv'][i]).astype(np.float16)
        BIA[i, 0, BAO:BAO + D] = inputs['bao'][i].astype(np.float16)
        WiT = (inputs['Wi'][i] * g2[None, :]).T           # [768, 3072]
        for g in range(6):
            dst = WI1 if g < 3 else WI2
            gg = g % 3
            for c in range(KC):
                dst[i, :, (gg * 6 + c) * 512:(gg * 6 + c + 1) * 512] = \
                    WiT[c * 128:(c + 1) * 128,
                        g * 512:(g + 1) * 512].astype(np.float16)
        bi_full = (b2 @ inputs['Wi'][i].T + inputs['bi'][i]).astype(np.float16)
        BIA[i, :, BI:BI + 24] = bi_full.reshape(24, 128).T
        WoT = inputs['Wo'][i].T                            # [3072, 768]
        for c in range(HC):
            dst = WO1 if c < 12 else WO2
            cc = c % 12
            dst[i, :, cc * 768:(cc + 1) * 768] = \
                WoT[c * 128:(c + 1) * 128].astype(np.float16)
        BIA[i, 0, BO:BO + D] = inputs['bo'][i].astype(np.float16)

    P.update(WQK=WQK, WVO=WVO, WI1=WI1, WI2=WI2, WO1=WO1, WO2=WO2, BIA=BIA)

    # MoE / classifier packed tensors
    MOE = np.zeros((NE, 128, M1_COLS), np.float16)
    CW1 = np.zeros((NE, 7, 128, 2 * KC * D2), np.float16)

    for e in range(NE):
        i = 2 * e
        g2, b2 = inputs['ln2_g'][i], inputs['ln2_b'][i]
        Au = inputs['A_up'][e]; Bu = inputs['B_up'][e]
        Ad = inputs['A_down'][e]; Bd = inputs['B_down'][e]
        AuTf = np.concatenate([(Au[ee] * g2[None, :]).T for ee in range(E)], axis=1)
        for c in range(KC):
            MOE[e, :, AU_O + c * ER:AU_O + (c + 1) * ER] = \
                AuTf[c * 128:(c + 1) * 128].astype(np.float16)
        MOE[e, 0, LUP_O:LUP_O + ER] = np.concatenate(
            [b2 @ Au[ee].T for ee in range(E)]).astype(np.float16)
        AdTf = np.concatenate([Ad[ee].T for ee in range(E)], axis=1)   # [H, 120]
        for c in range(HC):
            MOE[e, :, AD_O + c * ER:AD_O + (c + 1) * ER] = \
                AdTf[c * 128:(c + 1) * 128].astype(np.float16)
        for ee in range(E):
            Cm = Ad[ee] @ Bu[ee]                                        # [r, r']
            MOE[e, ee * RK:(ee + 1) * RK,
                CB_O + ee * RK:CB_O + (ee + 1) * RK] = \
                (SCALE * Cm.T).astype(np.float16)
        Bdf = np.concatenate([Bd[ee].T for ee in range(E)], axis=0) * SCALE  # [120, D]
        MOE[e, :ER, BDF_O:BDF_O + D] = Bdf.astype(np.float16)
        BdfDf = (2.0 * Bdf.T / f32(D))                                  # [D, 120]
        for c in range(KC):
            MOE[e, :, BDD_O + c * ER:BDD_O + (c + 1) * ER] = \
                BdfDf[c * 128:(c + 1) * 128].astype(np.float16)
        Bdm = Bdf.mean(axis=1)                                          # [120]
        for ee in range(E):
            MOE[e, ee * RK:(ee + 1) * RK, BDM_O + ee] = \
                Bdm[ee * RK:(ee + 1) * RK].astype(np.float16)
            sl = slice(ee * RK, (ee + 1) * RK)
            MOE[e, ee * RK:(ee + 1) * RK, GR_O + ee * RK:GR_O + (ee + 1) * RK] = \
                ((Bdf[sl] @ Bdf[sl].T) / f32(D)).astype(np.float16)
        # classifier weights; cW1 in 7 blocks of 2 diseases
        cW1e = inputs['cW1'][e]
        for d in range(ND):
            W1T = cW1e[d].T                                             # [768, 384]
            dd, dh = d // 2, d % 2
            for c in range(KC):
                CW1[e, dd, :, (dh * KC + c) * D2:(dh * KC + c + 1) * D2] = \
                    W1T[c * 128:(c + 1) * 128].astype(np.float16)
        # clsPack rows (d*8+j): g|b|w2|bias, then cb2 col
        for d in range(ND):
            for j in range(8):
                r = d * 8 + j
                MOE[e, r, CLS_O + 0:CLS_O + D2] = inputs['clng'][e][d]
                MOE[e, r, CLS_O + D2:CLS_O + 2 * D2] = inputs['clnb'][e][d]
                MOE[e, r, CLS_O + 2 * D2:CLS_O + 3 * D2] = inputs['cW2'][e][d]
                MOE[e, r, CLS_O + 3 * D2:CLS_O + 4 * D2] = inputs['cb1'][e][d]
                MOE[e, r, CB2_O] = inputs['cb2'][e][d]

    P.update(MOE=MOE, CW1=CW1)

    # ---- constant packs ----
    CPK = np.zeros((128, CPK_COLS), np.float16)

    CPK[0, SEL2_O:SEL2_O + 64] = 1.0
    CPK[1, SEL2_O + 64:SEL2_O + 128] = 1.0
    for d in range(ND):
        for j in range(8):
            CPK[d * 8 + j, RM_O + d] = 1.0
            CPK[d * 8 + j, SELJ_O + j] = 1.0
    for j in range(8):
        CPK[j, SELB_O + 104 + j] = 1.0

    ks = np.arange(128)
    CPK[0, ATTLT_O:ATTLT_O + 128] = 1.0
    qreal = (ks % TS < S).astype(np.float16)
    for h in range(NH):
        CPK[0, ATTRT_O + h * 128:ATTRT_O + (h + 1) * 128] = -30000.0 * qreal
    for j in range(4):
        CPK[1 + j, ATTLT_O:ATTLT_O + 128] = \
            ((ks // TS == j) & (ks % TS < S)).astype(np.float16)
        blk = ((ks // TS == j) & (ks % TS < S)).astype(np.float16) * 30000.0
        for h in range(NH):
            CPK[1 + j, ATTRT_O + h * 128:ATTRT_O + (h + 1) * 128] = blk

    mask = inputs['mask']; cnt = mask.sum(axis=0)
    for tg in range(2):
        for d in range(ND):
            for bl in range(4):
                col = d * 4 + bl
                CPK[bl * TS + 1: bl * TS + 1 + SR,
                    MPOOL_O + tg * 56 + col] = (mask[:, d] / cnt[d]).astype(np.float16)
    for tg in range(2):
        for bl in range(4):
            j = tg * 4 + bl
            CPK[j, IND2_O + tg * 128 + bl * TS:IND2_O + tg * 128 + (bl + 1) * TS] = 1.0
    for ee in range(E):
        CPK[ee * RK:(ee + 1) * RK, SEG_O + ee] = 1.0
        CPK[ee, REP_O + ee * RK:REP_O + (ee + 1) * RK] = 1.0

    qq, kk = np.meshgrid(np.arange(128), np.arange(128), indexing='ij')
    CPK[:, ATTM2_O:ATTM2_O + 128] = (((qq // TS) == (kk // TS))
                                     & ((kk % TS) < S)).astype(np.float16)
    m2t = (((qq // TS) == (kk // TS)) & ((qq % TS) < S)).astype(np.float16)
    CPK[:, ATTM2T_O:ATTM2T_O + 128] = m2t
    CPK[:, ATTM2T_O + 128:ATTM2T_O + 256] = m2t

    maskS = np.zeros((TS, ND), f32)
    maskS[1:1 + SR] = mask
    CPK[:, MASKS_O:MASKS_O + ND] = np.tile(maskS, (4, 1)).astype(np.float16)
    CPK[:, FG_O:FG_O + D] = np.tile(inputs['fg'][None, :],
                                    (128, 1)).astype(np.float16)
    CPK[:, FB_O:FB_O + D] = np.tile(inputs['fb'][None, :],
                                    (128, 1)).astype(np.float16)

    P.update(CPK=CPK)
    return P


def _shard_x0(inputs):
    """Per-core [128, 2, 768] initial residual streams (token = tg*128+p)."""
    cls = np.asarray(inputs['cls_token'][0, 0], f32)
    rf = np.asarray(inputs['region_features'], f32)
    shards = []
    for c in range(NC):
        x0 = np.zeros((NT, D), f32)
        for bl in range(BPC):
            b = c * BPC + bl
            x0[bl * TS] = cls
            x0[bl * TS + 1: bl * TS + 1 + SR] = rf[b]
        shards.append(np.ascontiguousarray(
            x0.reshape(2, 128, D).transpose(1, 0, 2)))
    return shards


# ----------------------------------------------------------------------------
# Bass/Tile program
# ----------------------------------------------------------------------------

def _build(sim_gelu=False):
    nc = bacc.Bacc("TRN2", target_bir_lowering=False, debug=False)
    _PHASES.clear()

    def _mark(label):
        _PHASES.append((label, nc.next_id()))

    def din(name, shape, dt):
        return nc.dram_tensor(name, list(shape), dt, kind="ExternalInput")

    t_x0 = din("x0", (128, 2, D), F32)
    t_WQK = din("WQK", (L, 128, WQK_COLS), F16)
    t_WVO = din("WVO", (L, 128, WVO_COLS), F16)
    t_WI1 = din("WI1", (L, 128, WI_COLS), F16)
    t_WI2 = din("WI2", (L, 128, WI_COLS), F16)
    t_WO1 = din("WO1", (L, 128, WO_COLS), F16)
    t_WO2 = din("WO2", (L, 128, WO_COLS), F16)
    t_BIA = din("BIA", (L, 128, B_COLS), F16)
    t_MOE = din("MOE", (NE, 128, M1_COLS), F16)
    t_CW1 = din("CW1", (NE, 7, 128, 2 * KC * D2), F16)
    t_CPK = din("CPK", (128, CPK_COLS), F16)
    t_out = nc.dram_tensor("out", [128, 2, D], F32, kind="ExternalOutput")

    with tile.TileContext(nc) as tc:
        with (
            tc.tile_pool(name="const", bufs=1) as cpool,
            tc.tile_pool(name="resid", bufs=1) as hpool,
            tc.tile_pool(name="wstream", bufs=3) as wpool,
            tc.tile_pool(name="wbias", bufs=2) as bpool,
            tc.tile_pool(name="wmoe", bufs=1) as wmpool,
            tc.tile_pool(name="wcls", bufs=2) as wcpool,
            tc.tile_pool(name="acts", bufs=1) as apool,
            tc.tile_pool(name="scrA", bufs=3) as sapool,
            tc.tile_pool(name="scrB", bufs=1) as spool,
            tc.tile_pool(name="small", bufs=1) as mpool,
            tc.tile_pool(name="psG", bufs=2, space="PSUM") as psG,
            tc.tile_pool(name="psY", bufs=3, space="PSUM") as psY,
        ):
            dma = nc.sync.dma_start

            def act_gelu(dst, src, bias=None):
                if not sim_gelu:
                    if bias is None:
                        nc.scalar.activation(dst, src, Act.Gelu)
                    else:
                        nc.scalar.activation(dst, src, Act.Gelu, bias=bias,
                                             scale=1.0)
                    return
                shp = list(dst.shape)
                y = sapool.tile(shp, F32, tag="gel_y", name="gel_y")
                if bias is None:
                    nc.scalar.activation(y[:], src, Act.Identity)
                else:
                    nc.scalar.activation(y[:], src, Act.Identity, bias=bias,
                                         scale=1.0)
                u = sapool.tile(shp, F32, tag="gel_u", name="gel_u")
                nc.vector.tensor_tensor(out=u[:], in0=y[:], in1=y[:],
                                        op=AluOp.mult)
                nc.vector.tensor_tensor(out=u[:], in0=u[:], in1=y[:],
                                        op=AluOp.mult)
                nc.vector.tensor_scalar(out=u[:], in0=u[:], scalar1=0.044715,
                                        scalar2=None, op0=AluOp.mult)
                nc.vector.tensor_tensor(out=u[:], in0=u[:], in1=y[:],
                                        op=AluOp.add)
                nc.scalar.activation(u[:], u[:], Act.Tanh, scale=0.7978845608)
                nc.vector.tensor_scalar(out=u[:], in0=u[:], scalar1=1.0,
                                        scalar2=0.5, op0=AluOp.add,
                                        op1=AluOp.mult)
                nc.vector.tensor_tensor(out=dst, in0=u[:], in1=y[:],
                                        op=AluOp.mult)

            # ---------------- constants ----------------
            ident = cpool.tile([128, 128], F16)
            from concourse.masks import make_identity
            make_identity(nc, ident[:])
            ident32 = cpool.tile([128, 128], F32)
            make_identity(nc, ident32[:])
            onesc = cpool.tile([1, 512], F16)   # K=1 matmul lhsT/rhs ones
            nc.vector.memset(onesc[:], 1.0)
            ones_k = cpool.tile([128, 1], F16)  # column-sum matmul rhs
            nc.vector.memset(ones_k[:], 1.0)
            ones15 = cpool.tile([E, 1], F16)
            nc.vector.memset(ones15[:], 1.0)
            cpk = cpool.tile([128, CPK_COLS], F16)
            dma(cpk[:], t_CPK[:])
            c_sel2 = cpk[0:2, SEL2_O:SEL2_O + 128]
            c_Rm = cpk[0:112, RM_O:RM_O + ND].rearrange(
                "p (a b) -> p a b", a=1)
            c_SelJ = cpk[0:112, SELJ_O:SELJ_O + 8]
            c_selB = cpk[0:8, SELB_O:SELB_O + 224]
            c_attLT = cpk[0:5, ATTLT_O:ATTLT_O + 128]
            c_attRT = cpk[0:5, ATTRT_O:ATTRT_O + NH * 128]
            c_Mpool = cpk[:, MPOOL_O:MPOOL_O + 112].rearrange(
                "p (t m) -> p t m", t=2)
            c_Ind2 = cpk[0:8, IND2_O:IND2_O + 256].rearrange(
                "p (t m) -> p t m", t=2)
            c_SegSel0 = cpk[0:ER, SEG_O:SEG_O + E]
            c_RepSel = cpk[0:E, REP_O:REP_O + ER]
            c_maskS = cpk[:, MASKS_O:MASKS_O + ND]
            c_M2 = cpk[:, ATTM2_O:ATTM2_O + 128].rearrange(
                "p (a b) -> p a b", a=1)
            c_M2T2 = cpk[:, ATTM2T_O:ATTM2T_O + 256].rearrange(
                "p (a b) -> p a b", a=2)
            c_fgB = cpk[:, FG_O:FG_O + D]
            c_fbB = cpk[:, FB_O:FB_O + D]
            magic_t = cpool.tile([128, 256], I32)
            nc.vector.memset(magic_t[:], 0x5f3759df)
            dumact = cpool.tile([1, 2], F32)
            nc.vector.memset(dumact[:], 0.0)

            def rsqrt_dve(dst, var_ap, eps):
                """dst = 1/sqrt(var_ap + eps), DVE-only (bit hack + 2 Newton)."""
                shp = list(dst.shape)
                p, n = shp[0], int(np.prod(shp[1:]))
                v = mpool.tile(shp, F32, tag="rsqv", bufs=2, name="rsqv")
                nc.vector.tensor_scalar(out=v[:], in0=var_ap, scalar1=float(eps),
                                        scalar2=None, op0=AluOp.add)
                ti_ = mpool.tile(shp, I32, tag="rsqt", bufs=2, name="rsqt")
                nc.vector.tensor_scalar(out=ti_[:], in0=v[:].bitcast(I32),
                                        scalar1=1, scalar2=None,
                                        op0=AluOp.logical_shift_right)
                mg = magic_t[:p].rearrange("p n -> p n")[:, :n]
                nc.vector.tensor_tensor(out=ti_[:], in0=mg.rearrange(
                    "p (a b) -> p a b", a=1) if len(shp) == 3 else mg,
                    in1=ti_[:], op=AluOp.subtract)
                y = ti_[:].bitcast(F32)
                a_ = mpool.tile(shp, F32, tag="rsqa", bufs=2, name="rsqa")
                for it in range(2):
                    nc.vector.tensor_tensor(out=a_[:], in0=v[:], in1=y,
                                            op=AluOp.mult)
                    nc.vector.tensor_tensor(out=a_[:], in0=a_[:], in1=y,
                                            op=AluOp.mult)
                    nc.vector.tensor_scalar(out=a_[:], in0=a_[:], scalar1=-0.5,
                                            scalar2=1.5, op0=AluOp.mult,
                                            op1=AluOp.add)
                    nc.vector.tensor_tensor(out=dst if it == 1 else
                                            ti_[:].bitcast(F32),
                                            in0=y, in1=a_[:], op=AluOp.mult)

            # ---------------- persistent activations ----------------
            h = hpool.tile([128, 2, D], F32)
            res1 = hpool.tile([128, 2, D], F32)
            dma(h[:], t_x0[:])

            def ln_stats_tg(src, mvs, tg):
                st = mpool.tile([128, 2, 6], F32, tag="lnst", bufs=2)
                xs = src[:, tg, :].rearrange("p (a b) -> p a b", a=2)
                for a in range(2):
                    nc.vector.bn_stats(st[:, a, :], xs[:, a, :])
                nc.vector.bn_aggr(mvs[:, tg, :], st[:])

            def ln_stats(src, eps):
                """emit stats+rsqrt for both tgs; returns (mvs, rst2)."""
                mvs = mpool.tile([128, 2, 2], F32, tag="lnmv", bufs=2)
                for tg in range(2):
                    ln_stats_tg(src, mvs, tg)
                rst2 = mpool.tile([128, 2], F32, tag="lnrs", bufs=2)
                rsqrt_dve(rst2[:], mvs[:, :, 1], eps)
                return mvs, rst2

            def ln_apply(src, dst, stats, per_tg=None):
                mvs, rst2 = stats
                for tg in range(2):
                    nc.vector.tensor_scalar(
                        out=dst[:, tg, :], in0=src[:, tg, :],
                        scalar1=mvs[:, tg, 0:1], scalar2=rst2[:, tg:tg + 1],
                        op0=AluOp.subtract, op1=AluOp.mult)
                    if per_tg is not None:
                        per_tg(tg)

            def layernorm_16(src, dst, eps, per_tg=None):
                ln_apply(src, dst, ln_stats(src, eps), per_tg)

            def transpose6_tg(src16, dst, tg):
                for c in range(KC):
                    pt = psY.tile([128, 128], F16, tag="sm")
                    nc.tensor.transpose(pt[:], src16[:, tg, c * 128:(c + 1) * 128],
                                        ident[:])
                    nc.scalar.activation(dst[:, c, tg * 128:(tg + 1) * 128],
                                         pt[:], Act.Copy)

            def transpose6(src16, dst):
                """src16 [128, 2, D] f16 -> dst [128, KC, 256] f16 (feature-major)."""
                for c in range(KC):
                    for tg in range(2):
                        pt = psY.tile([128, 128], F16, tag="sm")
                        nc.tensor.transpose(pt[:], src16[:, tg, c * 128:(c + 1) * 128],
                                            ident[:])
                        nc.scalar.activation(dst[:, c, tg * 128:(tg + 1) * 128],
                                             pt[:], Act.Copy)

            # ---------------- layers ----------------
            h_stats = ln_stats(h, 1e-12)
            for i in range(L):
                even = (i % 2 == 0)
                e = i // 2
                _mark(f'L{i}.dma')
                # ---- all weight DMAs for the layer, in consumption order ----
                b_t = bpool.tile([128, B_COLS], F16, tag="bias")
                dma(b_t[:], t_BIA[i, :, :])
                w_qk = wpool.tile([128, WQK_COLS], F16, tag="w", name="w_qk")
                dma(w_qk[:], t_WQK[i, :, :])
                w_vo = wpool.tile([128, WVO_COLS], F16, tag="w", name="w_vo")
                dma(w_vo[:], t_WVO[i, :, :])
                w_i1 = wpool.tile([128, WI_COLS], F16, tag="w", name="w_i1")
                dma(w_i1[:], t_WI1[i, :, :])
                w_i2 = wpool.tile([128, WI_COLS], F16, tag="w", name="w_i2")
                dma(w_i2[:], t_WI2[i, :, :])
                w_o1 = wpool.tile([128, WO_COLS], F16, tag="w", name="w_o1")
                dma(w_o1[:], t_WO1[i, :, :])
                w_o2 = wpool.tile([128, WO_COLS], F16, tag="w", name="w_o2")
                dma(w_o2[:], t_WO2[i, :, :])

                _mark(f'L{i}.ln1')
                # LN1 -> n1 (f16) -> n1T (stats precomputed at end of prev layer)
                n1 = apool.tile([128, 2, D], F16, tag="n1")
                n1T = apool.tile([128, KC, 256], F16, tag="n1T")
                ln_apply(h, n1, h_stats,
                         per_tg=lambda tg: transpose6_tg(n1, n1T, tg))
                # hoist Exp act-table load off the attention critical path
                if not sim_gelu:
                    nc.scalar.activation(dumact[:, 0:1], dumact[:, 0:1],
                                         Act.Exp)

                _mark(f'L{i}.qk')
                # QK^T (transposed out; bias via Act-Identity biased drains)
                qkT = apool.tile([128, 12, 256], F16, tag="bigact")
                for g in range(3):
                    grps = [psG.tile([128, 2, 512], F32, tag="grp",
                                     name=f"qkg{g}{hf}") for hf in range(2)]
                    for c in range(KC):
                        for j in range(4):
                            nc.tensor.matmul(grps[j // 2][:, j % 2, :256],
                                             w_qk[:, (g * 6 + c) * 512 + j * 128:
                                                  (g * 6 + c) * 512 + (j + 1) * 128],
                                             n1T[:, c, :],
                                             start=(c == 0), stop=(c == KC - 1))
                    for hf in range(2):
                        for sub in range(2):
                            fc = g * 4 + 2 * hf + sub
                            nc.scalar.activation(
                                qkT[:, fc, :], grps[hf][:, sub, :256],
                                Act.Identity, bias=b_t[:, BQK + fc:BQK + fc + 1],
                                scale=1.0)

                _mark(f'L{i}.v')
                # V (untransposed: [tok, dv]); bias via K=1 matmul
                V = apool.tile([128, 2, D], F16, tag="V")
                grpv = [psG.tile([128, 2, 512], F32, tag="grp",
                                 name=f"vg{tg}") for tg in range(2)]
                for j in range(4):
                    sl = slice((j % 2) * 384, (j % 2 + 1) * 384)
                    nc.tensor.matmul(grpv[j // 2][:, j % 2, :384],
                                     onesc[:1, :128],
                                     b_t[0:1, BV + sl.start:BV + sl.stop],
                                     start=True, stop=False)
                for c in range(KC):
                    for j in range(4):
                        tg, n = j // 2, j % 2
                        sl = slice(n * 384, (n + 1) * 384)
                        nc.tensor.matmul(grpv[tg][:, n, :384],
                                         n1T[:, c, tg * 128:(tg + 1) * 128],
                                         w_vo[:, WVO_V + c * 768 + sl.start:
                                              WVO_V + c * 768 + sl.stop],
                                         start=False, stop=(c == KC - 1))
                for tg in range(2):
                    nc.scalar.activation(
                        V[:, tg, :].rearrange("p (n d) -> p n d", n=2),
                        grpv[tg][:, :, :384], Act.Copy)

                _mark(f'L{i}.attn')
                # attention — scoresT[k, q] layout (keys restricted to own tg),
                # no transposes: V [tok, dv] is directly the o-matmul lhsT.
                oT = apool.tile([128, KC, 256], F16, tag="oT")
                for tg in range(2):
                    tgs = slice(tg * 128, (tg + 1) * 128)
                    expT = sapool.tile([128, NH, 128], F16, tag="expT")
                    for fc in range(KC):
                        pssc = psY.tile([128, 256], F32, tag="sm")
                        for pp in range(2):
                            hd = 2 * fc + pp
                            off = pp * 64
                            ps_h = slice(pp * 128, (pp + 1) * 128)
                            nc.tensor.matmul(pssc[:, ps_h], c_attLT[:],
                                             c_attRT[:, hd * 128:(hd + 1) * 128],
                                             start=True, stop=False)
                            nc.tensor.matmul(pssc[:, ps_h],
                                             qkT[off:off + 64, 6 + fc, tgs],
                                             qkT[off:off + 64, fc, tgs],
                                             start=False, stop=True)
                        nc.scalar.activation(
                            expT[:, 2 * fc:2 * fc + 2, :], pssc[:], Act.Exp)
                    # per-(q, head) softmax denominators via PE column sums
                    ps_rs = psY.tile([128, NH], F32, tag="sm")
                    for hd in range(NH):
                        nc.tensor.matmul(ps_rs[:, hd:hd + 1],
                                         expT[:, hd, :],
                                         ones_k[:], start=True, stop=True)
                    rinv = mpool.tile([128, NH], F32, tag="rinv", bufs=2)
                    nc.vector.reciprocal(rinv[:], ps_rs[:])
                    rinv16 = mpool.tile([128, NH], F16, tag="rinv16", bufs=2)
                    nc.scalar.activation(rinv16[:], rinv[:], Act.Copy)
                    for c in range(KC):
                        # psn[p, q] = rinv16[q, 2c + p//64] via stride-0 lhsT
                        psn = psY.tile([128, 128], F32, tag="sm")
                        for hh in range(2):
                            lhsT = rinv16[:, 2 * c + hh:2 * c + hh + 1] \
                                .to_broadcast((128, 1, 64))[:, 0, :]
                            nc.tensor.matmul(psn[hh * 64:(hh + 1) * 64, :],
                                             lhsT, ident[:],
                                             start=True, stop=True)
                        psnS = sapool.tile([128, 128], F16, tag="psnS")
                        nc.scalar.activation(psnS[:], psn[:], Act.Copy)
                        pso = psY.tile([128, 128], F32, tag="sm")
                        for hh in range(2):
                            hd = 2 * c + hh
                            nc.tensor.matmul(pso[hh * 64:(hh + 1) * 64, :],
                                             V[:, tg, hd * 64:(hd + 1) * 64],
                                             expT[:, hd, :],
                                             start=True, stop=True)
                        nc.vector.tensor_tensor(out=oT[:, c, tgs],
                                                in0=pso[:], in1=psnS[:],
                                                op=AluOp.mult)
                # hoist Gelu act-table load off the FFN critical path
                if not sim_gelu:
                    nc.scalar.activation(dumact[:, 1:2], dumact[:, 1:2],
                                         Act.Gelu)

                _mark(f'L{i}.ao')
                # AO projection + residual
                if even:
                    attnH = apool.tile([128, 2, D], F16, tag="n1")
                grpa = [psG.tile([128, 2, 512], F32, tag="grp",
                                 name=f"aog{tg}") for tg in range(2)]
                for j in range(4):
                    sl = slice((j % 2) * 384, (j % 2 + 1) * 384)
                    nc.tensor.matmul(grpa[j // 2][:, j % 2, :384],
                                     onesc[:1, :128],
                                     b_t[0:1, BAO + sl.start:BAO + sl.stop],
                                     start=True, stop=False)
                for c in range(KC):
                    for j in range(4):
                        tg, n = j // 2, j % 2
                        sl = slice(n * 384, (n + 1) * 384)
                        nc.tensor.matmul(grpa[tg][:, n, :384],
                                         oT[:, c, tg * 128:(tg + 1) * 128],
                                         w_vo[:, WVO_AO + c * 768 + sl.start:
                                              WVO_AO + c * 768 + sl.stop],
                                         start=False, stop=(c == KC - 1))
                mvs2 = mpool.tile([128, 2, 2], F32, tag="lnmv", bufs=2)
                for tg in range(2):
                    if even:
                        nc.scalar.activation(
                            attnH[:, tg, :].rearrange("p (n d) -> p n d", n=2),
                            grpa[tg][:, :, :384], Act.Copy)
                    nc.vector.tensor_tensor(
                        out=res1[:, tg, :].rearrange("p (n d) -> p n d", n=2),
                        in0=grpa[tg][:, :, :384],
                        in1=h[:, tg, :].rearrange("p (n d) -> p n d", n=2),
                        op=AluOp.add)
                    ln_stats_tg(res1, mvs2, tg)
                rst2b = mpool.tile([128, 2], F32, tag="lnrs", bufs=2)
                rsqrt_dve(rst2b[:], mvs2[:, :, 1], 1e-12)

                if even:
                    _mark(f'L{i}.moe_pool')
                    # single merged MoE pack DMA (weights for the whole tail);
                    # issued before the pooling matmuls to cover its latency
                    moepk = wmpool.tile([128, M1_COLS], F16, tag="moepk")
                    dma(moepk[:], t_MOE[e, :, :])
                    # prefetch the first two classifier weight blocks so the
                    # first emit_cls_dd calls during FFN-up don't stall
                    cw1_tiles = {}

                    def issue_cw1(dd):
                        w_ch = wcpool.tile([128, 2, KC, D2], F16, tag="wc1",
                                           name="wc1_c")
                        dma(w_ch[:], t_CW1[e, dd, :, :].rearrange(
                            "p (h c z) -> p h c z", h=2, c=KC))
                        cw1_tiles[dd] = w_ch

                    issue_cw1(0)
                    issue_cw1(1)
                    # pooled^T [128, KC, 112] (cols (d, tg*4+bl) after scatter)
                    pooledT = apool.tile([128, KC, 112], F16, tag="pooledT")
                    pview = pooledT.rearrange("p c (d g) -> p c d g", g=8)
                    for c in range(KC):
                        for tg in range(2):
                            ps = psY.tile([128, 4 * ND], F32, tag="sm")
                            nc.tensor.matmul(ps[:],
                                             attnH[:, tg, c * 128:(c + 1) * 128],
                                             c_Mpool[:, tg, :],
                                             start=True, stop=True)
                            pv = ps[:].rearrange("p (d g) -> p d g", g=4)
                            nc.vector.tensor_copy(
                                pview[:, c, :, tg * 4:tg * 4 + 4], pv)
                    c_cg = moepk[0:112, CLS_O + 0:CLS_O + D2]
                    c_cbt = moepk[0:112, CLS_O + D2:CLS_O + 2 * D2]
                    c_w2 = moepk[0:112, CLS_O + 2 * D2:CLS_O + 3 * D2]
                    c_b1 = moepk[0:112, CLS_O + 3 * D2:CLS_O + 4 * D2]
                    c_c2 = moepk[0:112, CB2_O:CB2_O + 1]
                    w_au = moepk[:, AU_O:AU_O + KC * ER].rearrange(
                        "p (c r) -> p c r", r=ER)
                    b_lup = moepk[0:1, LUP_O:LUP_O + ER]
                    w_ad = moepk[:, AD_O:AD_O + HC * ER].rearrange(
                        "p (c r) -> p c r", r=ER)
                    w_cb = moepk[0:ER, CB_O:CB_O + ER]
                    w_bdd = moepk[:, BDD_O:BDD_O + KC * ER].rearrange(
                        "p (c r) -> p c r", r=ER)
                    w_gram = moepk[0:ER, GR_O:GR_O + ER]
                    w_bdm = moepk[0:ER, BDM_O:BDM_O + E]
                    w_bdf = moepk[0:ER, BDF_O:BDF_O + D]
                    psz = psY.tile([112, D2], F32, tag="zacc", bufs=1)
                    cls_state = {}

                    def emit_cls_dd(dd):
                        w_ch = cw1_tiles.pop(dd)
                        if dd + 2 <= 6:
                            issue_cw1(dd + 2)
                        for dh in range(2):
                            d = 2 * dd + dh
                            psd_ = psY.tile([8, D2], F32, tag="sm")
                            for c in range(KC):
                                nc.tensor.matmul(psd_[:],
                                                 pooledT[:, c, d * 8:(d + 1) * 8],
                                                 w_ch[:, dh, c, :],
                                                 start=(c == 0), stop=(c == KC - 1))
                            zd = sapool.tile([8, D2], F16, tag="zd")
                            nc.scalar.activation(zd[:], psd_[:], Act.Copy)
                            nc.tensor.matmul(psz[:],
                                             c_selB[:, 104 - 8 * d:216 - 8 * d],
                                             zd[:], start=(d == 0),
                                             stop=(d == ND - 1))

                def emit_cls_finish():
                    zsb = spool.tile([112, D2], F32, tag="zsb")
                    nc.vector.tensor_tensor(out=zsb[:], in0=psz[:], in1=c_b1[:],
                                            op=AluOp.add)
                    zst = mpool.tile([112, 6], F32, tag="lnstz")
                    nc.vector.bn_stats(zst[:], zsb[:])
                    zmv = mpool.tile([112, 2], F32, tag="lnmvz")
                    nc.vector.bn_aggr(zmv[:], zst[:])
                    zrstd = mpool.tile([112, 1], F32, tag="zrstd")
                    rsqrt_dve(zrstd[:], zmv[:, 1:2], 1e-5)
                    zn = spool.tile([112, D2], F32, tag="zn")
                    nc.vector.tensor_scalar(out=zn[:], in0=zsb[:],
                                            scalar1=zmv[:, 0:1], scalar2=zrstd[:],
                                            op0=AluOp.subtract, op1=AluOp.mult)
                    nc.vector.tensor_tensor(out=zn[:], in0=zn[:], in1=c_cg[:],
                                            op=AluOp.mult)
                    nc.vector.tensor_tensor(out=zn[:], in0=zn[:], in1=c_cbt[:],
                                            op=AluOp.add)
                    zg = spool.tile([112, D2], F32, tag="zg")
                    act_gelu(zg[:], zn[:])
                    nc.vector.tensor_tensor(out=zg[:], in0=zg[:], in1=c_w2[:],
                                            op=AluOp.mult)
                    ppre = mpool.tile([112, 1], F32, tag="ppre")
                    nc.vector.reduce_sum(ppre[:], zg[:], axis=mybir.AxisListType.X)
                    nc.vector.tensor_tensor(out=ppre[:], in0=ppre[:], in1=c_c2[:],
                                            op=AluOp.add)
                    rp = mpool.tile([112, 1, ND], F16, tag="rp")
                    nc.vector.tensor_tensor(out=rp[:], in0=c_Rm[:],
                                            in1=ppre[:].to_broadcast((112, 1, ND)),
                                            op=AluOp.mult)
                    psda = psY.tile([8, ND], F32, tag="sm")
                    nc.tensor.matmul(psda[:], c_SelJ[:], rp[:, 0, :],
                                     start=True, stop=True)
                    da = mpool.tile([8, ND], F16, tag="da")
                    nc.vector.tensor_scalar(out=da[:], in0=psda[:], scalar1=0.0,
                                            scalar2=None, op0=AluOp.is_gt)

                    _mark(f'L{i}.moe_rout')
                    # routing weights w [128, tg, E] f32
                    w_rt = spool.tile([128, 2, E], F32, tag="wrt")
                    nact = mpool.tile([128, 2], F32, tag="nact")
                    for tg in range(2):
                        psd = psY.tile([128, ND], F32, tag="sm")
                        nc.tensor.matmul(psd[:], c_Ind2[:, tg, :], da[:],
                                         start=True, stop=True)
                        nc.vector.tensor_tensor(out=w_rt[:, tg, 0:ND], in0=psd[:],
                                                in1=c_maskS[:], op=AluOp.mult)
                        nc.vector.reduce_sum(nact[:, tg:tg + 1], w_rt[:, tg, 0:ND],
                                             axis=mybir.AxisListType.X)
                        nc.vector.tensor_scalar(out=nact[:, tg:tg + 1],
                                                in0=nact[:, tg:tg + 1],
                                                scalar1=1.0, scalar2=None,
                                                op0=AluOp.add)
                    rnact = mpool.tile([128, 2], F32, tag="rnact")
                    nc.vector.reciprocal(rnact[:], nact[:])
                    for tg in range(2):
                        nc.vector.tensor_scalar(out=w_rt[:, tg, 0:ND],
                                                in0=w_rt[:, tg, 0:ND],
                                                scalar1=rnact[:, tg:tg + 1],
                                                scalar2=None, op0=AluOp.mult)
                        nc.vector.tensor_copy(w_rt[:, tg, ND:E], rnact[:, tg:tg + 1])
                    wT = mpool.tile([E, 256], F32, tag="wT")
                    for tg in range(2):
                        pt = psY.tile([E, 128], F32, tag="sm")
                        nc.tensor.transpose(pt[:], w_rt[:, tg, :], ident32[:])
                        nc.vector.tensor_copy(wT[:, tg * 128:(tg + 1) * 128], pt[:])
                    cls_state['wT'] = wT

                _mark(f'L{i}.ln2')
                # LN2 -> n2 -> n2T (stats computed during AO drains)
                n2 = apool.tile([128, 2, D], F16, tag="n2")
                n2T = apool.tile([128, KC, 256], F16, tag="n2T")
                ln_apply(res1, n2, (mvs2, rst2b),
                         per_tg=lambda tg: transpose6_tg(n2, n2T, tg))

                _mark(f'L{i}.up')
                # FFN up (transposed out) + gelu with folded bias
                interT = apool.tile([128, HC, 256], F16, tag="bigact")
                for g in range(6):
                    w_i = w_i1 if g < 3 else w_i2
                    gg = g % 3
                    grps = [psG.tile([128, 2, 512], F32, tag="grp",
                                     name=f"upg{g}{hf}") for hf in range(2)]
                    for c in range(KC):
                        for j in range(4):
                            nc.tensor.matmul(grps[j // 2][:, j % 2, :256],
                                             w_i[:, (gg * 6 + c) * 512 + j * 128:
                                                 (gg * 6 + c) * 512 + (j + 1) * 128],
                                             n2T[:, c, :],
                                             start=(c == 0), stop=(c == KC - 1))
                    for hf in range(2):
                        for sub in range(2):
                            hc = g * 4 + 2 * hf + sub
                            act_gelu(interT[:, hc, :], grps[hf][:, sub, :256],
                                     bias=b_t[:, BI + hc:BI + hc + 1])
                    if even:
                        _mark(f'L{i}.moe_cls')
                        if g < 5:
                            emit_cls_dd(g)
                        else:
                            emit_cls_dd(5)
                            emit_cls_dd(6)
                        _mark(f'L{i}.up')

                if even:
                    _mark(f'L{i}.moe_lora')
                    # LoRA rails
                    ps = psY.tile([ER, 256], F32, tag="sm")
                    nc.tensor.matmul(ps[:], b_lup, onesc[:1, :256],
                                     start=True, stop=False)
                    for c in range(KC):
                        nc.tensor.matmul(ps[:], w_au[:, c, :], n2T[:, c, :],
                                         start=False, stop=(c == KC - 1))
                    lup_rT = spool.tile([ER, 256], F16, tag="luprT")
                    nc.vector.tensor_copy(lup_rT[:], ps[:])

                    ps2 = psY.tile([ER, 256], F32, tag="sm")
                    nc.tensor.matmul(ps2[:], w_cb, lup_rT[:], start=True, stop=False)
                    for c in range(HC):
                        nc.tensor.matmul(ps2[:], w_ad[:, c, :], interT[:, c, :],
                                         start=False, stop=(c == HC - 1))
                    ldr16 = spool.tile([ER, 256], F16, tag="ldr16")
                    nc.vector.tensor_copy(ldr16[:], ps2[:])
                    ldr32 = spool.tile([ER, 256], F32, tag="ldr32")
                    nc.vector.tensor_copy(ldr32[:], ps2[:])

                _mark(f'L{i}.down')
                # FFN down
                if even:
                    base = apool.tile([128, 2, D], F16, tag="base")
                grpd = [psG.tile([128, 2, 512], F32, tag="grp",
                                 name=f"dng{tg}") for tg in range(2)]
                for j in range(4):
                    sl = slice((j % 2) * 384, (j % 2 + 1) * 384)
                    nc.tensor.matmul(grpd[j // 2][:, j % 2, :384],
                                     onesc[:1, :128],
                                     b_t[0:1, BO + sl.start:BO + sl.stop],
                                     start=True, stop=False)
                for c in range(HC):
                    w_o = w_o1 if c < 12 else w_o2
                    cc = c % 12
                    for j in range(4):
                        tg, n = j // 2, j % 2
                        sl = slice(n * 384, (n + 1) * 384)
                        nc.tensor.matmul(grpd[tg][:, n, :384],
                                         interT[:, c, tg * 128:(tg + 1) * 128],
                                         w_o[:, cc * 768 + sl.start:
                                             cc * 768 + sl.stop],
                                         start=False, stop=(c == HC - 1))
                if even:
                    _mark(f'L{i}.moe_cls2')
                    emit_cls_finish()
                for tg in range(2):
                    if not even:
                        nc.vector.tensor_tensor(
                            out=h[:, tg, :].rearrange("p (n d) -> p n d", n=2),
                            in0=grpd[tg][:, :, :384],
                            in1=res1[:, tg, :].rearrange("p (n d) -> p n d", n=2),
                            op=AluOp.add)
                    else:
                        nc.vector.tensor_copy(
                            base[:, tg, :].rearrange("p (n d) -> p n d", n=2),
                            grpd[tg][:, :, :384])

                if not even:
                    h_stats = ln_stats(h, 1e-12)
                    continue

                _mark(f'L{i}.moe_stats')
                # ================= MoE / classifier tail =================
                # base stats (mu, ms = var + mu^2), transposed to rows
                mums = mpool.tile([128, 2, 2], F32, tag="mums")   # [:, tg, (mu,ms)]
                for tg in range(2):
                    st = mpool.tile([128, 3, 6], F32, tag="lnst", bufs=2)
                    xs = base[:, tg, :].rearrange("p (a b) -> p a b", a=3)
                    for a in range(3):
                        nc.vector.bn_stats(st[:, a, :], xs[:, a, :])
                    mv = mpool.tile([128, 2], F32, tag="lnmv", bufs=2)
                    nc.vector.bn_aggr(mv[:], st[:])
                    nc.vector.tensor_copy(mums[:, tg, 0:1], mv[:, 0:1])
                    # ms = var + mu^2
                    musq = mpool.tile([128, 1], F32, tag="musq")
                    nc.vector.tensor_tensor(out=musq[:], in0=mv[:, 0:1],
                                            in1=mv[:, 0:1], op=AluOp.mult)
                    nc.vector.tensor_tensor(out=mums[:, tg, 1:2], in0=mv[:, 1:2],
                                            in1=musq[:], op=AluOp.add)
                muT = mpool.tile([1, 256], F32, tag="muT")
                msT = mpool.tile([1, 256], F32, tag="msT")
                for tg in range(2):
                    pt = psY.tile([1, 128], F32, tag="sm")
                    nc.tensor.transpose(pt[:], mums[:, tg, 0:1], ident32[:])
                    nc.vector.tensor_copy(muT[:, tg * 128:(tg + 1) * 128], pt[:])
                    pt2 = psY.tile([1, 128], F32, tag="sm")
                    nc.tensor.transpose(pt2[:], mums[:, tg, 1:2], ident32[:])
                    nc.vector.tensor_copy(msT[:, tg * 128:(tg + 1) * 128], pt2[:])

                baseT = apool.tile([128, KC, 256], F16, tag="n1T")
                transpose6(base, baseT)

                # (cls finish + routing emitted during down via emit_cls_finish)
                _mark(f'L{i}.moe_g')
                # G^T (cross term, x2 folded in BdfD) and quad term
                psg = psY.tile([ER, 256], F32, tag="sm")
                for c in range(KC):
                    nc.tensor.matmul(psg[:], w_bdd[:, c, :], baseT[:, c, :],
                                     start=(c == 0), stop=(c == KC - 1))
                Pcross = spool.tile([ER, 256], F16, tag="pcross")
                nc.vector.tensor_tensor(out=Pcross[:], in0=psg[:],
                                        in1=ldr32[:], op=AluOp.mult)

                psq = psY.tile([ER, 256], F32, tag="sm")
                nc.tensor.matmul(psq[:], w_gram, ldr16[:],
                                 start=True, stop=True)
                Pquad = spool.tile([ER, 256], F16, tag="pquad")
                nc.vector.tensor_tensor(out=Pquad[:], in0=psq[:], in1=ldr32[:],
                                        op=AluOp.mult)

                # mu_e^T [E, 256]
                muT16 = mpool.tile([1, 256], F16, tag="muT16")
                nc.vector.tensor_copy(muT16[:], muT[:])
                msT16 = mpool.tile([1, 256], F16, tag="msT16")
                nc.vector.tensor_copy(msT16[:], msT[:])
                psmu = psY.tile([E, 256], F32, tag="sm")
                nc.tensor.matmul(psmu[:], w_bdm, ldr16[:], start=True, stop=False)
                nc.tensor.matmul(psmu[:], onesc[:1, :E], muT16[:],
                                 start=False, stop=True)
                muE = mpool.tile([E, 256], F32, tag="muE")
                nc.vector.tensor_copy(muE[:], psmu[:])

                # ms^T then var, rho
                psms = psY.tile([E, 256], F32, tag="sm")
                nc.tensor.matmul(psms[:], c_SegSel0, Pcross[:],
                                 start=True, stop=False)
                nc.tensor.matmul(psms[:], c_SegSel0, Pquad[:],
                                 start=False, stop=False)
                nc.tensor.matmul(psms[:], onesc[:1, :E], msT16[:],
                                 start=False, stop=True)
                musqE = mpool.tile([E, 256], F32, tag="musqE")
                nc.vector.tensor_tensor(out=musqE[:], in0=muE[:], in1=muE[:],
                                        op=AluOp.mult)
                varE = mpool.tile([E, 256], F32, tag="varE")
                nc.vector.tensor_tensor(out=varE[:], in0=psms[:], in1=musqE[:],
                                        op=AluOp.subtract)
                rho = mpool.tile([E, 256], F32, tag="rho")
                rsqrt_dve(rho[:], varE[:], 1e-5)

                # s_e = w * rho ; pack [sE | sE*muE] -> column sums -> scal/off
                packSO = mpool.tile([E, 512], F16, tag="packSO")
                wT = cls_state['wT']
                nc.vector.tensor_tensor(out=packSO[:, 0:256], in0=wT[:], in1=rho[:],
                                        op=AluOp.mult)
                nc.vector.tensor_tensor(out=packSO[:, 256:512],
                                        in0=packSO[:, 0:256], in1=muE[:],
                                        op=AluOp.mult)
                psso = psY.tile([1, 512], F32, tag="sm")
                nc.tensor.matmul(psso[:], ones15[:], packSO[:],
                                 start=True, stop=True)
                soT = mpool.tile([1, 512], F32, tag="soT")
                nc.vector.tensor_copy(soT[:], psso[:])
                scal = mpool.tile([128, 2], F32, tag="scal")
                off = mpool.tile([128, 2], F32, tag="off")
                for tg in range(2):
                    pt = psY.tile([128, 1], F32, tag="sm")
                    nc.tensor.transpose(pt[:], soT[:, tg * 128:(tg + 1) * 128],
                                        ident32[:1, :1])
                    nc.vector.tensor_copy(scal[:, tg:tg + 1], pt[:])
                    pt2 = psY.tile([128, 1], F32, tag="sm")
                    nc.tensor.transpose(pt2[:],
                                        soT[:, 256 + tg * 128:256 + (tg + 1) * 128],
                                        ident32[:1, :1])
                    nc.vector.tensor_copy(off[:, tg:tg + 1], pt2[:])

                # ls^T = ldown_r^T * repeat(s_e)
                psrep = psY.tile([ER, 256], F32, tag="sm")
                nc.tensor.matmul(psrep[:], c_RepSel, packSO[:, 0:256],
                                 start=True, stop=True)
                srep = mpool.tile([ER, 256], F32, tag="srep")
                nc.vector.tensor_copy(srep[:], psrep[:])
                lsT = spool.tile([ER, 256], F16, tag="lsT")
                nc.vector.tensor_tensor(out=lsT[:], in0=srep[:], in1=ldr32[:],
                                        op=AluOp.mult)

                _mark(f'L{i}.moe_fin')
                # final: h = (res1 - off) + (base*scal + ldown_mix)
                grpf = [psG.tile([128, 2, 512], F32, tag="grp",
                                 name=f"fing{tg}") for tg in range(2)]
                for j in range(4):
                    tg, n = j // 2, j % 2
                    sl = slice(n * 384, (n + 1) * 384)
                    nc.tensor.matmul(grpf[tg][:, n, :384],
                                     lsT[:, tg * 128:(tg + 1) * 128],
                                     w_bdf[:, sl], start=True, stop=True)
                for j in range(4):
                    tg, n = j // 2, j % 2
                    sl = slice(n * 384, (n + 1) * 384)
                    tmp = spool.tile([128, 384], F32, tag="ffn_tmp")
                    nc.vector.scalar_tensor_tensor(
                        out=tmp[:], in0=base[:, tg, sl],
                        scalar=scal[:, tg:tg + 1],
                        in1=grpf[tg][:, n, :384], op0=AluOp.mult, op1=AluOp.add)
                    nc.vector.scalar_tensor_tensor(
                        out=h[:, tg, sl], in0=res1[:, tg, sl],
                        scalar=off[:, tg:tg + 1], in1=tmp[:],
                        op0=AluOp.subtract, op1=AluOp.add)
                h_stats = ln_stats(h, 1e-12)

            _mark('final_ln')
            # ---------------- final LN ----------------
            hf = apool.tile([128, 2, D], F32, tag="base")
            ln_apply(h, hf, h_stats)   # writes f32 since tile dtype f32
            ot = apool.tile([128, 2, D], F32, tag="bigact",
                            name="ot")
            for tg in range(2):
                nc.vector.tensor_tensor(out=ot[:, tg, :], in0=hf[:, tg, :],
                                        in1=c_fgB, op=AluOp.mult)
                nc.vector.tensor_tensor(out=ot[:, tg, :], in0=ot[:, tg, :],
                                        in1=c_fbB, op=AluOp.add)
            dma(t_out[:], ot[:])

    nc.compile()
    return nc


_CACHE = {}


def _get_nc(sim_gelu=False):
    key = ("nc", sim_gelu)
    if key not in _CACHE:
        _CACHE[key] = _build(sim_gelu)
    return _CACHE[key]


def kernel(**inputs):
    inputs = {k: np.asarray(v) for k, v in inputs.items()}
    P = _prep(inputs)
    shards = _shard_x0(inputs)
    nc = _get_nc()
    base_map = {k: np.ascontiguousarray(v) for k, v in P.items()}
    in_maps = []
    for c in range(NC):
        m = dict(base_map)
        m["x0"] = np.ascontiguousarray(shards[c])
        in_maps.append(m)
    res = bass_utils.run_bass_kernel_spmd(nc, in_maps, core_ids=list(range(NC)))
    out = np.zeros((B, S, D), f32)
    for c in range(NC):
        oc = res.results[c]["out"].transpose(1, 0, 2).reshape(NT, D)
        for bl in range(BPC):
            out[c * BPC + bl] = oc[bl * TS: bl * TS + S]
    return out


# revision 18
# speedup vs baseline: 1.0950x; 1.0521x over previous
"""Trainium2 Bass kernel for nn_MedicalVisionTransformer (MoE-LoRA ViT).

Strategy: data-parallel over batch (8 cores x 8 batch items). Each core holds
its 256-token (8 batches x 32 slots: 30 real + 2 pad) residual stream in SBUF
for all 12 layers; only weights stream from HBM in fp16. MoE LoRA experts are
collapsed algebraically (rank-8 C matrices; per-expert LayerNorm folded into
per-token scalars via B_down Gram matrices) so no [B,S,E,H]/[B,S,E,D] tensor
is ever materialized.

v2: weights stream as a few large per-layer DMAs (SP sequencer / HWDGE were
instruction-count bound at ~100 DMAs/layer); per-partition biases (qk, FFN-up)
are folded into Activation-engine biased copies instead of K=1 matmuls; the
Exp/Gelu activation-table switches are hoisted off the critical path with
dummy ops.
"""

import sys

sys.path.insert(0, "/opt/trn_rl_repo")

import numpy as np

import concourse.bass as bass
import concourse.mybir as mybir
import concourse.tile as tile
from concourse import bacc
from concourse import bass_utils

f32 = np.float32
F32 = mybir.dt.float32
F16 = mybir.dt.float16
F32R = mybir.dt.float32r
I32 = mybir.dt.int32

B, SR, D, H, L, NH, ND, E, RK = 64, 29, 768, 3072, 12, 12, 14, 15, 8
S = SR + 1
SCALE = f32(16.0 / 8.0)
NE = L // 2
DH = D // NH
NC = 8
BPC = B // NC          # batches per core
TS = 32                # token slot per batch (30 real + 2 pad)
NT = BPC * TS          # 256 tokens per core
D2 = D // 2            # 384
ER = E * RK            # 120
KC = D // 128          # 6 feature chunks
HC = H // 128          # 24 hidden chunks

AluOp = mybir.AluOpType
Act = mybir.ActivationFunctionType

_PHASES = []   # (label, first_instruction_id) markers for profiling

# ---- packed weight block column offsets (f16 cols) ----
WQK_COLS = 9216      # 18 blocks of 512  (g*6+c)
WVO_V = 0            # 6 blocks of 768
WVO_AO = 4608        # 6 blocks of 768
WVO_COLS = 9216
WI_COLS = 9216       # 18 blocks of 512 per half ((g%3)*6+c)
WO_COLS = 9216       # 12 blocks of 768 per half
BQK = 0              # [128,12] per-chunk qk bias
BI = 12              # [128,24] per-chunk FFN-up bias
BV = 40              # row-0 strips
BAO = 808
BO = 1576
B_COLS = 2344
# moe pack offsets
AU_O, AD_O, BDD_O = 0, 720, 3600
CB_O, GR_O, BDM_O, BDF_O = 4320, 4440, 4560, 4575
CLS_O, CB2_O, LUP_O = 5343, 6879, 6880
M1_COLS = 7000
# const pack (f16) offsets
SEL2_O, RM_O, SELJ_O, SELB_O = 0, 128, 142, 150
ATTLT_O, ATTRT_O, MPOOL_O, IND2_O = 374, 502, 2038, 2150
SEG_O, REP_O = 2406, 2421
MASKS_O, FG_O, FB_O = 2560, 2574, 3342
ATTM2_O = 4110
ATTM2T_O = 4240
CPK_COLS = 4500


# ----------------------------------------------------------------------------
# Host-side weight preparation (pure numpy; done once per kernel() call)
# ----------------------------------------------------------------------------

def _prep(inputs):
    P = {}
    qs = f32(1.0 / np.sqrt(DH))

    WQK = np.zeros((L, 128, WQK_COLS), np.float16)
    WVO = np.zeros((L, 128, WVO_COLS), np.float16)
    WI1 = np.zeros((L, 128, WI_COLS), np.float16)
    WI2 = np.zeros((L, 128, WI_COLS), np.float16)
    WO1 = np.zeros((L, 128, WO_COLS), np.float16)
    WO2 = np.zeros((L, 128, WO_COLS), np.float16)
    BIA = np.zeros((L, 128, B_COLS), np.float16)

    for i in range(L):
        g1, b1 = inputs['ln1_g'][i], inputs['ln1_b'][i]
        g2, b2 = inputs['ln2_g'][i], inputs['ln2_b'][i]
        WqT = (inputs['Wq'][i] * g1[None, :]).T * qs      # [in, out]
        WkT = (inputs['Wk'][i] * g1[None, :]).T
        bq = (b1 @ inputs['Wq'][i].T + inputs['bq'][i]) * qs
        bk = b1 @ inputs['Wk'][i].T + inputs['bk'][i]
        qk = np.concatenate([WqT, WkT], axis=1)           # [768, 1536]
        for g in range(3):
            for c in range(KC):
                WQK[i, :, (g * 6 + c) * 512:(g * 6 + c + 1) * 512] = \
                    qk[c * 128:(c + 1) * 128,
                       g * 512:(g + 1) * 512].astype(np.float16)
        bqk_full = np.concatenate([bq, bk]).astype(np.float16)   # [1536]
        BIA[i, :, BQK:BQK + 12] = bqk_full.reshape(12, 128).T
        WvT = (inputs['Wv'][i] * g1[None, :]).T
        WaoT = inputs['Wao'][i].T
        for c in range(KC):
            WVO[i, :, WVO_V + c * 768:WVO_V + (c + 1) * 768] = \
                WvT[c * 128:(c + 1) * 128].astype(np.float16)
            WVO[i, :, WVO_AO + c * 768:WVO_AO + (c + 1) * 768] = \
                WaoT[c * 128:(c + 1) * 128].astype(np.float16)
        BIA[i, 0, BV:BV + D] = (b1 @ inputs['Wv'][i].T
                                + inputs['bv'][i]).astype(np.float16)
        BIA[i, 0, BAO:BAO + D] = inputs['bao'][i].astype(np.float16)
        WiT = (inputs['Wi'][i] * g2[None, :]).T           # [768, 3072]
        for g in range(6):
            dst = WI1 if g < 3 else WI2
            gg = g % 3
            for c in range(KC):
                dst[i, :, (gg * 6 + c) * 512:(gg * 6 + c + 1) * 512] = \
                    WiT[c * 128:(c + 1) * 128,
                        g * 512:(g + 1) * 512].astype(np.float16)
        bi_full = (b2 @ inputs['Wi'][i].T + inputs['bi'][i]).astype(np.float16)
        BIA[i, :, BI:BI + 24] = bi_full.reshape(24, 128).T
        WoT = inputs['Wo'][i].T                            # [3072, 768]
        for c in range(HC):
            dst = WO1 if c < 12 else WO2
            cc = c % 12
            dst[i, :, cc * 768:(cc + 1) * 768] = \
                WoT[c * 128:(c + 1) * 128].astype(np.float16)
        BIA[i, 0, BO:BO + D] = inputs['bo'][i].astype(np.float16)

    P.update(WQK=WQK, WVO=WVO, WI1=WI1, WI2=WI2, WO1=WO1, WO2=WO2, BIA=BIA)

    # MoE / classifier packed tensors
    MOE = np.zeros((NE, 128, M1_COLS), np.float16)
    CW1 = np.zeros((NE, 7, 128, 2 * KC * D2), np.float16)

    for e in range(NE):
        i = 2 * e
        g2, b2 = inputs['ln2_g'][i], inputs['ln2_b'][i]
        Au = inputs['A_up'][e]; Bu = inputs['B_up'][e]
        Ad = inputs['A_down'][e]; Bd = inputs['B_down'][e]
        AuTf = np.concatenate([(Au[ee] * g2[None, :]).T for ee in range(E)], axis=1)
        for c in range(KC):
            MOE[e, :, AU_O + c * ER:AU_O + (c + 1) * ER] = \
                AuTf[c * 128:(c + 1) * 128].astype(np.float16)
        MOE[e, 0, LUP_O:LUP_O + ER] = np.concatenate(
            [b2 @ Au[ee].T for ee in range(E)]).astype(np.float16)
        AdTf = np.concatenate([Ad[ee].T for ee in range(E)], axis=1)   # [H, 120]
        for c in range(HC):
            MOE[e, :, AD_O + c * ER:AD_O + (c + 1) * ER] = \
                AdTf[c * 128:(c + 1) * 128].astype(np.float16)
        for ee in range(E):
            Cm = Ad[ee] @ Bu[ee]                                        # [r, r']
            MOE[e, ee * RK:(ee + 1) * RK,
                CB_O + ee * RK:CB_O + (ee + 1) * RK] = \
                (SCALE * Cm.T).astype(np.float16)
        Bdf = np.concatenate([Bd[ee].T for ee in range(E)], axis=0) * SCALE  # [120, D]
        MOE[e, :ER, BDF_O:BDF_O + D] = Bdf.astype(np.float16)
        BdfDf = (2.0 * Bdf.T / f32(D))                                  # [D, 120]
        for c in range(KC):
            MOE[e, :, BDD_O + c * ER:BDD_O + (c + 1) * ER] = \
                BdfDf[c * 128:(c + 1) * 128].astype(np.float16)
        Bdm = Bdf.mean(axis=1)                                          # [120]
        for ee in range(E):
            MOE[e, ee * RK:(ee + 1) * RK, BDM_O + ee] = \
                Bdm[ee * RK:(ee + 1) * RK].astype(np.float16)
            sl = slice(ee * RK, (ee + 1) * RK)
            MOE[e, ee * RK:(ee + 1) * RK, GR_O + ee * RK:GR_O + (ee + 1) * RK] = \
                ((Bdf[sl] @ Bdf[sl].T) / f32(D)).astype(np.float16)
        # classifier weights; cW1 in 7 blocks of 2 diseases
        cW1e = inputs['cW1'][e]
        for d in range(ND):
            W1T = cW1e[d].T                                             # [768, 384]
            dd, dh = d // 2, d % 2
            for c in range(KC):
                CW1[e, dd, :, (dh * KC + c) * D2:(dh * KC + c + 1) * D2] = \
                    W1T[c * 128:(c + 1) * 128].astype(np.float16)
        # clsPack rows (d*8+j): g|b|w2|bias, then cb2 col
        for d in range(ND):
            for j in range(8):
                r = d * 8 + j
                MOE[e, r, CLS_O + 0:CLS_O + D2] = inputs['clng'][e][d]
                MOE[e, r, CLS_O + D2:CLS_O + 2 * D2] = inputs['clnb'][e][d]
                MOE[e, r, CLS_O + 2 * D2:CLS_O + 3 * D2] = inputs['cW2'][e][d]
                MOE[e, r, CLS_O + 3 * D2:CLS_O + 4 * D2] = inputs['cb1'][e][d]
                MOE[e, r, CB2_O] = inputs['cb2'][e][d]

    P.update(MOE=MOE, CW1=CW1)

    # ---- constant packs ----
    CPK = np.zeros((128, CPK_COLS), np.float16)

    CPK[0, SEL2_O:SEL2_O + 64] = 1.0
    CPK[1, SEL2_O + 64:SEL2_O + 128] = 1.0
    for d in range(ND):
        for j in range(8):
            CPK[d * 8 + j, RM_O + d] = 1.0
            CPK[d * 8 + j, SELJ_O + j] = 1.0
    for j in range(8):
        CPK[j, SELB_O + 104 + j] = 1.0

    ks = np.arange(128)
    CPK[0, ATTLT_O:ATTLT_O + 128] = 1.0
    qreal = (ks % TS < S).astype(np.float16)
    for h in range(NH):
        CPK[0, ATTRT_O + h * 128:ATTRT_O + (h + 1) * 128] = -30000.0 * qreal
    for j in range(4):
        CPK[1 + j, ATTLT_O:ATTLT_O + 128] = \
            ((ks // TS == j) & (ks % TS < S)).astype(np.float16)
        blk = ((ks // TS == j) & (ks % TS < S)).astype(np.float16) * 30000.0
        for h in range(NH):
            CPK[1 + j, ATTRT_O + h * 128:ATTRT_O + (h + 1) * 128] = blk

    mask = inputs['mask']; cnt = mask.sum(axis=0)
    for tg in range(2):
        for d in range(ND):
            for bl in range(4):
                col = d * 4 + bl
                CPK[bl * TS + 1: bl * TS + 1 + SR,
                    MPOOL_O + tg * 56 + col] = (mask[:, d] / cnt[d]).astype(np.float16)
    for tg in range(2):
        for bl in range(4):
            j = tg * 4 + bl
            CPK[j, IND2_O + tg * 128 + bl * TS:IND2_O + tg * 128 + (bl + 1) * TS] = 1.0
    for ee in range(E):
        CPK[ee * RK:(ee + 1) * RK, SEG_O + ee] = 1.0
        CPK[ee, REP_O + ee * RK:REP_O + (ee + 1) * RK] = 1.0

    qq, kk = np.meshgrid(np.arange(128), np.arange(128), indexing='ij')
    CPK[:, ATTM2_O:ATTM2_O + 128] = (((qq // TS) == (kk // TS))
                                     & ((kk % TS) < S)).astype(np.float16)
    m2t = (((qq // TS) == (kk // TS)) & ((qq % TS) < S)).astype(np.float16)
    CPK[:, ATTM2T_O:ATTM2T_O + 128] = m2t
    CPK[:, ATTM2T_O + 128:ATTM2T_O + 256] = m2t

    maskS = np.zeros((TS, ND), f32)
    maskS[1:1 + SR] = mask
    CPK[:, MASKS_O:MASKS_O + ND] = np.tile(maskS, (4, 1)).astype(np.float16)
    CPK[:, FG_O:FG_O + D] = np.tile(inputs['fg'][None, :],
                                    (128, 1)).astype(np.float16)
    CPK[:, FB_O:FB_O + D] = np.tile(inputs['fb'][None, :],
                                    (128, 1)).astype(np.float16)

    P.update(CPK=CPK)
    return P


def _shard_x0(inputs):
    """Per-core [128, 2, 768] initial residual streams (token = tg*128+p)."""
    cls = np.asarray(inputs['cls_token'][0, 0], f32)
    rf = np.asarray(inputs['region_features'], f32)
    shards = []
    for c in range(NC):
        x0 = np.zeros((NT, D), f32)
        for bl in range(BPC):
            b = c * BPC + bl
            x0[bl * TS] = cls
            x0[bl * TS + 1: bl * TS + 1 + SR] = rf[b]
        shards.append(np.ascontiguousarray(
            x0.reshape(2, 128, D).transpose(1, 0, 2)))
    return shards


# ----------------------------------------------------------------------------
# Bass/Tile program
# ----------------------------------------------------------------------------

def _build(sim_gelu=False):
    nc = bacc.Bacc("TRN2", target_bir_lowering=False, debug=False)
    _PHASES.clear()

    def _mark(label):
        _PHASES.append((label, nc.next_id()))

    def din(name, shape, dt):
        return nc.dram_tensor(name, list(shape), dt, kind="ExternalInput")

    t_x0 = din("x0", (128, 2, D), F32)
    t_WQK = din("WQK", (L, 128, WQK_COLS), F16)
    t_WVO = din("WVO", (L, 128, WVO_COLS), F16)
    t_WI1 = din("WI1", (L, 128, WI_COLS), F16)
    t_WI2 = din("WI2", (L, 128, WI_COLS), F16)
    t_WO1 = din("WO1", (L, 128, WO_COLS), F16)
    t_WO2 = din("WO2", (L, 128, WO_COLS), F16)
    t_BIA = din("BIA", (L, 128, B_COLS), F16)
    t_MOE = din("MOE", (NE, 128, M1_COLS), F16)
    t_CW1 = din("CW1", (NE, 7, 128, 2 * KC * D2), F16)
    t_CPK = din("CPK", (128, CPK_COLS), F16)
    t_out = nc.dram_tensor("out", [128, 2, D], F32, kind="ExternalOutput")

    with tile.TileContext(nc) as tc:
        with (
            tc.tile_pool(name="const", bufs=1) as cpool,
            tc.tile_pool(name="resid", bufs=1) as hpool,
            tc.tile_pool(name="wstream", bufs=3) as wpool,
            tc.tile_pool(name="wbias", bufs=2) as bpool,
            tc.tile_pool(name="wmoe", bufs=1) as wmpool,
            tc.tile_pool(name="wcls", bufs=2) as wcpool,
            tc.tile_pool(name="acts", bufs=1) as apool,
            tc.tile_pool(name="scrA", bufs=3) as sapool,
            tc.tile_pool(name="scrB", bufs=1) as spool,
            tc.tile_pool(name="small", bufs=1) as mpool,
            tc.tile_pool(name="psG", bufs=2, space="PSUM") as psG,
            tc.tile_pool(name="psY", bufs=3, space="PSUM") as psY,
        ):
            dma = nc.sync.dma_start

            def act_gelu(dst, src, bias=None):
                if not sim_gelu:
                    if bias is None:
                        nc.scalar.activation(dst, src, Act.Gelu)
                    else:
                        nc.scalar.activation(dst, src, Act.Gelu, bias=bias,
                                             scale=1.0)
                    return
                shp = list(dst.shape)
                y = sapool.tile(shp, F32, tag="gel_y", name="gel_y")
                if bias is None:
                    nc.scalar.activation(y[:], src, Act.Identity)
                else:
                    nc.scalar.activation(y[:], src, Act.Identity, bias=bias,
                                         scale=1.0)
                u = sapool.tile(shp, F32, tag="gel_u", name="gel_u")
                nc.vector.tensor_tensor(out=u[:], in0=y[:], in1=y[:],
                                        op=AluOp.mult)
                nc.vector.tensor_tensor(out=u[:], in0=u[:], in1=y[:],
                                        op=AluOp.mult)
                nc.vector.tensor_scalar(out=u[:], in0=u[:], scalar1=0.044715,
                                        scalar2=None, op0=AluOp.mult)
                nc.vector.tensor_tensor(out=u[:], in0=u[:], in1=y[:],
                                        op=AluOp.add)
                nc.scalar.activation(u[:], u[:], Act.Tanh, scale=0.7978845608)
                nc.vector.tensor_scalar(out=u[:], in0=u[:], scalar1=1.0,
                                        scalar2=0.5, op0=AluOp.add,
                                        op1=AluOp.mult)
                nc.vector.tensor_tensor(out=dst, in0=u[:], in1=y[:],
                                        op=AluOp.mult)

            # ---------------- constants ----------------
            ident = cpool.tile([128, 128], F16)
            from concourse.masks import make_identity
            make_identity(nc, ident[:])
            ident32 = cpool.tile([128, 128], F32)
            make_identity(nc, ident32[:])
            onesc = cpool.tile([1, 512], F16)   # K=1 matmul lhsT/rhs ones
            nc.vector.memset(onesc[:], 1.0)
            ones_k = cpool.tile([128, 1], F16)  # column-sum matmul rhs
            nc.vector.memset(ones_k[:], 1.0)
            ones15 = cpool.tile([E, 1], F16)
            nc.vector.memset(ones15[:], 1.0)
            cpk = cpool.tile([128, CPK_COLS], F16)
            dma(cpk[:], t_CPK[:])
            c_sel2 = cpk[0:2, SEL2_O:SEL2_O + 128]
            c_Rm = cpk[0:112, RM_O:RM_O + ND].rearrange(
                "p (a b) -> p a b", a=1)
            c_SelJ = cpk[0:112, SELJ_O:SELJ_O + 8]
            c_selB = cpk[0:8, SELB_O:SELB_O + 224]
            c_attLT = cpk[0:5, ATTLT_O:ATTLT_O + 128]
            c_attRT = cpk[0:5, ATTRT_O:ATTRT_O + NH * 128]
            c_Mpool = cpk[:, MPOOL_O:MPOOL_O + 112].rearrange(
                "p (t m) -> p t m", t=2)
            c_Ind2 = cpk[0:8, IND2_O:IND2_O + 256].rearrange(
                "p (t m) -> p t m", t=2)
            c_SegSel0 = cpk[0:ER, SEG_O:SEG_O + E]
            c_RepSel = cpk[0:E, REP_O:REP_O + ER]
            c_maskS = cpk[:, MASKS_O:MASKS_O + ND]
            c_M2 = cpk[:, ATTM2_O:ATTM2_O + 128].rearrange(
                "p (a b) -> p a b", a=1)
            c_M2T2 = cpk[:, ATTM2T_O:ATTM2T_O + 256].rearrange(
                "p (a b) -> p a b", a=2)
            c_fgB = cpk[:, FG_O:FG_O + D]
            c_fbB = cpk[:, FB_O:FB_O + D]
            magic_t = cpool.tile([128, 256], I32)
            nc.vector.memset(magic_t[:], 0x5f3759df)
            dumact = cpool.tile([1, 2], F32)
            nc.vector.memset(dumact[:], 0.0)

            def rsqrt_dve(dst, var_ap, eps):
                """dst = 1/sqrt(var_ap + eps), DVE-only (bit hack + 2 Newton)."""
                shp = list(dst.shape)
                p, n = shp[0], int(np.prod(shp[1:]))
                v = mpool.tile(shp, F32, tag="rsqv", bufs=2, name="rsqv")
                nc.vector.tensor_scalar(out=v[:], in0=var_ap, scalar1=float(eps),
                                        scalar2=None, op0=AluOp.add)
                ti_ = mpool.tile(shp, I32, tag="rsqt", bufs=2, name="rsqt")
                nc.vector.tensor_scalar(out=ti_[:], in0=v[:].bitcast(I32),
                                        scalar1=1, scalar2=None,
                                        op0=AluOp.logical_shift_right)
                mg = magic_t[:p].rearrange("p n -> p n")[:, :n]
                nc.vector.tensor_tensor(out=ti_[:], in0=mg.rearrange(
                    "p (a b) -> p a b", a=1) if len(shp) == 3 else mg,
                    in1=ti_[:], op=AluOp.subtract)
                y = ti_[:].bitcast(F32)
                a_ = mpool.tile(shp, F32, tag="rsqa", bufs=2, name="rsqa")
                for it in range(2):
                    nc.vector.tensor_tensor(out=a_[:], in0=v[:], in1=y,
                                            op=AluOp.mult)
                    nc.vector.tensor_tensor(out=a_[:], in0=a_[:], in1=y,
                                            op=AluOp.mult)
                    nc.vector.tensor_scalar(out=a_[:], in0=a_[:], scalar1=-0.5,
                                            scalar2=1.5, op0=AluOp.mult,
                                            op1=AluOp.add)
                    nc.vector.tensor_tensor(out=dst if it == 1 else
                                            ti_[:].bitcast(F32),
                                            in0=y, in1=a_[:], op=AluOp.mult)

            # ---------------- persistent activations ----------------
            h = hpool.tile([128, 2, D], F32)
            res1 = hpool.tile([128, 2, D], F32)
            dma(h[:], t_x0[:])

            def ln_stats_tg(src, mvs, tg):
                st = mpool.tile([128, 2, 6], F32, tag="lnst", bufs=2)
                xs = src[:, tg, :].rearrange("p (a b) -> p a b", a=2)
                for a in range(2):
                    nc.vector.bn_stats(st[:, a, :], xs[:, a, :])
                nc.vector.bn_aggr(mvs[:, tg, :], st[:])

            def ln_stats(src, eps):
                """emit stats+rsqrt for both tgs; returns (mvs, rst2)."""
                mvs = mpool.tile([128, 2, 2], F32, tag="lnmv", bufs=2)
                for tg in range(2):
                    ln_stats_tg(src, mvs, tg)
                rst2 = mpool.tile([128, 2], F32, tag="lnrs", bufs=2)
                rsqrt_dve(rst2[:], mvs[:, :, 1], eps)
                return mvs, rst2

            def ln_apply(src, dst, stats, per_tg=None):
                mvs, rst2 = stats
                for tg in range(2):
                    nc.vector.tensor_scalar(
                        out=dst[:, tg, :], in0=src[:, tg, :],
                        scalar1=mvs[:, tg, 0:1], scalar2=rst2[:, tg:tg + 1],
                        op0=AluOp.subtract, op1=AluOp.mult)
                    if per_tg is not None:
                        per_tg(tg)

            def layernorm_16(src, dst, eps, per_tg=None):
                ln_apply(src, dst, ln_stats(src, eps), per_tg)

            def transpose6_tg(src16, dst, tg):
                for c in range(KC):
                    pt = psY.tile([128, 128], F16, tag="sm")
                    nc.tensor.transpose(pt[:], src16[:, tg, c * 128:(c + 1) * 128],
                                        ident[:])
                    nc.vector.tensor_copy(dst[:, c, tg * 128:(tg + 1) * 128],
                                          pt[:])

            def transpose6(src16, dst):
                """src16 [128, 2, D] f16 -> dst [128, KC, 256] f16 (feature-major)."""
                for c in range(KC):
                    for tg in range(2):
                        pt = psY.tile([128, 128], F16, tag="sm")
                        nc.tensor.transpose(pt[:], src16[:, tg, c * 128:(c + 1) * 128],
                                            ident[:])
                        nc.vector.tensor_copy(dst[:, c, tg * 128:(tg + 1) * 128],
                                              pt[:])

            # ---------------- layers ----------------
            h_stats = ln_stats(h, 1e-12)
            for i in range(L):
                even = (i % 2 == 0)
                e = i // 2
                _mark(f'L{i}.dma')
                # ---- all weight DMAs for the layer, in consumption order ----
                b_t = bpool.tile([128, B_COLS], F16, tag="bias")
                dma(b_t[:], t_BIA[i, :, :])
                w_qk = wpool.tile([128, WQK_COLS], F16, tag="w", name="w_qk")
                dma(w_qk[:], t_WQK[i, :, :])
                w_vo = wpool.tile([128, WVO_COLS], F16, tag="w", name="w_vo")
                dma(w_vo[:], t_WVO[i, :, :])
                w_i1 = wpool.tile([128, WI_COLS], F16, tag="w", name="w_i1")
                dma(w_i1[:], t_WI1[i, :, :])
                w_i2 = wpool.tile([128, WI_COLS], F16, tag="w", name="w_i2")
                dma(w_i2[:], t_WI2[i, :, :])
                w_o1 = wpool.tile([128, WO_COLS], F16, tag="w", name="w_o1")
                dma(w_o1[:], t_WO1[i, :, :])
                w_o2 = wpool.tile([128, WO_COLS], F16, tag="w", name="w_o2")
                dma(w_o2[:], t_WO2[i, :, :])

                _mark(f'L{i}.ln1')
                # LN1 -> n1 (f16) -> n1T (stats precomputed at end of prev layer)
                n1 = apool.tile([128, 2, D], F16, tag="n1")
                n1T = apool.tile([128, KC, 256], F16, tag="n1T")
                ln_apply(h, n1, h_stats,
                         per_tg=lambda tg: transpose6_tg(n1, n1T, tg))
                # hoist Exp act-table load off the attention critical path
                if not sim_gelu:
                    nc.scalar.activation(dumact[:, 0:1], dumact[:, 0:1],
                                         Act.Exp)

                _mark(f'L{i}.qk')
                # QK^T (transposed out; bias via Act-Identity biased drains)
                qkT = apool.tile([128, 12, 256], F16, tag="bigact")
                for g in range(3):
                    grps = [psG.tile([128, 2, 512], F32, tag="grp",
                                     name=f"qkg{g}{hf}") for hf in range(2)]
                    for c in range(KC):
                        for j in range(4):
                            nc.tensor.matmul(grps[j // 2][:, j % 2, :256],
                                             w_qk[:, (g * 6 + c) * 512 + j * 128:
                                                  (g * 6 + c) * 512 + (j + 1) * 128],
                                             n1T[:, c, :],
                                             start=(c == 0), stop=(c == KC - 1))
                    for hf in range(2):
                        for sub in range(2):
                            fc = g * 4 + 2 * hf + sub
                            nc.scalar.activation(
                                qkT[:, fc, :], grps[hf][:, sub, :256],
                                Act.Identity, bias=b_t[:, BQK + fc:BQK + fc + 1],
                                scale=1.0)

                _mark(f'L{i}.v')
                # V (untransposed: [tok, dv]); bias via K=1 matmul
                V = apool.tile([128, 2, D], F16, tag="V")
                grpv = [psG.tile([128, 2, 512], F32, tag="grp",
                                 name=f"vg{tg}") for tg in range(2)]
                for j in range(4):
                    sl = slice((j % 2) * 384, (j % 2 + 1) * 384)
                    nc.tensor.matmul(grpv[j // 2][:, j % 2, :384],
                                     onesc[:1, :128],
                                     b_t[0:1, BV + sl.start:BV + sl.stop],
                                     start=True, stop=False)
                for c in range(KC):
                    for j in range(4):
                        tg, n = j // 2, j % 2
                        sl = slice(n * 384, (n + 1) * 384)
                        nc.tensor.matmul(grpv[tg][:, n, :384],
                                         n1T[:, c, tg * 128:(tg + 1) * 128],
                                         w_vo[:, WVO_V + c * 768 + sl.start:
                                              WVO_V + c * 768 + sl.stop],
                                         start=False, stop=(c == KC - 1))
                for tg in range(2):
                    nc.scalar.activation(
                        V[:, tg, :].rearrange("p (n d) -> p n d", n=2),
                        grpv[tg][:, :, :384], Act.Copy)

                _mark(f'L{i}.attn')
                # attention — scoresT[k, q] layout (keys restricted to own tg),
                # no transposes: V [tok, dv] is directly the o-matmul lhsT.
                oT = apool.tile([128, KC, 256], F16, tag="oT")
                for tg in range(2):
                    tgs = slice(tg * 128, (tg + 1) * 128)
                    expT = sapool.tile([128, NH, 128], F16, tag="expT")
                    for fc in range(KC):
                        pssc = psY.tile([128, 256], F32, tag="sm")
                        for pp in range(2):
                            hd = 2 * fc + pp
                            off = pp * 64
                            ps_h = slice(pp * 128, (pp + 1) * 128)
                            nc.tensor.matmul(pssc[:, ps_h], c_attLT[:],
                                             c_attRT[:, hd * 128:(hd + 1) * 128],
                                             start=True, stop=False)
                            nc.tensor.matmul(pssc[:, ps_h],
                                             qkT[off:off + 64, 6 + fc, tgs],
                                             qkT[off:off + 64, fc, tgs],
                                             start=False, stop=True)
                        nc.scalar.activation(
                            expT[:, 2 * fc:2 * fc + 2, :], pssc[:], Act.Exp)
                    # per-(q, head) softmax denominators via PE column sums
                    ps_rs = psY.tile([128, NH], F32, tag="sm")
                    for hd in range(NH):
                        nc.tensor.matmul(ps_rs[:, hd:hd + 1],
                                         expT[:, hd, :],
                                         ones_k[:], start=True, stop=True)
                    rinv = mpool.tile([128, NH], F32, tag="rinv", bufs=2)
                    nc.vector.reciprocal(rinv[:], ps_rs[:])
                    rinv16 = mpool.tile([128, NH], F16, tag="rinv16", bufs=2)
                    nc.vector.tensor_copy(rinv16[:], rinv[:])
                    for c in range(KC):
                        # psn[p, q] = rinv16[q, 2c + p//64] via stride-0 lhsT
                        psn = psY.tile([128, 128], F32, tag="sm")
                        for hh in range(2):
                            lhsT = rinv16[:, 2 * c + hh:2 * c + hh + 1] \
                                .to_broadcast((128, 1, 64))[:, 0, :]
                            nc.tensor.matmul(psn[hh * 64:(hh + 1) * 64, :],
                                             lhsT, ident[:],
                                             start=True, stop=True)
                        psnS = sapool.tile([128, 128], F16, tag="psnS")
                        nc.vector.tensor_copy(psnS[:], psn[:])
                        pso = psY.tile([128, 128], F32, tag="sm")
                        for hh in range(2):
                            hd = 2 * c + hh
                            nc.tensor.matmul(pso[hh * 64:(hh + 1) * 64, :],
                                             V[:, tg, hd * 64:(hd + 1) * 64],
                                             expT[:, hd, :],
                                             start=True, stop=True)
                        nc.vector.tensor_tensor(out=oT[:, c, tgs],
                                                in0=pso[:], in1=psnS[:],
                                                op=AluOp.mult)
                # hoist Gelu act-table load off the FFN critical path
                if not sim_gelu:
                    nc.scalar.activation(dumact[:, 1:2], dumact[:, 1:2],
                                         Act.Gelu)

                _mark(f'L{i}.ao')
                # AO projection + residual
                if even:
                    attnH = apool.tile([128, 2, D], F16, tag="n1")
                grpa = [psG.tile([128, 2, 512], F32, tag="grp",
                                 name=f"aog{tg}") for tg in range(2)]
                for j in range(4):
                    sl = slice((j % 2) * 384, (j % 2 + 1) * 384)
                    nc.tensor.matmul(grpa[j // 2][:, j % 2, :384],
                                     onesc[:1, :128],
                                     b_t[0:1, BAO + sl.start:BAO + sl.stop],
                                     start=True, stop=False)
                for c in range(KC):
                    for j in range(4):
                        tg, n = j // 2, j % 2
                        sl = slice(n * 384, (n + 1) * 384)
                        nc.tensor.matmul(grpa[tg][:, n, :384],
                                         oT[:, c, tg * 128:(tg + 1) * 128],
                                         w_vo[:, WVO_AO + c * 768 + sl.start:
                                              WVO_AO + c * 768 + sl.stop],
                                         start=False, stop=(c == KC - 1))
                mvs2 = mpool.tile([128, 2, 2], F32, tag="lnmv", bufs=2)
                for tg in range(2):
                    if even:
                        nc.scalar.activation(
                            attnH[:, tg, :].rearrange("p (n d) -> p n d", n=2),
                            grpa[tg][:, :, :384], Act.Copy)
                    nc.vector.tensor_tensor(
                        out=res1[:, tg, :].rearrange("p (n d) -> p n d", n=2),
                        in0=grpa[tg][:, :, :384],
                        in1=h[:, tg, :].rearrange("p (n d) -> p n d", n=2),
                        op=AluOp.add)
                    ln_stats_tg(res1, mvs2, tg)
                rst2b = mpool.tile([128, 2], F32, tag="lnrs", bufs=2)
                rsqrt_dve(rst2b[:], mvs2[:, :, 1], 1e-12)

                if even:
                    _mark(f'L{i}.moe_pool')
                    # single merged MoE pack DMA (weights for the whole tail);
                    # issued before the pooling matmuls to cover its latency
                    moepk = wmpool.tile([128, M1_COLS], F16, tag="moepk")
                    dma(moepk[:], t_MOE[e, :, :])
                    # prefetch the first two classifier weight blocks so the
                    # first emit_cls_dd calls during FFN-up don't stall
                    cw1_tiles = {}

                    def issue_cw1(dd):
                        w_ch = wcpool.tile([128, 2, KC, D2], F16, tag="wc1",
                                           name="wc1_c")
                        dma(w_ch[:], t_CW1[e, dd, :, :].rearrange(
                            "p (h c z) -> p h c z", h=2, c=KC))
                        cw1_tiles[dd] = w_ch

                    issue_cw1(0)
                    issue_cw1(1)
                    # pooled^T [128, KC, 112] (cols (d, tg*4+bl) after scatter)
                    pooledT = apool.tile([128, KC, 112], F16, tag="pooledT")
                    pview = pooledT.rearrange("p c (d g) -> p c d g", g=8)
                    for c in range(KC):
                        for tg in range(2):
                            ps = psY.tile([128, 4 * ND], F32, tag="sm")
                            nc.tensor.matmul(ps[:],
                                             attnH[:, tg, c * 128:(c + 1) * 128],
                                             c_Mpool[:, tg, :],
                                             start=True, stop=True)
                            pv = ps[:].rearrange("p (d g) -> p d g", g=4)
                            nc.vector.tensor_copy(
                                pview[:, c, :, tg * 4:tg * 4 + 4], pv)
                    c_cg = moepk[0:112, CLS_O + 0:CLS_O + D2]
                    c_cbt = moepk[0:112, CLS_O + D2:CLS_O + 2 * D2]
                    c_w2 = moepk[0:112, CLS_O + 2 * D2:CLS_O + 3 * D2]
                    c_b1 = moepk[0:112, CLS_O + 3 * D2:CLS_O + 4 * D2]
                    c_c2 = moepk[0:112, CB2_O:CB2_O + 1]
                    w_au = moepk[:, AU_O:AU_O + KC * ER].rearrange(
                        "p (c r) -> p c r", r=ER)
                    b_lup = moepk[0:1, LUP_O:LUP_O + ER]
                    w_ad = moepk[:, AD_O:AD_O + HC * ER].rearrange(
                        "p (c r) -> p c r", r=ER)
                    w_cb = moepk[0:ER, CB_O:CB_O + ER]
                    w_bdd = moepk[:, BDD_O:BDD_O + KC * ER].rearrange(
                        "p (c r) -> p c r", r=ER)
                    w_gram = moepk[0:ER, GR_O:GR_O + ER]
                    w_bdm = moepk[0:ER, BDM_O:BDM_O + E]
                    w_bdf = moepk[0:ER, BDF_O:BDF_O + D]
                    psz = psY.tile([112, D2], F32, tag="zacc", bufs=1)
                    cls_state = {}

                    def emit_cls_dd(dd):
                        w_ch = cw1_tiles.pop(dd)
                        if dd + 2 <= 6:
                            issue_cw1(dd + 2)
                        for dh in range(2):
                            d = 2 * dd + dh
                            psd_ = psY.tile([8, D2], F32, tag="sm")
                            for c in range(KC):
                                nc.tensor.matmul(psd_[:],
                                                 pooledT[:, c, d * 8:(d + 1) * 8],
                                                 w_ch[:, dh, c, :],
                                                 start=(c == 0), stop=(c == KC - 1))
                            zd = sapool.tile([8, D2], F16, tag="zd")
                            nc.scalar.activation(zd[:], psd_[:], Act.Copy)
                            nc.tensor.matmul(psz[:],
                                             c_selB[:, 104 - 8 * d:216 - 8 * d],
                                             zd[:], start=(d == 0),
                                             stop=(d == ND - 1))

                def emit_cls_finish():
                    zsb = spool.tile([112, D2], F32, tag="zsb")
                    nc.vector.tensor_tensor(out=zsb[:], in0=psz[:], in1=c_b1[:],
                                            op=AluOp.add)
                    zst = mpool.tile([112, 6], F32, tag="lnstz")
                    nc.vector.bn_stats(zst[:], zsb[:])
                    zmv = mpool.tile([112, 2], F32, tag="lnmvz")
                    nc.vector.bn_aggr(zmv[:], zst[:])
                    zrstd = mpool.tile([112, 1], F32, tag="zrstd")
                    rsqrt_dve(zrstd[:], zmv[:, 1:2], 1e-5)
                    zn = spool.tile([112, D2], F32, tag="zn")
                    nc.vector.tensor_scalar(out=zn[:], in0=zsb[:],
                                            scalar1=zmv[:, 0:1], scalar2=zrstd[:],
                                            op0=AluOp.subtract, op1=AluOp.mult)
                    nc.vector.tensor_tensor(out=zn[:], in0=zn[:], in1=c_cg[:],
                                            op=AluOp.mult)
                    nc.vector.tensor_tensor(out=zn[:], in0=zn[:], in1=c_cbt[:],
                                            op=AluOp.add)
                    zg = spool.tile([112, D2], F32, tag="zg")
                    act_gelu(zg[:], zn[:])
                    nc.vector.tensor_tensor(out=zg[:], in0=zg[:], in1=c_w2[:],
                                            op=AluOp.mult)
                    ppre = mpool.tile([112, 1], F32, tag="ppre")
                    nc.vector.reduce_sum(ppre[:], zg[:], axis=mybir.AxisListType.X)
                    nc.vector.tensor_tensor(out=ppre[:], in0=ppre[:], in1=c_c2[:],
                                            op=AluOp.add)
                    rp = mpool.tile([112, 1, ND], F16, tag="rp")
                    nc.vector.tensor_tensor(out=rp[:], in0=c_Rm[:],
                                            in1=ppre[:].to_broadcast((112, 1, ND)),
                                            op=AluOp.mult)
                    psda = psY.tile([8, ND], F32, tag="sm")
                    nc.tensor.matmul(psda[:], c_SelJ[:], rp[:, 0, :],
                                     start=True, stop=True)
                    da = mpool.tile([8, ND], F16, tag="da")
                    nc.vector.tensor_scalar(out=da[:], in0=psda[:], scalar1=0.0,
                                            scalar2=None, op0=AluOp.is_gt)

                    _mark(f'L{i}.moe_rout')
                    # routing weights w [128, tg, E] f32
                    w_rt = spool.tile([128, 2, E], F32, tag="wrt")
                    nact = mpool.tile([128, 2], F32, tag="nact")
                    for tg in range(2):
                        psd = psY.tile([128, ND], F32, tag="sm")
                        nc.tensor.matmul(psd[:], c_Ind2[:, tg, :], da[:],
                                         start=True, stop=True)
                        nc.vector.tensor_tensor(out=w_rt[:, tg, 0:ND], in0=psd[:],
                                                in1=c_maskS[:], op=AluOp.mult)
                        nc.vector.reduce_sum(nact[:, tg:tg + 1], w_rt[:, tg, 0:ND],
                                             axis=mybir.AxisListType.X)
                        nc.vector.tensor_scalar(out=nact[:, tg:tg + 1],
                                                in0=nact[:, tg:tg + 1],
                                                scalar1=1.0, scalar2=None,
                                                op0=AluOp.add)
                    rnact = mpool.tile([128, 2], F32, tag="rnact")
                    nc.vector.reciprocal(rnact[:], nact[:])
                    for tg in range(2):
                        nc.vector.tensor_scalar(out=w_rt[:, tg, 0:ND],
                                                in0=w_rt[:, tg, 0:ND],
                                                scalar1=rnact[:, tg:tg + 1],
                                                scalar2=None, op0=AluOp.mult)
                        nc.vector.tensor_copy(w_rt[:, tg, ND:E], rnact[:, tg:tg + 1])
                    wT = mpool.tile([E, 256], F32, tag="wT")
                    for tg in range(2):
                        pt = psY.tile([E, 128], F32, tag="sm")
                        nc.tensor.transpose(pt[:], w_rt[:, tg, :], ident32[:])
                        nc.vector.tensor_copy(wT[:, tg * 128:(tg + 1) * 128], pt[:])
                    cls_state['wT'] = wT

                _mark(f'L{i}.ln2')
                # LN2 -> n2 -> n2T (stats computed during AO drains)
                n2 = apool.tile([128, 2, D], F16, tag="n2")
                n2T = apool.tile([128, KC, 256], F16, tag="n2T")
                ln_apply(res1, n2, (mvs2, rst2b),
                         per_tg=lambda tg: transpose6_tg(n2, n2T, tg))

                _mark(f'L{i}.up')
                # FFN up (transposed out) + gelu with folded bias
                interT = apool.tile([128, HC, 256], F16, tag="bigact")
                for g in range(6):
                    w_i = w_i1 if g < 3 else w_i2
                    gg = g % 3
                    grps = [psG.tile([128, 2, 512], F32, tag="grp",
                                     name=f"upg{g}{hf}") for hf in range(2)]
                    for c in range(KC):
                        for j in range(4):
                            nc.tensor.matmul(grps[j // 2][:, j % 2, :256],
                                             w_i[:, (gg * 6 + c) * 512 + j * 128:
                                                 (gg * 6 + c) * 512 + (j + 1) * 128],
                                             n2T[:, c, :],
                                             start=(c == 0), stop=(c == KC - 1))
                    for hf in range(2):
                        for sub in range(2):
                            hc = g * 4 + 2 * hf + sub
                            act_gelu(interT[:, hc, :], grps[hf][:, sub, :256],
                                     bias=b_t[:, BI + hc:BI + hc + 1])
                    if even:
                        _mark(f'L{i}.moe_cls')
                        if g < 5:
                            emit_cls_dd(g)
                        else:
                            emit_cls_dd(5)
                            emit_cls_dd(6)
                        _mark(f'L{i}.up')

                if even:
                    _mark(f'L{i}.moe_lora')
                    # LoRA rails
                    ps = psY.tile([ER, 256], F32, tag="sm")
                    nc.tensor.matmul(ps[:], b_lup, onesc[:1, :256],
                                     start=True, stop=False)
                    for c in range(KC):
                        nc.tensor.matmul(ps[:], w_au[:, c, :], n2T[:, c, :],
                                         start=False, stop=(c == KC - 1))
                    lup_rT = spool.tile([ER, 256], F16, tag="luprT")
                    nc.vector.tensor_copy(lup_rT[:], ps[:])

                    ps2 = psY.tile([ER, 256], F32, tag="sm")
                    nc.tensor.matmul(ps2[:], w_cb, lup_rT[:], start=True, stop=False)
                    for c in range(HC):
                        nc.tensor.matmul(ps2[:], w_ad[:, c, :], interT[:, c, :],
                                         start=False, stop=(c == HC - 1))
                    ldr16 = spool.tile([ER, 256], F16, tag="ldr16")
                    nc.vector.tensor_copy(ldr16[:], ps2[:])
                    ldr32 = spool.tile([ER, 256], F32, tag="ldr32")
                    nc.vector.tensor_copy(ldr32[:], ps2[:])

                _mark(f'L{i}.down')
                # FFN down
                if even:
                    base = apool.tile([128, 2, D], F16, tag="base")
                grpd = [psG.tile([128, 2, 512], F32, tag="grp",
                                 name=f"dng{tg}") for tg in range(2)]
                for j in range(4):
                    sl = slice((j % 2) * 384, (j % 2 + 1) * 384)
                    nc.tensor.matmul(grpd[j // 2][:, j % 2, :384],
                                     onesc[:1, :128],
                                     b_t[0:1, BO + sl.start:BO + sl.stop],
                                     start=True, stop=False)
                for c in range(HC):
                    w_o = w_o1 if c < 12 else w_o2
                    cc = c % 12
                    for j in range(4):
                        tg, n = j // 2, j % 2
                        sl = slice(n * 384, (n + 1) * 384)
                        nc.tensor.matmul(grpd[tg][:, n, :384],
                                         interT[:, c, tg * 128:(tg + 1) * 128],
                                         w_o[:, cc * 768 + sl.start:
                                             cc * 768 + sl.stop],
                                         start=False, stop=(c == HC - 1))
                if even:
                    _mark(f'L{i}.moe_cls2')
                    emit_cls_finish()
                for tg in range(2):
                    if not even:
                        nc.vector.tensor_tensor(
                            out=h[:, tg, :].rearrange("p (n d) -> p n d", n=2),
                            in0=grpd[tg][:, :, :384],
                            in1=res1[:, tg, :].rearrange("p (n d) -> p n d", n=2),
                            op=AluOp.add)
                    else:
                        nc.vector.tensor_copy(
                            base[:, tg, :].rearrange("p (n d) -> p n d", n=2),
                            grpd[tg][:, :, :384])

                if not even:
                    h_stats = ln_stats(h, 1e-12)
                    continue

                _mark(f'L{i}.moe_stats')
                # ================= MoE / classifier tail =================
                # base stats (mu, ms = var + mu^2), transposed to rows
                mums = mpool.tile([128, 2, 2], F32, tag="mums")   # [:, tg, (mu,ms)]
                for tg in range(2):
                    st = mpool.tile([128, 3, 6], F32, tag="lnst", bufs=2)
                    xs = base[:, tg, :].rearrange("p (a b) -> p a b", a=3)
                    for a in range(3):
                        nc.vector.bn_stats(st[:, a, :], xs[:, a, :])
                    mv = mpool.tile([128, 2], F32, tag="lnmv", bufs=2)
                    nc.vector.bn_aggr(mv[:], st[:])
                    nc.vector.tensor_copy(mums[:, tg, 0:1], mv[:, 0:1])
                    # ms = var + mu^2
                    musq = mpool.tile([128, 1], F32, tag="musq")
                    nc.vector.tensor_tensor(out=musq[:], in0=mv[:, 0:1],
                                            in1=mv[:, 0:1], op=AluOp.mult)
                    nc.vector.tensor_tensor(out=mums[:, tg, 1:2], in0=mv[:, 1:2],
                                            in1=musq[:], op=AluOp.add)
                muT = mpool.tile([1, 256], F32, tag="muT")
                msT = mpool.tile([1, 256], F32, tag="msT")
                for tg in range(2):
                    pt = psY.tile([1, 128], F32, tag="sm")
                    nc.tensor.transpose(pt[:], mums[:, tg, 0:1], ident32[:])
                    nc.vector.tensor_copy(muT[:, tg * 128:(tg + 1) * 128], pt[:])
                    pt2 = psY.tile([1, 128], F32, tag="sm")
                    nc.tensor.transpose(pt2[:], mums[:, tg, 1:2], ident32[:])
                    nc.vector.tensor_copy(msT[:, tg * 128:(tg + 1) * 128], pt2[:])

                baseT = apool.tile([128, KC, 256], F16, tag="n1T")
                transpose6(base, baseT)

                # (cls finish + routing emitted during down via emit_cls_finish)
                _mark(f'L{i}.moe_g')
                # G^T (cross term, x2 folded in BdfD) and quad term
                psg = psY.tile([ER, 256], F32, tag="sm")
                for c in range(KC):
                    nc.tensor.matmul(psg[:], w_bdd[:, c, :], baseT[:, c, :],
                                     start=(c == 0), stop=(c == KC - 1))
                Pcross = spool.tile([ER, 256], F16, tag="pcross")
                nc.vector.tensor_tensor(out=Pcross[:], in0=psg[:],
                                        in1=ldr32[:], op=AluOp.mult)

                psq = psY.tile([ER, 256], F32, tag="sm")
                nc.tensor.matmul(psq[:], w_gram, ldr16[:],
                                 start=True, stop=True)
                Pquad = spool.tile([ER, 256], F16, tag="pquad")
                nc.vector.tensor_tensor(out=Pquad[:], in0=psq[:], in1=ldr32[:],
                                        op=AluOp.mult)

                # mu_e^T [E, 256]
                muT16 = mpool.tile([1, 256], F16, tag="muT16")
                nc.vector.tensor_copy(muT16[:], muT[:])
                msT16 = mpool.tile([1, 256], F16, tag="msT16")
                nc.vector.tensor_copy(msT16[:], msT[:])
                psmu = psY.tile([E, 256], F32, tag="sm")
                nc.tensor.matmul(psmu[:], w_bdm, ldr16[:], start=True, stop=False)
                nc.tensor.matmul(psmu[:], onesc[:1, :E], muT16[:],
                                 start=False, stop=True)
                muE = mpool.tile([E, 256], F32, tag="muE")
                nc.vector.tensor_copy(muE[:], psmu[:])

                # ms^T then var, rho
                psms = psY.tile([E, 256], F32, tag="sm")
                nc.tensor.matmul(psms[:], c_SegSel0, Pcross[:],
                                 start=True, stop=False)
                nc.tensor.matmul(psms[:], c_SegSel0, Pquad[:],
                                 start=False, stop=False)
                nc.tensor.matmul(psms[:], onesc[:1, :E], msT16[:],
                                 start=False, stop=True)
                musqE = mpool.tile([E, 256], F32, tag="musqE")
                nc.vector.tensor_tensor(out=musqE[:], in0=muE[:], in1=muE[:],
                                        op=AluOp.mult)
                varE = mpool.tile([E, 256], F32, tag="varE")
                nc.vector.tensor_tensor(out=varE[:], in0=psms[:], in1=musqE[:],
                                        op=AluOp.subtract)
                rho = mpool.tile([E, 256], F32, tag="rho")
                rsqrt_dve(rho[:], varE[:], 1e-5)

                # s_e = w * rho ; pack [sE | sE*muE] -> column sums -> scal/off
                packSO = mpool.tile([E, 512], F16, tag="packSO")
                wT = cls_state['wT']
                nc.vector.tensor_tensor(out=packSO[:, 0:256], in0=wT[:], in1=rho[:],
                                        op=AluOp.mult)
                nc.vector.tensor_tensor(out=packSO[:, 256:512],
                                        in0=packSO[:, 0:256], in1=muE[:],
                                        op=AluOp.mult)
                psso = psY.tile([1, 512], F32, tag="sm")
                nc.tensor.matmul(psso[:], ones15[:], packSO[:],
                                 start=True, stop=True)
                soT = mpool.tile([1, 512], F32, tag="soT")
                nc.vector.tensor_copy(soT[:], psso[:])
                scal = mpool.tile([128, 2], F32, tag="scal")
                off = mpool.tile([128, 2], F32, tag="off")
                for tg in range(2):
                    pt = psY.tile([128, 1], F32, tag="sm")
                    nc.tensor.transpose(pt[:], soT[:, tg * 128:(tg + 1) * 128],
                                        ident32[:1, :1])
                    nc.vector.tensor_copy(scal[:, tg:tg + 1], pt[:])
                    pt2 = psY.tile([128, 1], F32, tag="sm")
                    nc.tensor.transpose(pt2[:],
                                        soT[:, 256 + tg * 128:256 + (tg + 1) * 128],
                                        ident32[:1, :1])
                    nc.vector.tensor_copy(off[:, tg:tg + 1], pt2[:])

                # ls^T = ldown_r^T * repeat(s_e)
                psrep = psY.tile([ER, 256], F32, tag="sm")
                nc.tensor.matmul(psrep[:], c_RepSel, packSO[:, 0:256],
                                 start=True, stop=True)
                srep = mpool.tile([ER, 256], F32, tag="srep")
                nc.vector.tensor_copy(srep[:], psrep[:])
                lsT = spool.tile([ER, 256], F16, tag="lsT")
                nc.vector.tensor_tensor(out=lsT[:], in0=srep[:], in1=ldr32[:],
                                        op=AluOp.mult)

                _mark(f'L{i}.moe_fin')
                # final: h = (res1 - off) + (base*scal + ldown_mix)
                grpf = [psG.tile([128, 2, 512], F32, tag="grp",
                                 name=f"fing{tg}") for tg in range(2)]
                for j in range(4):
                    tg, n = j // 2, j % 2
                    sl = slice(n * 384, (n + 1) * 384)
                    nc.tensor.matmul(grpf[tg][:, n, :384],
                                     lsT[:, tg * 128:(tg + 1) * 128],
                                     w_bdf[:, sl], start=True, stop=True)
                for j in range(4):
                    tg, n = j // 2, j % 2
                    sl = slice(n * 384, (n + 1) * 384)
                    tmp = spool.tile([128, 384], F32, tag="ffn_tmp")
                    nc.vector.scalar_tensor_tensor(
                        out=tmp[:], in0=base[:, tg, sl],
                        scalar=scal[:, tg:tg + 1],
                        in1=grpf[tg][:, n, :384], op0=AluOp.mult, op1=AluOp.add)
                    nc.vector.scalar_tensor_tensor(
                        out=h[:, tg, sl], in0=res1[:, tg, sl],
                        scalar=off[:, tg:tg + 1], in1=tmp[:],
                        op0=AluOp.subtract, op1=AluOp.add)
                h_stats = ln_stats(h, 1e-12)

            _mark('final_ln')
            # ---------------- final LN ----------------
            hf = apool.tile([128, 2, D], F32, tag="base")
            ln_apply(h, hf, h_stats)   # writes f32 since tile dtype f32
            ot = apool.tile([128, 2, D], F32, tag="bigact",
                            name="ot")
            for tg in range(2):
                nc.vector.tensor_tensor(out=ot[:, tg, :], in0=hf[:, tg, :],
                                        in1=c_fgB, op=AluOp.mult)
                nc.vector.tensor_tensor(out=ot[:, tg, :], in0=ot[:, tg, :],
                                        in1=c_fbB, op=AluOp.add)
            dma(t_out[:], ot[:])

    nc.compile()
    return nc


_CACHE = {}


def _get_nc(sim_gelu=False):
    key = ("nc", sim_gelu)
    if key not in _CACHE:
        _CACHE[key] = _build(sim_gelu)
    return _CACHE[key]


def kernel(**inputs):
    inputs = {k: np.asarray(v) for k, v in inputs.items()}
    P = _prep(inputs)
    shards = _shard_x0(inputs)
    nc = _get_nc()
    base_map = {k: np.ascontiguousarray(v) for k, v in P.items()}
    in_maps = []
    for c in range(NC):
        m = dict(base_map)
        m["x0"] = np.ascontiguousarray(shards[c])
        in_maps.append(m)
    res = bass_utils.run_bass_kernel_spmd(nc, in_maps, core_ids=list(range(NC)))
    out = np.zeros((B, S, D), f32)
    for c in range(NC):
        oc = res.results[c]["out"].transpose(1, 0, 2).reshape(NT, D)
        for bl in range(BPC):
            out[c * BPC + bl] = oc[bl * TS: bl * TS + S]
    return out


# revision 21
# speedup vs baseline: 1.0955x; 1.0004x over previous
"""Trainium2 Bass kernel for nn_MedicalVisionTransformer (MoE-LoRA ViT).

Strategy: data-parallel over batch (8 cores x 8 batch items). Each core holds
its 256-token (8 batches x 32 slots: 30 real + 2 pad) residual stream in SBUF
for all 12 layers; only weights stream from HBM in fp16. MoE LoRA experts are
collapsed algebraically (rank-8 C matrices; per-expert LayerNorm folded into
per-token scalars via B_down Gram matrices) so no [B,S,E,H]/[B,S,E,D] tensor
is ever materialized.

v2: weights stream as a few large per-layer DMAs (SP sequencer / HWDGE were
instruction-count bound at ~100 DMAs/layer); per-partition biases (qk, FFN-up)
are folded into Activation-engine biased copies instead of K=1 matmuls; the
Exp/Gelu activation-table switches are hoisted off the critical path with
dummy ops.
"""

import sys

sys.path.insert(0, "/opt/trn_rl_repo")

import numpy as np

import concourse.bass as bass
import concourse.mybir as mybir
import concourse.tile as tile
from concourse import bacc
from concourse import bass_utils

f32 = np.float32
F32 = mybir.dt.float32
F16 = mybir.dt.float16
F32R = mybir.dt.float32r
I32 = mybir.dt.int32

B, SR, D, H, L, NH, ND, E, RK = 64, 29, 768, 3072, 12, 12, 14, 15, 8
S = SR + 1
SCALE = f32(16.0 / 8.0)
NE = L // 2
DH = D // NH
NC = 8
BPC = B // NC          # batches per core
TS = 32                # token slot per batch (30 real + 2 pad)
NT = BPC * TS          # 256 tokens per core
D2 = D // 2            # 384
ER = E * RK            # 120
KC = D // 128          # 6 feature chunks
HC = H // 128          # 24 hidden chunks

AluOp = mybir.AluOpType
Act = mybir.ActivationFunctionType

_PHASES = []   # (label, first_instruction_id) markers for profiling

# ---- packed weight block column offsets (f16 cols) ----
WQK_COLS = 9216      # 18 blocks of 512  (g*6+c)
WVO_V = 0            # 6 blocks of 768
WVO_AO = 4608        # 6 blocks of 768
WVO_COLS = 9216
WI_COLS = 9216       # 18 blocks of 512 per half ((g%3)*6+c)
WO_COLS = 9216       # 12 blocks of 768 per half
BQK = 0              # [128,12] per-chunk qk bias
BI = 12              # [128,24] per-chunk FFN-up bias
BV = 40              # row-0 strips
BAO = 808
BO = 1576
B_COLS = 2344
# moe pack offsets
AU_O, AD_O, BDD_O = 0, 720, 3600
CB_O, GR_O, BDM_O, BDF_O = 4320, 4440, 4560, 4575
CLS_O, CB2_O, LUP_O = 5343, 6879, 6880
M1_COLS = 7000
# const pack (f16) offsets
SEL2_O, RM_O, SELJ_O, SELB_O = 0, 128, 142, 150
ATTLT_O, ATTRT_O, MPOOL_O, IND2_O = 374, 502, 2038, 2150
SEG_O, REP_O = 2406, 2421
MASKS_O, FG_O, FB_O = 2560, 2574, 3342
ATTM2_O = 4110
ATTM2T_O = 4240
CPK_COLS = 4500


# ----------------------------------------------------------------------------
# Host-side weight preparation (pure numpy; done once per kernel() call)
# ----------------------------------------------------------------------------

def _prep(inputs):
    P = {}
    qs = f32(1.0 / np.sqrt(DH))

    WQK = np.zeros((L, 128, WQK_COLS), np.float16)
    WVO = np.zeros((L, 128, WVO_COLS), np.float16)
    WI1 = np.zeros((L, 128, WI_COLS), np.float16)
    WI2 = np.zeros((L, 128, WI_COLS), np.float16)
    WO1 = np.zeros((L, 128, WO_COLS), np.float16)
    WO2 = np.zeros((L, 128, WO_COLS), np.float16)
    BIA = np.zeros((L, 128, B_COLS), np.float16)

    for i in range(L):
        g1, b1 = inputs['ln1_g'][i], inputs['ln1_b'][i]
        g2, b2 = inputs['ln2_g'][i], inputs['ln2_b'][i]
        WqT = (inputs['Wq'][i] * g1[None, :]).T * qs      # [in, out]
        WkT = (inputs['Wk'][i] * g1[None, :]).T
        bq = (b1 @ inputs['Wq'][i].T + inputs['bq'][i]) * qs
        bk = b1 @ inputs['Wk'][i].T + inputs['bk'][i]
        qk = np.concatenate([WqT, WkT], axis=1)           # [768, 1536]
        for g in range(3):
            for c in range(KC):
                WQK[i, :, (g * 6 + c) * 512:(g * 6 + c + 1) * 512] = \
                    qk[c * 128:(c + 1) * 128,
                       g * 512:(g + 1) * 512].astype(np.float16)
        bqk_full = np.concatenate([bq, bk]).astype(np.float16)   # [1536]
        BIA[i, :, BQK:BQK + 12] = bqk_full.reshape(12, 128).T
        WvT = (inputs['Wv'][i] * g1[None, :]).T
        WaoT = inputs['Wao'][i].T
        for c in range(KC):
            WVO[i, :, WVO_V + c * 768:WVO_V + (c + 1) * 768] = \
                WvT[c * 128:(c + 1) * 128].astype(np.float16)
            WVO[i, :, WVO_AO + c * 768:WVO_AO + (c + 1) * 768] = \
                WaoT[c * 128:(c + 1) * 128].astype(np.float16)
        BIA[i, 0, BV:BV + D] = (b1 @ inputs['Wv'][i].T
                                + inputs['bv'][i]).astype(np.float16)
        BIA[i, 0, BAO:BAO + D] = inputs['bao'][i].astype(np.float16)
        WiT = (inputs['Wi'][i] * g2[None, :]).T           # [768, 3072]
        for g in range(6):
            dst = WI1 if g < 3 else WI2
            gg = g % 3
            for c in range(KC):
                dst[i, :, (gg * 6 + c) * 512:(gg * 6 + c + 1) * 512] = \
                    WiT[c * 128:(c + 1) * 128,
                        g * 512:(g + 1) * 512].astype(np.float16)
        bi_full = (b2 @ inputs['Wi'][i].T + inputs['bi'][i]).astype(np.float16)
        BIA[i, :, BI:BI + 24] = bi_full.reshape(24, 128).T
        WoT = inputs['Wo'][i].T                            # [3072, 768]
        for c in range(HC):
            dst = WO1 if c < 12 else WO2
            cc = c % 12
            dst[i, :, cc * 768:(cc + 1) * 768] = \
                WoT[c * 128:(c + 1) * 128].astype(np.float16)
        BIA[i, 0, BO:BO + D] = inputs['bo'][i].astype(np.float16)

    P.update(WQK=WQK, WVO=WVO, WI1=WI1, WI2=WI2, WO1=WO1, WO2=WO2, BIA=BIA)

    # MoE / classifier packed tensors
    MOE = np.zeros((NE, 128, M1_COLS), np.float16)
    CW1 = np.zeros((NE, 7, 128, 2 * KC * D2), np.float16)

    for e in range(NE):
        i = 2 * e
        g2, b2 = inputs['ln2_g'][i], inputs['ln2_b'][i]
        Au = inputs['A_up'][e]; Bu = inputs['B_up'][e]
        Ad = inputs['A_down'][e]; Bd = inputs['B_down'][e]
        AuTf = np.concatenate([(Au[ee] * g2[None, :]).T for ee in range(E)], axis=1)
        for c in range(KC):
            MOE[e, :, AU_O + c * ER:AU_O + (c + 1) * ER] = \
                AuTf[c * 128:(c + 1) * 128].astype(np.float16)
        MOE[e, 0, LUP_O:LUP_O + ER] = np.concatenate(
            [b2 @ Au[ee].T for ee in range(E)]).astype(np.float16)
        AdTf = np.concatenate([Ad[ee].T for ee in range(E)], axis=1)   # [H, 120]
        for c in range(HC):
            MOE[e, :, AD_O + c * ER:AD_O + (c + 1) * ER] = \
                AdTf[c * 128:(c + 1) * 128].astype(np.float16)
        for ee in range(E):
            Cm = Ad[ee] @ Bu[ee]                                        # [r, r']
            MOE[e, ee * RK:(ee + 1) * RK,
                CB_O + ee * RK:CB_O + (ee + 1) * RK] = \
                (SCALE * Cm.T).astype(np.float16)
        Bdf = np.concatenate([Bd[ee].T for ee in range(E)], axis=0) * SCALE  # [120, D]
        MOE[e, :ER, BDF_O:BDF_O + D] = Bdf.astype(np.float16)
        BdfDf = (2.0 * Bdf.T / f32(D))                                  # [D, 120]
        for c in range(KC):
            MOE[e, :, BDD_O + c * ER:BDD_O + (c + 1) * ER] = \
                BdfDf[c * 128:(c + 1) * 128].astype(np.float16)
        Bdm = Bdf.mean(axis=1)                                          # [120]
        for ee in range(E):
            MOE[e, ee * RK:(ee + 1) * RK, BDM_O + ee] = \
                Bdm[ee * RK:(ee + 1) * RK].astype(np.float16)
            sl = slice(ee * RK, (ee + 1) * RK)
            MOE[e, ee * RK:(ee + 1) * RK, GR_O + ee * RK:GR_O + (ee + 1) * RK] = \
                ((Bdf[sl] @ Bdf[sl].T) / f32(D)).astype(np.float16)
        # classifier weights; cW1 in 7 blocks of 2 diseases
        cW1e = inputs['cW1'][e]
        for d in range(ND):
            W1T = cW1e[d].T                                             # [768, 384]
            dd, dh = d // 2, d % 2
            for c in range(KC):
                CW1[e, dd, :, (dh * KC + c) * D2:(dh * KC + c + 1) * D2] = \
                    W1T[c * 128:(c + 1) * 128].astype(np.float16)
        # clsPack rows (d*8+j): g|b|w2|bias, then cb2 col
        for d in range(ND):
            for j in range(8):
                r = d * 8 + j
                MOE[e, r, CLS_O + 0:CLS_O + D2] = inputs['clng'][e][d]
                MOE[e, r, CLS_O + D2:CLS_O + 2 * D2] = inputs['clnb'][e][d]
                MOE[e, r, CLS_O + 2 * D2:CLS_O + 3 * D2] = inputs['cW2'][e][d]
                MOE[e, r, CLS_O + 3 * D2:CLS_O + 4 * D2] = inputs['cb1'][e][d]
                MOE[e, r, CB2_O] = inputs['cb2'][e][d]

    P.update(MOE=MOE, CW1=CW1)

    # ---- constant packs ----
    CPK = np.zeros((128, CPK_COLS), np.float16)

    CPK[0, SEL2_O:SEL2_O + 64] = 1.0
    CPK[1, SEL2_O + 64:SEL2_O + 128] = 1.0
    for d in range(ND):
        for j in range(8):
            CPK[d * 8 + j, RM_O + d] = 1.0
            CPK[d * 8 + j, SELJ_O + j] = 1.0
    for j in range(8):
        CPK[j, SELB_O + 104 + j] = 1.0

    ks = np.arange(128)
    CPK[0, ATTLT_O:ATTLT_O + 128] = 1.0
    qreal = (ks % TS < S).astype(np.float16)
    for h in range(NH):
        CPK[0, ATTRT_O + h * 128:ATTRT_O + (h + 1) * 128] = -30000.0 * qreal
    for j in range(4):
        CPK[1 + j, ATTLT_O:ATTLT_O + 128] = \
            ((ks // TS == j) & (ks % TS < S)).astype(np.float16)
        blk = ((ks // TS == j) & (ks % TS < S)).astype(np.float16) * 30000.0
        for h in range(NH):
            CPK[1 + j, ATTRT_O + h * 128:ATTRT_O + (h + 1) * 128] = blk

    mask = inputs['mask']; cnt = mask.sum(axis=0)
    for tg in range(2):
        for d in range(ND):
            for bl in range(4):
                col = d * 4 + bl
                CPK[bl * TS + 1: bl * TS + 1 + SR,
                    MPOOL_O + tg * 56 + col] = (mask[:, d] / cnt[d]).astype(np.float16)
    for tg in range(2):
        for bl in range(4):
            j = tg * 4 + bl
            CPK[j, IND2_O + tg * 128 + bl * TS:IND2_O + tg * 128 + (bl + 1) * TS] = 1.0
    for ee in range(E):
        CPK[ee * RK:(ee + 1) * RK, SEG_O + ee] = 1.0
        CPK[ee, REP_O + ee * RK:REP_O + (ee + 1) * RK] = 1.0

    qq, kk = np.meshgrid(np.arange(128), np.arange(128), indexing='ij')
    CPK[:, ATTM2_O:ATTM2_O + 128] = (((qq // TS) == (kk // TS))
                                     & ((kk % TS) < S)).astype(np.float16)
    m2t = (((qq // TS) == (kk // TS)) & ((qq % TS) < S)).astype(np.float16)
    CPK[:, ATTM2T_O:ATTM2T_O + 128] = m2t
    CPK[:, ATTM2T_O + 128:ATTM2T_O + 256] = m2t

    maskS = np.zeros((TS, ND), f32)
    maskS[1:1 + SR] = mask
    CPK[:, MASKS_O:MASKS_O + ND] = np.tile(maskS, (4, 1)).astype(np.float16)
    CPK[:, FG_O:FG_O + D] = np.tile(inputs['fg'][None, :],
                                    (128, 1)).astype(np.float16)
    CPK[:, FB_O:FB_O + D] = np.tile(inputs['fb'][None, :],
                                    (128, 1)).astype(np.float16)

    P.update(CPK=CPK)
    return P


def _shard_x0(inputs):
    """Per-core [128, 2, 768] initial residual streams (token = tg*128+p)."""
    cls = np.asarray(inputs['cls_token'][0, 0], f32)
    rf = np.asarray(inputs['region_features'], f32)
    shards = []
    for c in range(NC):
        x0 = np.zeros((NT, D), f32)
        for bl in range(BPC):
            b = c * BPC + bl
            x0[bl * TS] = cls
            x0[bl * TS + 1: bl * TS + 1 + SR] = rf[b]
        shards.append(np.ascontiguousarray(
            x0.reshape(2, 128, D).transpose(1, 0, 2)))
    return shards


# ----------------------------------------------------------------------------
# Bass/Tile program
# ----------------------------------------------------------------------------

def _build(sim_gelu=False):
    nc = bacc.Bacc("TRN2", target_bir_lowering=False, debug=False)
    _PHASES.clear()

    def _mark(label):
        _PHASES.append((label, nc.next_id()))

    def din(name, shape, dt):
        return nc.dram_tensor(name, list(shape), dt, kind="ExternalInput")

    t_x0 = din("x0", (128, 2, D), F32)
    t_WQK = din("WQK", (L, 128, WQK_COLS), F16)
    t_WVO = din("WVO", (L, 128, WVO_COLS), F16)
    t_WI1 = din("WI1", (L, 128, WI_COLS), F16)
    t_WI2 = din("WI2", (L, 128, WI_COLS), F16)
    t_WO1 = din("WO1", (L, 128, WO_COLS), F16)
    t_WO2 = din("WO2", (L, 128, WO_COLS), F16)
    t_BIA = din("BIA", (L, 128, B_COLS), F16)
    t_MOE = din("MOE", (NE, 128, M1_COLS), F16)
    t_CW1 = din("CW1", (NE, 7, 128, 2 * KC * D2), F16)
    t_CPK = din("CPK", (128, CPK_COLS), F16)
    t_out = nc.dram_tensor("out", [128, 2, D], F32, kind="ExternalOutput")

    with tile.TileContext(nc) as tc:
        with (
            tc.tile_pool(name="const", bufs=1) as cpool,
            tc.tile_pool(name="resid", bufs=1) as hpool,
            tc.tile_pool(name="wstream", bufs=3) as wpool,
            tc.tile_pool(name="wbias", bufs=2) as bpool,
            tc.tile_pool(name="wmoe", bufs=1) as wmpool,
            tc.tile_pool(name="wcls", bufs=2) as wcpool,
            tc.tile_pool(name="acts", bufs=1) as apool,
            tc.tile_pool(name="scrA", bufs=3) as sapool,
            tc.tile_pool(name="scrB", bufs=1) as spool,
            tc.tile_pool(name="small", bufs=1) as mpool,
            tc.tile_pool(name="psG", bufs=2, space="PSUM") as psG,
            tc.tile_pool(name="psY", bufs=3, space="PSUM") as psY,
        ):
            dma = nc.sync.dma_start

            def act_gelu(dst, src, bias=None):
                if not sim_gelu:
                    if bias is None:
                        nc.scalar.activation(dst, src, Act.Gelu)
                    else:
                        nc.scalar.activation(dst, src, Act.Gelu, bias=bias,
                                             scale=1.0)
                    return
                shp = list(dst.shape)
                y = sapool.tile(shp, F32, tag="gel_y", name="gel_y")
                if bias is None:
                    nc.scalar.activation(y[:], src, Act.Identity)
                else:
                    nc.scalar.activation(y[:], src, Act.Identity, bias=bias,
                                         scale=1.0)
                u = sapool.tile(shp, F32, tag="gel_u", name="gel_u")
                nc.vector.tensor_tensor(out=u[:], in0=y[:], in1=y[:],
                                        op=AluOp.mult)
                nc.vector.tensor_tensor(out=u[:], in0=u[:], in1=y[:],
                                        op=AluOp.mult)
                nc.vector.tensor_scalar(out=u[:], in0=u[:], scalar1=0.044715,
                                        scalar2=None, op0=AluOp.mult)
                nc.vector.tensor_tensor(out=u[:], in0=u[:], in1=y[:],
                                        op=AluOp.add)
                nc.scalar.activation(u[:], u[:], Act.Tanh, scale=0.7978845608)
                nc.vector.tensor_scalar(out=u[:], in0=u[:], scalar1=1.0,
                                        scalar2=0.5, op0=AluOp.add,
                                        op1=AluOp.mult)
                nc.vector.tensor_tensor(out=dst, in0=u[:], in1=y[:],
                                        op=AluOp.mult)

            # ---------------- constants ----------------
            ident = cpool.tile([128, 128], F16)
            from concourse.masks import make_identity
            make_identity(nc, ident[:])
            ident32 = cpool.tile([128, 128], F32)
            make_identity(nc, ident32[:])
            onesc = cpool.tile([1, 512], F16)   # K=1 matmul lhsT/rhs ones
            nc.vector.memset(onesc[:], 1.0)
            ones_k = cpool.tile([128, 1], F16)  # column-sum matmul rhs
            nc.vector.memset(ones_k[:], 1.0)
            ones15 = cpool.tile([E, 1], F16)
            nc.vector.memset(ones15[:], 1.0)
            cpk = cpool.tile([128, CPK_COLS], F16)
            dma(cpk[:], t_CPK[:])
            c_sel2 = cpk[0:2, SEL2_O:SEL2_O + 128]
            c_Rm = cpk[0:112, RM_O:RM_O + ND].rearrange(
                "p (a b) -> p a b", a=1)
            c_SelJ = cpk[0:112, SELJ_O:SELJ_O + 8]
            c_selB = cpk[0:8, SELB_O:SELB_O + 224]
            c_attLT = cpk[0:5, ATTLT_O:ATTLT_O + 128]
            c_attRT = cpk[0:5, ATTRT_O:ATTRT_O + NH * 128]
            c_Mpool = cpk[:, MPOOL_O:MPOOL_O + 112].rearrange(
                "p (t m) -> p t m", t=2)
            c_Ind2 = cpk[0:8, IND2_O:IND2_O + 256].rearrange(
                "p (t m) -> p t m", t=2)
            c_SegSel0 = cpk[0:ER, SEG_O:SEG_O + E]
            c_RepSel = cpk[0:E, REP_O:REP_O + ER]
            c_maskS = cpk[:, MASKS_O:MASKS_O + ND]
            c_M2 = cpk[:, ATTM2_O:ATTM2_O + 128].rearrange(
                "p (a b) -> p a b", a=1)
            c_M2T2 = cpk[:, ATTM2T_O:ATTM2T_O + 256].rearrange(
                "p (a b) -> p a b", a=2)
            c_fgB = cpk[:, FG_O:FG_O + D]
            c_fbB = cpk[:, FB_O:FB_O + D]
            magic_t = cpool.tile([128, 256], I32)
            nc.vector.memset(magic_t[:], 0x5f3759df)
            dumact = cpool.tile([1, 2], F32)
            nc.vector.memset(dumact[:], 0.0)

            def rsqrt_dve(dst, var_ap, eps):
                """dst = 1/sqrt(var_ap + eps), DVE-only (bit hack + 2 Newton)."""
                shp = list(dst.shape)
                p, n = shp[0], int(np.prod(shp[1:]))
                v = mpool.tile(shp, F32, tag="rsqv", bufs=2, name="rsqv")
                nc.vector.tensor_scalar(out=v[:], in0=var_ap, scalar1=float(eps),
                                        scalar2=None, op0=AluOp.add)
                ti_ = mpool.tile(shp, I32, tag="rsqt", bufs=2, name="rsqt")
                nc.vector.tensor_scalar(out=ti_[:], in0=v[:].bitcast(I32),
                                        scalar1=1, scalar2=None,
                                        op0=AluOp.logical_shift_right)
                mg = magic_t[:p].rearrange("p n -> p n")[:, :n]
                nc.vector.tensor_tensor(out=ti_[:], in0=mg.rearrange(
                    "p (a b) -> p a b", a=1) if len(shp) == 3 else mg,
                    in1=ti_[:], op=AluOp.subtract)
                y = ti_[:].bitcast(F32)
                a_ = mpool.tile(shp, F32, tag="rsqa", bufs=2, name="rsqa")
                for it in range(2):
                    nc.vector.tensor_tensor(out=a_[:], in0=v[:], in1=y,
                                            op=AluOp.mult)
                    nc.vector.tensor_tensor(out=a_[:], in0=a_[:], in1=y,
                                            op=AluOp.mult)
                    nc.vector.tensor_scalar(out=a_[:], in0=a_[:], scalar1=-0.5,
                                            scalar2=1.5, op0=AluOp.mult,
                                            op1=AluOp.add)
                    nc.vector.tensor_tensor(out=dst if it == 1 else
                                            ti_[:].bitcast(F32),
                                            in0=y, in1=a_[:], op=AluOp.mult)

            # ---------------- persistent activations ----------------
            h = hpool.tile([128, 2, D], F32)
            res1 = hpool.tile([128, 2, D], F32)
            dma(h[:], t_x0[:])

            def ln_stats_tg(src, mvs, tg):
                st = mpool.tile([128, 2, 6], F32, tag="lnst", bufs=2)
                xs = src[:, tg, :].rearrange("p (a b) -> p a b", a=2)
                for a in range(2):
                    nc.vector.bn_stats(st[:, a, :], xs[:, a, :])
                nc.vector.bn_aggr(mvs[:, tg, :], st[:])

            def ln_stats(src, eps):
                """emit stats+rsqrt for both tgs; returns (mvs, rst2)."""
                mvs = mpool.tile([128, 2, 2], F32, tag="lnmv", bufs=2)
                for tg in range(2):
                    ln_stats_tg(src, mvs, tg)
                rst2 = mpool.tile([128, 2], F32, tag="lnrs", bufs=2)
                rsqrt_dve(rst2[:], mvs[:, :, 1], eps)
                return mvs, rst2

            def ln_apply(src, dst, stats, per_tg=None):
                mvs, rst2 = stats
                for tg in range(2):
                    nc.vector.tensor_scalar(
                        out=dst[:, tg, :], in0=src[:, tg, :],
                        scalar1=mvs[:, tg, 0:1], scalar2=rst2[:, tg:tg + 1],
                        op0=AluOp.subtract, op1=AluOp.mult)
                    if per_tg is not None:
                        per_tg(tg)

            def layernorm_16(src, dst, eps, per_tg=None):
                ln_apply(src, dst, ln_stats(src, eps), per_tg)

            def transpose6_tg(src16, dst, tg):
                for c in range(KC):
                    pt = psY.tile([128, 128], F16, tag="sm")
                    nc.tensor.transpose(pt[:], src16[:, tg, c * 128:(c + 1) * 128],
                                        ident[:])
                    nc.vector.tensor_copy(dst[:, c, tg * 128:(tg + 1) * 128],
                                          pt[:])

            def transpose6(src16, dst):
                """src16 [128, 2, D] f16 -> dst [128, KC, 256] f16 (feature-major)."""
                for c in range(KC):
                    for tg in range(2):
                        pt = psY.tile([128, 128], F16, tag="sm")
                        nc.tensor.transpose(pt[:], src16[:, tg, c * 128:(c + 1) * 128],
                                            ident[:])
                        nc.vector.tensor_copy(dst[:, c, tg * 128:(tg + 1) * 128],
                                              pt[:])

            # ---------------- layers ----------------
            h_stats = ln_stats(h, 1e-12)
            for i in range(L):
                even = (i % 2 == 0)
                e = i // 2
                _mark(f'L{i}.dma')
                # ---- all weight DMAs for the layer, in consumption order ----
                b_t = bpool.tile([128, B_COLS], F16, tag="bias")
                dma(b_t[:], t_BIA[i, :, :])
                w_qk = wpool.tile([128, WQK_COLS], F16, tag="w", name="w_qk")
                dma(w_qk[:], t_WQK[i, :, :])
                w_vo = wpool.tile([128, WVO_COLS], F16, tag="w", name="w_vo")
                dma(w_vo[:], t_WVO[i, :, :])
                w_i1 = wpool.tile([128, WI_COLS], F16, tag="w", name="w_i1")
                dma(w_i1[:], t_WI1[i, :, :])
                w_i2 = wpool.tile([128, WI_COLS], F16, tag="w", name="w_i2")
                dma(w_i2[:], t_WI2[i, :, :])
                w_o1 = wpool.tile([128, WO_COLS], F16, tag="w", name="w_o1")
                dma(w_o1[:], t_WO1[i, :, :])
                w_o2 = wpool.tile([128, WO_COLS], F16, tag="w", name="w_o2")
                dma(w_o2[:], t_WO2[i, :, :])

                _mark(f'L{i}.ln1')
                # LN1 -> n1 (f16) -> n1T (stats precomputed at end of prev layer)
                n1 = apool.tile([128, 2, D], F16, tag="n1")
                n1T = apool.tile([128, KC, 256], F16, tag="n1T")
                ln_apply(h, n1, h_stats,
                         per_tg=lambda tg: transpose6_tg(n1, n1T, tg))
                # hoist Exp act-table load off the attention critical path
                if not sim_gelu:
                    nc.scalar.activation(dumact[:, 0:1], dumact[:, 0:1],
                                         Act.Exp)

                _mark(f'L{i}.qk')
                # QK^T (transposed out; bias via Act-Identity biased drains)
                qkT = apool.tile([128, 12, 256], F16, tag="bigact")
                for g in range(3):
                    grps = [psG.tile([128, 2, 512], F32, tag="grp",
                                     name=f"qkg{g}{hf}") for hf in range(2)]
                    for c in range(KC):
                        for j in range(4):
                            nc.tensor.matmul(grps[j // 2][:, j % 2, :256],
                                             w_qk[:, (g * 6 + c) * 512 + j * 128:
                                                  (g * 6 + c) * 512 + (j + 1) * 128],
                                             n1T[:, c, :],
                                             start=(c == 0), stop=(c == KC - 1))
                    for hf in range(2):
                        for sub in range(2):
                            fc = g * 4 + 2 * hf + sub
                            nc.scalar.activation(
                                qkT[:, fc, :], grps[hf][:, sub, :256],
                                Act.Identity, bias=b_t[:, BQK + fc:BQK + fc + 1],
                                scale=1.0)

                _mark(f'L{i}.v')
                # V (untransposed: [tok, dv]); bias via K=1 matmul
                V = apool.tile([128, 2, D], F16, tag="V")
                grpv = [psG.tile([128, 2, 512], F32, tag="grp",
                                 name=f"vg{tg}") for tg in range(2)]
                for j in range(4):
                    sl = slice((j % 2) * 384, (j % 2 + 1) * 384)
                    nc.tensor.matmul(grpv[j // 2][:, j % 2, :384],
                                     onesc[:1, :128],
                                     b_t[0:1, BV + sl.start:BV + sl.stop],
                                     start=True, stop=False)
                for c in range(KC):
                    for j in range(4):
                        tg, n = j // 2, j % 2
                        sl = slice(n * 384, (n + 1) * 384)
                        nc.tensor.matmul(grpv[tg][:, n, :384],
                                         n1T[:, c, tg * 128:(tg + 1) * 128],
                                         w_vo[:, WVO_V + c * 768 + sl.start:
                                              WVO_V + c * 768 + sl.stop],
                                         start=False, stop=(c == KC - 1))
                for tg in range(2):
                    nc.vector.tensor_copy(
                        V[:, tg, :].rearrange("p (n d) -> p n d", n=2),
                        grpv[tg][:, :, :384])

                _mark(f'L{i}.attn')
                # attention — scoresT[k, q] layout (keys restricted to own tg),
                # no transposes: V [tok, dv] is directly the o-matmul lhsT.
                oT = apool.tile([128, KC, 256], F16, tag="oT")
                for tg in range(2):
                    tgs = slice(tg * 128, (tg + 1) * 128)
                    expT = sapool.tile([128, NH, 128], F16, tag="expT")
                    for fc in range(KC):
                        pssc = psY.tile([128, 256], F32, tag="sm")
                        for pp in range(2):
                            hd = 2 * fc + pp
                            off = pp * 64
                            ps_h = slice(pp * 128, (pp + 1) * 128)
                            nc.tensor.matmul(pssc[:, ps_h], c_attLT[:],
                                             c_attRT[:, hd * 128:(hd + 1) * 128],
                                             start=True, stop=False)
                            nc.tensor.matmul(pssc[:, ps_h],
                                             qkT[off:off + 64, 6 + fc, tgs],
                                             qkT[off:off + 64, fc, tgs],
                                             start=False, stop=True)
                        nc.scalar.activation(
                            expT[:, 2 * fc:2 * fc + 2, :], pssc[:], Act.Exp)
                    # per-(q, head) softmax denominators via PE column sums
                    ps_rs = psY.tile([128, NH], F32, tag="sm")
                    for hd in range(NH):
                        nc.tensor.matmul(ps_rs[:, hd:hd + 1],
                                         expT[:, hd, :],
                                         ones_k[:], start=True, stop=True)
                    rinv = mpool.tile([128, NH], F32, tag="rinv", bufs=2)
                    nc.vector.reciprocal(rinv[:], ps_rs[:])
                    rinv16 = mpool.tile([128, NH], F16, tag="rinv16", bufs=2)
                    nc.vector.tensor_copy(rinv16[:], rinv[:])
                    for c in range(KC):
                        # psn[p, q] = rinv16[q, 2c + p//64] via stride-0 lhsT
                        psn = psY.tile([128, 128], F32, tag="sm")
                        for hh in range(2):
                            lhsT = rinv16[:, 2 * c + hh:2 * c + hh + 1] \
                                .to_broadcast((128, 1, 64))[:, 0, :]
                            nc.tensor.matmul(psn[hh * 64:(hh + 1) * 64, :],
                                             lhsT, ident[:],
                                             start=True, stop=True)
                        psnS = sapool.tile([128, 128], F16, tag="psnS")
                        nc.vector.tensor_copy(psnS[:], psn[:])
                        pso = psY.tile([128, 128], F32, tag="sm")
                        for hh in range(2):
                            hd = 2 * c + hh
                            nc.tensor.matmul(pso[hh * 64:(hh + 1) * 64, :],
                                             V[:, tg, hd * 64:(hd + 1) * 64],
                                             expT[:, hd, :],
                                             start=True, stop=True)
                        nc.vector.tensor_tensor(out=oT[:, c, tgs],
                                                in0=pso[:], in1=psnS[:],
                                                op=AluOp.mult)
                # hoist Gelu act-table load off the FFN critical path
                if not sim_gelu:
                    nc.scalar.activation(dumact[:, 1:2], dumact[:, 1:2],
                                         Act.Gelu)

                _mark(f'L{i}.ao')
                # AO projection + residual
                if even:
                    attnH = apool.tile([128, 2, D], F16, tag="n1")
                grpa = [psG.tile([128, 2, 512], F32, tag="grp",
                                 name=f"aog{tg}") for tg in range(2)]
                for j in range(4):
                    sl = slice((j % 2) * 384, (j % 2 + 1) * 384)
                    nc.tensor.matmul(grpa[j // 2][:, j % 2, :384],
                                     onesc[:1, :128],
                                     b_t[0:1, BAO + sl.start:BAO + sl.stop],
                                     start=True, stop=False)
                for c in range(KC):
                    for j in range(4):
                        tg, n = j // 2, j % 2
                        sl = slice(n * 384, (n + 1) * 384)
                        nc.tensor.matmul(grpa[tg][:, n, :384],
                                         oT[:, c, tg * 128:(tg + 1) * 128],
                                         w_vo[:, WVO_AO + c * 768 + sl.start:
                                              WVO_AO + c * 768 + sl.stop],
                                         start=False, stop=(c == KC - 1))
                mvs2 = mpool.tile([128, 2, 2], F32, tag="lnmv", bufs=2)
                for tg in range(2):
                    if even:
                        nc.vector.tensor_copy(
                            attnH[:, tg, :].rearrange("p (n d) -> p n d", n=2),
                            grpa[tg][:, :, :384])
                    nc.vector.tensor_tensor(
                        out=res1[:, tg, :].rearrange("p (n d) -> p n d", n=2),
                        in0=grpa[tg][:, :, :384],
                        in1=h[:, tg, :].rearrange("p (n d) -> p n d", n=2),
                        op=AluOp.add)
                    ln_stats_tg(res1, mvs2, tg)
                rst2b = mpool.tile([128, 2], F32, tag="lnrs", bufs=2)
                rsqrt_dve(rst2b[:], mvs2[:, :, 1], 1e-12)

                if even:
                    _mark(f'L{i}.moe_pool')
                    # single merged MoE pack DMA (weights for the whole tail);
                    # issued before the pooling matmuls to cover its latency
                    moepk = wmpool.tile([128, M1_COLS], F16, tag="moepk")
                    dma(moepk[:], t_MOE[e, :, :])
                    # prefetch the first two classifier weight blocks so the
                    # first emit_cls_dd calls during FFN-up don't stall
                    cw1_tiles = {}

                    def issue_cw1(dd):
                        w_ch = wcpool.tile([128, 2, KC, D2], F16, tag="wc1",
                                           name="wc1_c")
                        dma(w_ch[:], t_CW1[e, dd, :, :].rearrange(
                            "p (h c z) -> p h c z", h=2, c=KC))
                        cw1_tiles[dd] = w_ch

                    issue_cw1(0)
                    issue_cw1(1)
                    # pooled^T [128, KC, 112] (cols (d, tg*4+bl) after scatter)
                    pooledT = apool.tile([128, KC, 112], F16, tag="pooledT")
                    pview = pooledT.rearrange("p c (d g) -> p c d g", g=8)
                    for c in range(KC):
                        for tg in range(2):
                            ps = psY.tile([128, 4 * ND], F32, tag="sm")
                            nc.tensor.matmul(ps[:],
                                             attnH[:, tg, c * 128:(c + 1) * 128],
                                             c_Mpool[:, tg, :],
                                             start=True, stop=True)
                            pv = ps[:].rearrange("p (d g) -> p d g", g=4)
                            nc.vector.tensor_copy(
                                pview[:, c, :, tg * 4:tg * 4 + 4], pv)
                    c_cg = moepk[0:112, CLS_O + 0:CLS_O + D2]
                    c_cbt = moepk[0:112, CLS_O + D2:CLS_O + 2 * D2]
                    c_w2 = moepk[0:112, CLS_O + 2 * D2:CLS_O + 3 * D2]
                    c_b1 = moepk[0:112, CLS_O + 3 * D2:CLS_O + 4 * D2]
                    c_c2 = moepk[0:112, CB2_O:CB2_O + 1]
                    w_au = moepk[:, AU_O:AU_O + KC * ER].rearrange(
                        "p (c r) -> p c r", r=ER)
                    b_lup = moepk[0:1, LUP_O:LUP_O + ER]
                    w_ad = moepk[:, AD_O:AD_O + HC * ER].rearrange(
                        "p (c r) -> p c r", r=ER)
                    w_cb = moepk[0:ER, CB_O:CB_O + ER]
                    w_bdd = moepk[:, BDD_O:BDD_O + KC * ER].rearrange(
                        "p (c r) -> p c r", r=ER)
                    w_gram = moepk[0:ER, GR_O:GR_O + ER]
                    w_bdm = moepk[0:ER, BDM_O:BDM_O + E]
                    w_bdf = moepk[0:ER, BDF_O:BDF_O + D]
                    psz = psY.tile([112, D2], F32, tag="zacc", bufs=1)
                    cls_state = {}

                    def emit_cls_dd(dd):
                        w_ch = cw1_tiles.pop(dd)
                        if dd + 2 <= 6:
                            issue_cw1(dd + 2)
                        for dh in range(2):
                            d = 2 * dd + dh
                            psd_ = psY.tile([8, D2], F32, tag="sm")
                            for c in range(KC):
                                nc.tensor.matmul(psd_[:],
                                                 pooledT[:, c, d * 8:(d + 1) * 8],
                                                 w_ch[:, dh, c, :],
                                                 start=(c == 0), stop=(c == KC - 1))
                            zd = sapool.tile([8, D2], F16, tag="zd")
                            nc.vector.tensor_copy(zd[:], psd_[:])
                            nc.tensor.matmul(psz[:],
                                             c_selB[:, 104 - 8 * d:216 - 8 * d],
                                             zd[:], start=(d == 0),
                                             stop=(d == ND - 1))

                def emit_cls_finish():
                    zsb = spool.tile([112, D2], F32, tag="zsb")
                    nc.vector.tensor_tensor(out=zsb[:], in0=psz[:], in1=c_b1[:],
                                            op=AluOp.add)
                    zst = mpool.tile([112, 6], F32, tag="lnstz")
                    nc.vector.bn_stats(zst[:], zsb[:])
                    zmv = mpool.tile([112, 2], F32, tag="lnmvz")
                    nc.vector.bn_aggr(zmv[:], zst[:])
                    zrstd = mpool.tile([112, 1], F32, tag="zrstd")
                    rsqrt_dve(zrstd[:], zmv[:, 1:2], 1e-5)
                    zn = spool.tile([112, D2], F32, tag="zn")
                    nc.vector.tensor_scalar(out=zn[:], in0=zsb[:],
                                            scalar1=zmv[:, 0:1], scalar2=zrstd[:],
                                            op0=AluOp.subtract, op1=AluOp.mult)
                    nc.vector.tensor_tensor(out=zn[:], in0=zn[:], in1=c_cg[:],
                                            op=AluOp.mult)
                    nc.vector.tensor_tensor(out=zn[:], in0=zn[:], in1=c_cbt[:],
                                            op=AluOp.add)
                    zg = spool.tile([112, D2], F32, tag="zg")
                    act_gelu(zg[:], zn[:])
                    nc.vector.tensor_tensor(out=zg[:], in0=zg[:], in1=c_w2[:],
                                            op=AluOp.mult)
                    ppre = mpool.tile([112, 1], F32, tag="ppre")
                    nc.vector.reduce_sum(ppre[:], zg[:], axis=mybir.AxisListType.X)
                    nc.vector.tensor_tensor(out=ppre[:], in0=ppre[:], in1=c_c2[:],
                                            op=AluOp.add)
                    rp = mpool.tile([112, 1, ND], F16, tag="rp")
                    nc.vector.tensor_tensor(out=rp[:], in0=c_Rm[:],
                                            in1=ppre[:].to_broadcast((112, 1, ND)),
                                            op=AluOp.mult)
                    psda = psY.tile([8, ND], F32, tag="sm")
                    nc.tensor.matmul(psda[:], c_SelJ[:], rp[:, 0, :],
                                     start=True, stop=True)
                    da = mpool.tile([8, ND], F16, tag="da")
                    nc.vector.tensor_scalar(out=da[:], in0=psda[:], scalar1=0.0,
                                            scalar2=None, op0=AluOp.is_gt)

                    _mark(f'L{i}.moe_rout')
                    # routing weights w [128, tg, E] f32
                    w_rt = spool.tile([128, 2, E], F32, tag="wrt")
                    nact = mpool.tile([128, 2], F32, tag="nact")
                    for tg in range(2):
                        psd = psY.tile([128, ND], F32, tag="sm")
                        nc.tensor.matmul(psd[:], c_Ind2[:, tg, :], da[:],
                                         start=True, stop=True)
                        nc.vector.tensor_tensor(out=w_rt[:, tg, 0:ND], in0=psd[:],
                                                in1=c_maskS[:], op=AluOp.mult)
                        nc.vector.reduce_sum(nact[:, tg:tg + 1], w_rt[:, tg, 0:ND],
                                             axis=mybir.AxisListType.X)
                        nc.vector.tensor_scalar(out=nact[:, tg:tg + 1],
                                                in0=nact[:, tg:tg + 1],
                                                scalar1=1.0, scalar2=None,
                                                op0=AluOp.add)
                    rnact = mpool.tile([128, 2], F32, tag="rnact")
                    nc.vector.reciprocal(rnact[:], nact[:])
                    for tg in range(2):
                        nc.vector.tensor_scalar(out=w_rt[:, tg, 0:ND],
                                                in0=w_rt[:, tg, 0:ND],
                                                scalar1=rnact[:, tg:tg + 1],
                                                scalar2=None, op0=AluOp.mult)
                        nc.vector.tensor_copy(w_rt[:, tg, ND:E], rnact[:, tg:tg + 1])
                    wT = mpool.tile([E, 256], F32, tag="wT")
                    for tg in range(2):
                        pt = psY.tile([E, 128], F32, tag="sm")
                        nc.tensor.transpose(pt[:], w_rt[:, tg, :], ident32[:])
                        nc.vector.tensor_copy(wT[:, tg * 128:(tg + 1) * 128], pt[:])
                    cls_state['wT'] = wT

                _mark(f'L{i}.ln2')
                # LN2 -> n2 -> n2T (stats computed during AO drains)
                n2 = apool.tile([128, 2, D], F16, tag="n2")
                n2T = apool.tile([128, KC, 256], F16, tag="n2T")
                ln_apply(res1, n2, (mvs2, rst2b),
                         per_tg=lambda tg: transpose6_tg(n2, n2T, tg))

                _mark(f'L{i}.up')
                # FFN up (transposed out) + gelu with folded bias
                interT = apool.tile([128, HC, 256], F16, tag="bigact")
                for g in range(6):
                    w_i = w_i1 if g < 3 else w_i2
                    gg = g % 3
                    grps = [psG.tile([128, 2, 512], F32, tag="grp",
                                     name=f"upg{g}{hf}") for hf in range(2)]
                    for c in range(KC):
                        for j in range(4):
                            nc.tensor.matmul(grps[j // 2][:, j % 2, :256],
                                             w_i[:, (gg * 6 + c) * 512 + j * 128:
                                                 (gg * 6 + c) * 512 + (j + 1) * 128],
                                             n2T[:, c, :],
                                             start=(c == 0), stop=(c == KC - 1))
                    for hf in range(2):
                        for sub in range(2):
                            hc = g * 4 + 2 * hf + sub
                            act_gelu(interT[:, hc, :], grps[hf][:, sub, :256],
                                     bias=b_t[:, BI + hc:BI + hc + 1])
                    if even:
                        _mark(f'L{i}.moe_cls')
                        if g < 5:
                            emit_cls_dd(g)
                        else:
                            emit_cls_dd(5)
                            emit_cls_dd(6)
                        _mark(f'L{i}.up')

                if even:
                    _mark(f'L{i}.moe_lora')
                    # LoRA rails
                    ps = psY.tile([ER, 256], F32, tag="sm")
                    nc.tensor.matmul(ps[:], b_lup, onesc[:1, :256],
                                     start=True, stop=False)
                    for c in range(KC):
                        nc.tensor.matmul(ps[:], w_au[:, c, :], n2T[:, c, :],
                                         start=False, stop=(c == KC - 1))
                    lup_rT = spool.tile([ER, 256], F16, tag="luprT")
                    nc.vector.tensor_copy(lup_rT[:], ps[:])

                    ps2 = psY.tile([ER, 256], F32, tag="sm")
                    nc.tensor.matmul(ps2[:], w_cb, lup_rT[:], start=True, stop=False)
                    for c in range(HC):
                        nc.tensor.matmul(ps2[:], w_ad[:, c, :], interT[:, c, :],
                                         start=False, stop=(c == HC - 1))
                    ldr16 = spool.tile([ER, 256], F16, tag="ldr16")
                    nc.vector.tensor_copy(ldr16[:], ps2[:])
                    ldr32 = spool.tile([ER, 256], F32, tag="ldr32")
                    nc.vector.tensor_copy(ldr32[:], ps2[:])

                _mark(f'L{i}.down')
                # FFN down
                if even:
                    base = apool.tile([128, 2, D], F16, tag="base")
                grpd = [psG.tile([128, 2, 512], F32, tag="grp",
                                 name=f"dng{tg}") for tg in range(2)]
                for j in range(4):
                    sl = slice((j % 2) * 384, (j % 2 + 1) * 384)
                    nc.tensor.matmul(grpd[j // 2][:, j % 2, :384],
                                     onesc[:1, :128],
                                     b_t[0:1, BO + sl.start:BO + sl.stop],
                                     start=True, stop=False)
                for c in range(HC):
                    w_o = w_o1 if c < 12 else w_o2
                    cc = c % 12
                    for j in range(4):
                        tg, n = j // 2, j % 2
                        sl = slice(n * 384, (n + 1) * 384)
                        nc.tensor.matmul(grpd[tg][:, n, :384],
                                         interT[:, c, tg * 128:(tg + 1) * 128],
                                         w_o[:, cc * 768 + sl.start:
                                             cc * 768 + sl.stop],
                                         start=False, stop=(c == HC - 1))
                if even:
                    _mark(f'L{i}.moe_cls2')
                    emit_cls_finish()
                for tg in range(2):
                    if not even:
                        nc.vector.tensor_tensor(
                            out=h[:, tg, :].rearrange("p (n d) -> p n d", n=2),
                            in0=grpd[tg][:, :, :384],
                            in1=res1[:, tg, :].rearrange("p (n d) -> p n d", n=2),
                            op=AluOp.add)
                    else:
                        nc.vector.tensor_copy(
                            base[:, tg, :].rearrange("p (n d) -> p n d", n=2),
                            grpd[tg][:, :, :384])

                if not even:
                    h_stats = ln_stats(h, 1e-12)
                    continue

                _mark(f'L{i}.moe_stats')
                # ================= MoE / classifier tail =================
                # base stats (mu, ms = var + mu^2), transposed to rows
                mums = mpool.tile([128, 2, 2], F32, tag="mums")   # [:, tg, (mu,ms)]
                for tg in range(2):
                    st = mpool.tile([128, 3, 6], F32, tag="lnst", bufs=2)
                    xs = base[:, tg, :].rearrange("p (a b) -> p a b", a=3)
                    for a in range(3):
                        nc.vector.bn_stats(st[:, a, :], xs[:, a, :])
                    mv = mpool.tile([128, 2], F32, tag="lnmv", bufs=2)
                    nc.vector.bn_aggr(mv[:], st[:])
                    nc.vector.tensor_copy(mums[:, tg, 0:1], mv[:, 0:1])
                    # ms = var + mu^2
                    musq = mpool.tile([128, 1], F32, tag="musq")
                    nc.vector.tensor_tensor(out=musq[:], in0=mv[:, 0:1],
                                            in1=mv[:, 0:1], op=AluOp.mult)
                    nc.vector.tensor_tensor(out=mums[:, tg, 1:2], in0=mv[:, 1:2],
                                            in1=musq[:], op=AluOp.add)
                muT = mpool.tile([1, 256], F32, tag="muT")
                msT = mpool.tile([1, 256], F32, tag="msT")
                for tg in range(2):
                    pt = psY.tile([1, 128], F32, tag="sm")
                    nc.tensor.transpose(pt[:], mums[:, tg, 0:1], ident32[:])
                    nc.vector.tensor_copy(muT[:, tg * 128:(tg + 1) * 128], pt[:])
                    pt2 = psY.tile([1, 128], F32, tag="sm")
                    nc.tensor.transpose(pt2[:], mums[:, tg, 1:2], ident32[:])
                    nc.vector.tensor_copy(msT[:, tg * 128:(tg + 1) * 128], pt2[:])

                baseT = apool.tile([128, KC, 256], F16, tag="n1T")
                transpose6(base, baseT)

                # (cls finish + routing emitted during down via emit_cls_finish)
                _mark(f'L{i}.moe_g')
                # G^T (cross term, x2 folded in BdfD) and quad term
                psg = psY.tile([ER, 256], F32, tag="sm")
                for c in range(KC):
                    nc.tensor.matmul(psg[:], w_bdd[:, c, :], baseT[:, c, :],
                                     start=(c == 0), stop=(c == KC - 1))
                Pcross = spool.tile([ER, 256], F16, tag="pcross")
                nc.vector.tensor_tensor(out=Pcross[:], in0=psg[:],
                                        in1=ldr32[:], op=AluOp.mult)

                psq = psY.tile([ER, 256], F32, tag="sm")
                nc.tensor.matmul(psq[:], w_gram, ldr16[:],
                                 start=True, stop=True)
                Pquad = spool.tile([ER, 256], F16, tag="pquad")
                nc.vector.tensor_tensor(out=Pquad[:], in0=psq[:], in1=ldr32[:],
                                        op=AluOp.mult)

                # mu_e^T [E, 256]
                muT16 = mpool.tile([1, 256], F16, tag="muT16")
                nc.vector.tensor_copy(muT16[:], muT[:])
                msT16 = mpool.tile([1, 256], F16, tag="msT16")
                nc.vector.tensor_copy(msT16[:], msT[:])
                psmu = psY.tile([E, 256], F32, tag="sm")
                nc.tensor.matmul(psmu[:], w_bdm, ldr16[:], start=True, stop=False)
                nc.tensor.matmul(psmu[:], onesc[:1, :E], muT16[:],
                                 start=False, stop=True)
                muE = mpool.tile([E, 256], F32, tag="muE")
                nc.vector.tensor_copy(muE[:], psmu[:])

                # ms^T then var, rho
                psms = psY.tile([E, 256], F32, tag="sm")
                nc.tensor.matmul(psms[:], c_SegSel0, Pcross[:],
                                 start=True, stop=False)
                nc.tensor.matmul(psms[:], c_SegSel0, Pquad[:],
                                 start=False, stop=False)
                nc.tensor.matmul(psms[:], onesc[:1, :E], msT16[:],
                                 start=False, stop=True)
                musqE = mpool.tile([E, 256], F32, tag="musqE")
                nc.vector.tensor_tensor(out=musqE[:], in0=muE[:], in1=muE[:],
                                        op=AluOp.mult)
                varE = mpool.tile([E, 256], F32, tag="varE")
                nc.vector.tensor_tensor(out=varE[:], in0=psms[:], in1=musqE[:],
                                        op=AluOp.subtract)
                rho = mpool.tile([E, 256], F32, tag="rho")
                rsqrt_dve(rho[:], varE[:], 1e-5)

                # s_e = w * rho ; pack [sE | sE*muE] -> column sums -> scal/off
                packSO = mpool.tile([E, 512], F16, tag="packSO")
                wT = cls_state['wT']
                nc.vector.tensor_tensor(out=packSO[:, 0:256], in0=wT[:], in1=rho[:],
                                        op=AluOp.mult)
                nc.vector.tensor_tensor(out=packSO[:, 256:512],
                                        in0=packSO[:, 0:256], in1=muE[:],
                                        op=AluOp.mult)
                psso = psY.tile([1, 512], F32, tag="sm")
                nc.tensor.matmul(psso[:], ones15[:], packSO[:],
                                 start=True, stop=True)
                soT = mpool.tile([1, 512], F32, tag="soT")
                nc.vector.tensor_copy(soT[:], psso[:])
                scal = mpool.tile([128, 2], F32, tag="scal")
                off = mpool.tile([128, 2], F32, tag="off")
                for tg in range(2):
                    pt = psY.tile([128, 1], F32, tag="sm")
                    nc.tensor.transpose(pt[:], soT[:, tg * 128:(tg + 1) * 128],
                                        ident32[:1, :1])
                    nc.vector.tensor_copy(scal[:, tg:tg + 1], pt[:])
                    pt2 = psY.tile([128, 1], F32, tag="sm")
                    nc.tensor.transpose(pt2[:],
                                        soT[:, 256 + tg * 128:256 + (tg + 1) * 128],
                                        ident32[:1, :1])
                    nc.vector.tensor_copy(off[:, tg:tg + 1], pt2[:])

                # ls^T = ldown_r^T * repeat(s_e)
                psrep = psY.tile([ER, 256], F32, tag="sm")
                nc.tensor.matmul(psrep[:], c_RepSel, packSO[:, 0:256],
                                 start=True, stop=True)
                srep = mpool.tile([ER, 256], F32, tag="srep")
                nc.vector.tensor_copy(srep[:], psrep[:])
                lsT = spool.tile([ER, 256], F16, tag="lsT")
                nc.vector.tensor_tensor(out=lsT[:], in0=srep[:], in1=ldr32[:],
                                        op=AluOp.mult)

                _mark(f'L{i}.moe_fin')
                # final: h = (res1 - off) + (base*scal + ldown_mix)
                grpf = [psG.tile([128, 2, 512], F32, tag="grp",
                                 name=f"fing{tg}") for tg in range(2)]
                for j in range(4):
                    tg, n = j // 2, j % 2
                    sl = slice(n * 384, (n + 1) * 384)
                    nc.tensor.matmul(grpf[tg][:, n, :384],
                                     lsT[:, tg * 128:(tg + 1) * 128],
                                     w_bdf[:, sl], start=True, stop=True)
                for j in range(4):
                    tg, n = j // 2, j % 2
                    sl = slice(n * 384, (n + 1) * 384)
                    tmp = spool.tile([128, 384], F32, tag="ffn_tmp")
                    nc.vector.scalar_tensor_tensor(
                        out=tmp[:], in0=base[:, tg, sl],
                        scalar=scal[:, tg:tg + 1],
                        in1=grpf[tg][:, n, :384], op0=AluOp.mult, op1=AluOp.add)
                    nc.vector.scalar_tensor_tensor(
                        out=h[:, tg, sl], in0=res1[:, tg, sl],
                        scalar=off[:, tg:tg + 1], in1=tmp[:],
                        op0=AluOp.subtract, op1=AluOp.add)
                h_stats = ln_stats(h, 1e-12)

            _mark('final_ln')
            # ---------------- final LN ----------------
            hf = apool.tile([128, 2, D], F32, tag="base")
            ln_apply(h, hf, h_stats)   # writes f32 since tile dtype f32
            ot = apool.tile([128, 2, D], F32, tag="bigact",
                            name="ot")
            for tg in range(2):
                nc.vector.tensor_tensor(out=ot[:, tg, :], in0=hf[:, tg, :],
                                        in1=c_fgB, op=AluOp.mult)
                nc.vector.tensor_tensor(out=ot[:, tg, :], in0=ot[:, tg, :],
                                        in1=c_fbB, op=AluOp.add)
            dma(t_out[:], ot[:])

    nc.compile()
    return nc


_CACHE = {}


def _get_nc(sim_gelu=False):
    key = ("nc", sim_gelu)
    if key not in _CACHE:
        _CACHE[key] = _build(sim_gelu)
    return _CACHE[key]


def kernel(**inputs):
    inputs = {k: np.asarray(v) for k, v in inputs.items()}
    P = _prep(inputs)
    shards = _shard_x0(inputs)
    nc = _get_nc()
    base_map = {k: np.ascontiguousarray(v) for k, v in P.items()}
    in_maps = []
    for c in range(NC):
        m = dict(base_map)
        m["x0"] = np.ascontiguousarray(shards[c])
        in_maps.append(m)
    res = bass_utils.run_bass_kernel_spmd(nc, in_maps, core_ids=list(range(NC)))
    out = np.zeros((B, S, D), f32)
    for c in range(NC):
        oc = res.results[c]["out"].transpose(1, 0, 2).reshape(NT, D)
        for bl in range(BPC):
            out[c * BPC + bl] = oc[bl * TS: bl * TS + S]
    return out
